# revision 21
# baseline (speedup 1.0000x reference)
"""EntNetHead Trainium2 kernel (v2: deferred normalization).

Data-parallel over batch B=64 across 8 NeuronCores (8 batch rows per core);
T=256 recurrent steps run on-chip per core.

Key idea vs v1: the per-step L2 normalization is linear, so the state is
kept UNNORMALIZED (hu) together with rn = 1/||hu|| per packed row.  The
matmuls use huT directly; rn folds into (a) the sigmoid's per-partition
scale, (b) one DVE scale of the z PSUM, (c) a lazily-materialized
h = rn*hu used by the state update.  The whole square/fold/rsqrt chain
runs OFF the critical path (ACT/Pool/PE idle slots).

Per-step PE block is 12 matmuls: the g-gram (s_t . hu) and q-gram
(q_{t-1} . hu) are fused into the z matmuls as 16 extra rhs columns,
staged next to the U chunks by the (otherwise idle) DMA engines.  The
gate's content bias s_t.keys is precomputed for all t as a per-partition
column (sKall) and enters through the sigmoid's bias AP. The z bias
keyV + s_t@W accumulates in a separate PSUM bank off the critical path.

Engine split per step:
  PE:   12 fused z/g/q matmuls, 4 bias matmuls (t+1), 3 transposes,
        1 fp16 norm-fold matmul
  DVE:  gate reduce, z-scale, z-add(bias), hu update, hT copy,
        q reduce, rsqrt seed
  ACT:  sigmoid, prelu, square+accum, bias PSUM->SBUF copy
  Pool: Newton rsqrt, h materialize, score scale, ss fp16 cast
  DMA:  per-step staging of sT/qT slices into the fused rhs buffer
"""

import sys

sys.path.insert(0, "/opt/trn_rl_repo")

from contextlib import ExitStack

import numpy as np

import concourse.bacc as bacc
import concourse.bass as bass
import concourse.tile as tile
from concourse import mybir
from concourse.bass_utils import run_bass_kernel_spmd

F32 = mybir.dt.float32
F16 = mybir.dt.float16
I32 = mybir.dt.int32
ALU = mybir.AluOpType
ACTF = mybir.ActivationFunctionType

T, B, H, K, L = 256, 64, 768, 5, 3
NC = 8
BL = B // NC          # 8 batch rows per core
R = K * BL            # 40 (k,b) rows
RP = 64               # padded stationary width
HC = H // 128         # 6 contraction chunks
HH = H // 2           # 384
BLK = 2 * (HH + 16)   # 800: per-chunk fused rhs block [U_h0|sT|qT|U_h1|sT|qT]
FW = HH + 16          # 400: fused matmul width per half


def _host_consts():
    selK = np.zeros((K, RP), np.float16)
    for k in range(K):
        selK[k, k * BL:(k + 1) * BL] = 1.0
    selKb = np.zeros((K, 8 * RP), np.float16)
    for b in range(BL):
        for k in range(K):
            selKb[k, b * RP + k * BL + b] = 1.0
    selB = np.zeros((128, 16 * RP), np.float16)
    for m in range(16):
        for b in range(BL):
            for k in range(K):
                selB[m * BL + b, m * RP + k * BL + b] = 1.0
    maskG = np.zeros((128, BL), np.float16)
    for p in range(128):
        maskG[p, p % BL] = 1.0
    I64 = np.zeros((128, RP), np.float16)
    for j in range(RP):
        I64[j, j] = 1.0
        I64[64 + j, j] = 1.0
    I128 = np.eye(128, dtype=np.float32)
    mask24 = np.zeros((R, BL * L), np.float32)
    for k in range(K):
        for b in range(BL):
            mask24[k * BL + b, b * L:(b + 1) * L] = 1.0
    ones1x128 = np.ones((1, 128), np.float32)
    P64h = np.zeros((128, 128), np.float16)
    for i in range(128):
        P64h[i, i % 64] = 1.0
        P64h[i, i % 64 + 64] = 1.0
    selK32 = selK.astype(np.float32)
    return {
        "c_selK": selK, "c_selKb": selKb, "c_selB32": selB, "c_maskG": maskG,
        "c_I64": I64, "c_I128": I128, "c_selK32": selK32,
        "c_mask24": mask24, "c_ones": ones1x128, "c_P64h": P64h,
        "c_I128h": np.eye(128, dtype=np.float16),
    }


def _build(nsteps, debug=False, use_prelu=True,
           stage_dma=True, use_pool=True, use_ttr=False):
    # use_ttr: InstTensorTensorReduce wedges TRN2 here (NRT_EXEC_UNIT_
    # UNRECOVERABLE) despite passing CoreSim -- keep the 2-op fallback.
    nc = bacc.Bacc("TRN2", target_bir_lowering=False, debug=False)
    NR = nsteps * BL      # feature rows per core
    ntt = (NR + 127) // 128

    d_fs = nc.dram_tensor("features_sentence", [nsteps, BL, H], F32, kind="ExternalInput")
    d_fe = nc.dram_tensor("features_entity", [nsteps, BL, H], F32, kind="ExternalInput")
    d_keys = nc.dram_tensor("keys", [K, H], F32, kind="ExternalInput")
    d_U = nc.dram_tensor("U", [H, H], F32, kind="ExternalInput")
    d_V = nc.dram_tensor("V", [H, H], F32, kind="ExternalInput")
    d_W = nc.dram_tensor("W", [H, H], F32, kind="ExternalInput")
    d_alpha = nc.dram_tensor("alpha", [1], F32, kind="ExternalInput")
    d_Wout = nc.dram_tensor("W_out", [K, L], F32, kind="ExternalInput")
    d_bout = nc.dram_tensor("b_out", [L], F32, kind="ExternalInput")
    d_selK = nc.dram_tensor("c_selK", [K, RP], F16, kind="ExternalInput")
    d_selKb = nc.dram_tensor("c_selKb", [K, 8 * RP], F16, kind="ExternalInput")
    d_selB32 = nc.dram_tensor("c_selB32", [128, 16 * RP], F16, kind="ExternalInput")
    d_maskG = nc.dram_tensor("c_maskG", [128, BL], F16, kind="ExternalInput")
    d_I64 = nc.dram_tensor("c_I64", [128, RP], F16, kind="ExternalInput")
    d_I128 = nc.dram_tensor("c_I128", [128, 128], F32, kind="ExternalInput")
    d_selK32 = nc.dram_tensor("c_selK32", [K, RP], F32, kind="ExternalInput")
    d_mask24 = nc.dram_tensor("c_mask24", [R, BL * L], F32, kind="ExternalInput")
    d_ones = nc.dram_tensor("c_ones", [1, 128], F32, kind="ExternalInput")
    d_P64h = nc.dram_tensor("c_P64h", [128, 128], F16, kind="ExternalInput")
    d_I128h = nc.dram_tensor("c_I128h", [128, 128], F16, kind="ExternalInput")
    d_out = nc.dram_tensor("preds", [NR, L], F32, kind="ExternalOutput")
    if debug:
        d_dhu = nc.dram_tensor("dbg_hu", [nsteps + 1, 128, HH], F16, kind="ExternalOutput")
        d_drn = nc.dram_tensor("dbg_rn", [nsteps + 1, 128], F32, kind="ExternalOutput")
        d_dg = nc.dram_tensor("dbg_g", [nsteps, 128], F32, kind="ExternalOutput")
        d_dz = nc.dram_tensor("dbg_z", [nsteps, 128, HH], F16, kind="ExternalOutput")
        d_dsc = nc.dram_tensor("dbg_sc", [128, nsteps], F32, kind="ExternalOutput")

    with tile.TileContext(nc) as tc, ExitStack() as ctx:
        ep = ctx.enter_context
        pool = nc.gpsimd if use_pool else nc.vector

        p_sT = ep(tc.tile_pool(name="sT", bufs=1))
        p_qT = ep(tc.tile_pool(name="qT", bufs=1))
        p_sW = ep(tc.tile_pool(name="sW", bufs=1))
        p_rhs = ep(tc.tile_pool(name="rhs", bufs=1))
        p_prm = ep(tc.tile_pool(name="prm", bufs=1))
        p_hu = ep(tc.tile_pool(name="hu", bufs=2))
        p_hm = ep(tc.tile_pool(name="hm", bufs=2))
        p_hT = ep(tc.tile_pool(name="hT", bufs=2))
        p_rn = ep(tc.tile_pool(name="rn", bufs=2))
        p_b16 = ep(tc.tile_pool(name="b16", bufs=2))
        p_e16 = ep(tc.tile_pool(name="e16", bufs=2))
        p_sml = ep(tc.tile_pool(name="sml", bufs=3))

        sT = p_sT.tile([128, HC * NR], F16)   # [h%128, c*NR + t*8+b]
        qT = p_qT.tile([128, HC * NR], F16)
        sW = p_sW.tile([128, ntt * H], F16)   # [row%128, tile*768+h]
        rhs2 = p_rhs.tile([128, 2 * HC * BLK], F16)  # fused rhs, 2 parities
        keyV = p_prm.tile([128, H], F16, tag="keyV")
        keys16 = p_prm.tile([128, H], F16, tag="keys16")
        keysT = p_prm.tile([128, HC * 8], F16, tag="keysT")
        sK = p_prm.tile([128, NR], F16, tag="sK")
        sKall = p_prm.tile([128, nsteps], F32, tag="sKall")
        selK16 = p_prm.tile([128, RP], F16, tag="selK16")
        selKb16 = p_prm.tile([128, 8 * RP], F16, tag="selKb16")
        selB32 = p_prm.tile([128, 16 * RP], F16, tag="selB32")
        maskG = p_prm.tile([128, BL], F16, tag="maskG")
        I64 = p_prm.tile([128, RP], F16, tag="I64")
        I128 = p_prm.tile([128, 128], F32, tag="I128")
        selK32 = p_prm.tile([128, RP], F32, tag="selK32")
        mask24 = p_prm.tile([128, BL * L], F32, tag="mask24")
        ones_r = p_prm.tile([128, 128], F32, tag="ones_r")
        P64h = p_prm.tile([128, 128], F16, tag="P64h")
        I128h = p_prm.tile([128, 128], F16, tag="I128h")
        alphav = p_prm.tile([128, 1], F32, tag="alphav")
        av_n = p_prm.tile([128, 1], F32, tag="av_n")
        av_p = p_prm.tile([128, 1], F32, tag="av_p")
        Wsel = p_prm.tile([128, BL * L], F32, tag="Wsel")
        bvec = p_prm.tile([128, 1], F32, tag="bvec")
        scores = p_prm.tile([128, nsteps], F32, tag="scores")

        dma = nc.sync.dma_start

        dma(selK16[0:K, :], d_selK.ap())
        dma(selKb16[0:K, :], d_selKb.ap())
        dma(selB32[:, :], d_selB32.ap())
        dma(maskG[:, :], d_maskG.ap())
        dma(I64[:, :], d_I64.ap())
        dma(I128[:, :], d_I128.ap())
        dma(selK32[0:K, :], d_selK32.ap())
        dma(mask24[0:R, :], d_mask24.ap())
        dma(ones_r[0:1, :], d_ones.ap())
        dma(P64h[:, :], d_P64h.ap())
        dma(I128h[:, :], d_I128h.ap())
        for b in range(BL):
            dma(bvec[b * L:(b + 1) * L, 0:1], bass.AP(d_bout, 0, [[1, L], [1, 1]]))

        def hts(hT_tile, c):
            if c < 3:
                return hT_tile[:, c * 128:c * 128 + RP]
            return hT_tile[:, (c - 3) * 128 + RP:(c - 2) * 128]

        def emit_transposes(h_tile, out_psum):
            for cc in range(3):
                nc.tensor.transpose(out_psum[:, cc * 128:(cc + 1) * 128],
                                    h_tile[:, cc * 128:(cc + 1) * 128],
                                    I128h[:, :])

        def stage(t, par):
            """Copy sT_t and qT_{max(t-1,0)} slices into rhs2 parity par."""
            tq = max(t - 1, 0)
            mv = dma if stage_dma else pool.tensor_copy
            for i in range(2):
                mv(bass.AP(rhs2.tensor,
                           rhs2.offset + par * HC * BLK + i * FW + HH,
                           [[2 * HC * BLK, 128], [BLK, HC], [1, 8]]),
                   bass.AP(sT.tensor, sT.offset + t * 8,
                           [[HC * NR, 128], [NR, HC], [1, 8]]))
                mv(bass.AP(rhs2.tensor,
                           rhs2.offset + par * HC * BLK + i * FW + HH + 8,
                           [[2 * HC * BLK, 128], [BLK, HC], [1, 8]]),
                   bass.AP(qT.tensor, qT.offset + tq * 8,
                           [[HC * NR, 128], [NR, HC], [1, 8]]))

        def mask_reduce(out_sc, in_ap, msk, accum):
            if use_ttr:
                nc.vector.tensor_tensor_reduce(
                    out_sc, in_ap, msk, 1.0, 0.0, ALU.mult, ALU.add, accum)
            else:
                nc.vector.tensor_mul(out_sc, in_ap, msk)
                nc.vector.tensor_reduce(accum, out_sc,
                                        mybir.AxisListType.X, ALU.add)

        def emit_bias(bP, tn):
            """biasP for step tn: keyV broadcast + s_tn @ W broadcast."""
            m, tb = tn % 16, (tn * BL) // 128
            nc.tensor.matmul(bP[0:RP, :], selK16[0:K, :], keyV[0:K, 0:HH],
                             start=True, stop=False, skip_group_check=True)
            nc.tensor.matmul(bP[64:128, :], selK16[0:K, :], keyV[0:K, HH:H],
                             start=True, stop=False, skip_group_check=True)
            lsel = selB32[:, m * RP:(m + 1) * RP]
            nc.tensor.matmul(bP[0:RP, :], lsel, sW[:, tb * H:tb * H + HH],
                             start=False, stop=True, skip_group_check=True)
            nc.tensor.matmul(bP[64:128, :], lsel, sW[:, tb * H + HH:(tb + 1) * H],
                             start=False, stop=True, skip_group_check=True)

        with tc.tile_pool(name="pr32", bufs=2) as p32, \
             tc.tile_pool(name="prps", bufs=4, space="PSUM") as pps:

            def ptile(shape, dt):
                return pps.tile(shape, dt, tag="ps", name="ps")

            # keys
            kn = p32.tile([128, H], F32, tag="kn")
            dma(kn[0:K, :], d_keys.ap())
            nc.vector.tensor_copy(keys16[0:K, :], kn[0:K, :])
            # U (chunked fp32 load -> cast-copied into both parities of rhs2)
            for c in range(HC):
                u1 = p32.tile([128, H], F32, tag="u1")
                dma(u1[:, :], d_U.ap()[c * 128:(c + 1) * 128, :])
                for par in range(2):
                    for i in range(2):
                        dst = bass.AP(
                            rhs2.tensor,
                            rhs2.offset + par * HC * BLK + c * BLK + i * FW,
                            [[2 * HC * BLK, 128], [1, HH]])
                        nc.vector.tensor_copy(dst, u1[:, i * HH:(i + 1) * HH])
            # keysT via PE transpose
            tps_ = ptile([128, HC * 8], F16)
            for c in range(HC):
                nc.tensor.transpose(tps_[:, c * 8:c * 8 + K],
                                    keys16[0:K, c * 128:(c + 1) * 128],
                                    I64[0:K, 0:K])
            for c in range(HC):
                nc.vector.tensor_copy(keysT[:, c * 8:c * 8 + K],
                                      tps_[:, c * 8:c * 8 + K])
            # V -> keyV = keys @ V  (chunked)
            kvp0 = ptile([128, HH], F32)
            kvp1 = ptile([128, HH], F32)
            for c in range(HC):
                v1 = p32.tile([128, H], F32, tag="u1")
                dma(v1[:, :], d_V.ap()[c * 128:(c + 1) * 128, :])
                v16c = p32.tile([128, H], F16, tag="v16c")
                nc.vector.tensor_copy(v16c[:, :], v1[:, :])
                nc.tensor.matmul(kvp0[0:K, :], keysT[:, c * 8:c * 8 + K],
                                 v16c[:, 0:HH],
                                 start=(c == 0), stop=(c == HC - 1))
                nc.tensor.matmul(kvp1[0:K, :], keysT[:, c * 8:c * 8 + K],
                                 v16c[:, HH:H],
                                 start=(c == 0), stop=(c == HC - 1))
            nc.vector.tensor_copy(keyV[0:K, 0:HH], kvp0[0:K, :])
            nc.vector.tensor_copy(keyV[0:K, HH:H], kvp1[0:K, :])
            # W (fp16) for sW matmuls (chunked load)
            w16 = p32.tile([128, HC * H], F16, tag="big16w")
            for c in range(HC):
                w1 = p32.tile([128, H], F32, tag="u1")
                dma(w1[:, :], d_W.ap()[c * 128:(c + 1) * 128, :])
                nc.vector.tensor_copy(w16[:, c * H:(c + 1) * H], w1[:, :])

            # alpha -> all partitions
            asb = p32.tile([128, 1], F32, tag="asb")
            dma(asb[0:1, 0:1], bass.AP(d_alpha, 0, [[1, 1], [1, 1]]))
            alp = ptile([128, 1], F32)
            nc.tensor.matmul(alp[:, :], ones_r[0:1, :], asb[0:1, 0:1],
                             start=True, stop=True)
            nc.vector.tensor_copy(alphav[:, :], alp[:, :])
            nc.vector.tensor_scalar(av_n[:, :], alphav[:, :], -0.5, 0.5,
                                    ALU.mult, ALU.add)
            nc.vector.tensor_scalar(av_p[:, :], alphav[:, :], 0.5, 0.5,
                                    ALU.mult, ALU.add)

            # W_out -> Wsel
            wo = p32.tile([128, L], F32, tag="wo")
            dma(wo[0:K, :], d_Wout.ap())
            wrp = ptile([128, L], F32)
            nc.tensor.matmul(wrp[0:R, :], selK32[0:K, 0:R], wo[0:K, :],
                             start=True, stop=True)
            wrs = p32.tile([128, L], F32, tag="wrs")
            nc.vector.tensor_copy(wrs[0:R, :], wrp[0:R, :])
            for b in range(BL):
                nc.vector.tensor_mul(Wsel[0:R, b * L:(b + 1) * L],
                                     wrs[0:R, :], mask24[0:R, b * L:(b + 1) * L])

            # features: DMA, transpose (and sW for the sentence stream)
            def do_feat(dram, dstT, with_sw):
                for tb in range(ntt):
                    fn = p32.tile([128, H], F32, tag="fnat")
                    nrow = min(128, NR - tb * 128)
                    src = bass.AP(dram, tb * 128 * H, [[H, nrow], [1, H]])
                    dma(fn[0:nrow, :], src)
                    for grp in range(2):
                        tp = ptile([128, 3 * 128], F32)
                        for j in range(3):
                            c = grp * 3 + j
                            nc.tensor.transpose(
                                tp[:, j * nrow:(j + 1) * nrow],
                                fn[0:nrow, c * 128:(c + 1) * 128],
                                I128[0:nrow, 0:nrow])
                        dst = bass.AP(
                            dstT.tensor,
                            dstT.offset + (grp * 3) * NR + tb * 128,
                            [[HC * NR, 128], [NR, 3], [1, nrow]])
                        nc.vector.tensor_copy(
                            dst,
                            tp[:, 0:3 * nrow].rearrange("p (a b) -> p a b", a=3))
                    if with_sw:
                        for i in range(2):
                            swp = ptile([128, HH], F32)
                            for c in range(HC):
                                lhs = dstT[:, c * NR + tb * 128:
                                           c * NR + tb * 128 + nrow]
                                nc.tensor.matmul(
                                    swp[0:nrow, :], lhs,
                                    w16[:, c * H + i * HH:c * H + (i + 1) * HH],
                                    start=(c == 0), stop=(c == HC - 1))
                            nc.vector.tensor_copy(
                                sW[0:nrow, tb * H + i * HH:tb * H + (i + 1) * HH],
                                swp[0:nrow, :])

            if NR < 128:
                nc.vector.memset(sW[:, :], 0.0)
            do_feat(d_fs, sT, True)
            do_feat(d_fe, qT, False)

            # sK = keys @ s^T  [5, NR]
            for q in range((NR + 511) // 512):
                ncol = min(512, NR - q * 512)
                skp = ptile([128, 512], F32)
                for c in range(HC):
                    nc.tensor.matmul(
                        skp[0:K, 0:ncol], keysT[:, c * 8:c * 8 + K],
                        sT[:, c * NR + q * 512:c * NR + q * 512 + ncol],
                        start=(c == 0), stop=(c == HC - 1))
                nc.vector.tensor_copy(sK[0:K, q * 512:q * 512 + ncol],
                                      skp[0:K, 0:ncol])

            # sKall[p, t] = sK[k(p), t*8 + b(p)]  (gate content bias per row)
            skap = ptile([128, nsteps], F32)
            sKr = sK[:, :].rearrange("p (t b) -> p b t", b=BL)
            for b in range(BL):
                sel = selKb16[0:K, b * RP:(b + 1) * RP]
                src = sKr[0:K, b:b + 1, 0:nsteps]
                nc.tensor.matmul(skap[0:RP, :], sel, src,
                                 start=(b == 0), stop=(b == BL - 1),
                                 skip_group_check=True)
                nc.tensor.matmul(skap[64:128, :], sel, src,
                                 start=(b == 0), stop=(b == BL - 1),
                                 skip_group_check=True)
            nc.vector.tensor_copy(sKall[:, :], skap[:, :])

            # hu_0 = h0 = keys broadcast to (k,b) rows (zero into pad rows)
            h0p = ptile([128, HH], F32)
            nc.tensor.matmul(h0p[0:RP, :], selK16[0:K, :], keys16[0:K, 0:HH],
                             start=True, stop=True, skip_group_check=True)
            nc.tensor.matmul(h0p[64:128, :], selK16[0:K, :], keys16[0:K, HH:H],
                             start=True, stop=True, skip_group_check=True)
            hu_cur = p_hu.tile([128, HH], F16, tag="hu")
            nc.vector.tensor_copy(hu_cur[:, :], h0p[:, :])

            tp0 = ptile([128, 3 * 128], F16)
            emit_transposes(hu_cur, tp0)
            hT_cur = p_hT.tile([128, 3 * 128], F16, tag="hT")
            nc.vector.tensor_copy(hT_cur[:, :], tp0[:, :])

        # PSUM pools for the main loop (opened after the prologue pool frees)
        p_zps = ep(tc.tile_pool(name="zps", bufs=2, space="PSUM"))
        p_bps = ep(tc.tile_pool(name="bps", bufs=2, space="PSUM"))
        p_tps = ep(tc.tile_pool(name="tps", bufs=1, space="PSUM"))
        p_sps = ep(tc.tile_pool(name="sps", bufs=1, space="PSUM"))
        p_eps = ep(tc.tile_pool(name="eps", bufs=1, space="PSUM"))

        # rn_0 = 1, h_mat_0 = hu_0
        rn_cur = p_rn.tile([128, 1], F32, tag="rn")
        nc.vector.memset(rn_cur[:, :], 1.0)
        nc.vector.memset(scores[:, :], 0.0)
        hm_cur = p_hm.tile([128, HH], F16, tag="hm")
        nc.vector.tensor_copy(hm_cur[:, :], hu_cur[:, :])

        # stage steps 0 and 1; bias for step 0
        stage(0, 0)
        if nsteps > 1:
            stage(1, 1)
        bP0 = p_bps.tile([128, HH], F32, tag="b")
        emit_bias(bP0, 0)
        b16_cur = p_b16.tile([128, HH], F16, tag="b16")
        nc.scalar.copy(b16_cur[:, :], bP0[:, :])

        # ---- main loop ----
        ss_n = ss16_n = None  # SBUF ss (fp32) and fp16 copy for step t+1
        for t in range(nsteps):
            par = t % 2

            # rsqrt chain for rn_t (t>=1): fold -> seed -> Newton -> h_mat
            if t > 0:
                ssp = p_sps.tile([128, 1], F32, tag="ssp")
                nc.tensor.matmul(ssp[:, :], P64h[:, :], ss16_n[:, :],
                                 start=True, stop=True)
                sdi = p_sml.tile([128, 1], I32, tag="sdi")
                nc.vector.tensor_scalar(sdi[:, :], ssp.bitcast(I32)[:, :], 1,
                                        None, ALU.logical_shift_right)
                ss2 = p_sml.tile([128, 1], F32, tag="ss2")
                nc.vector.tensor_copy(ss2[:, :], ssp[:, :])
                pool.tensor_scalar(sdi[:, :], sdi[:, :], -1, 0x5F3759DF,
                                        ALU.mult, ALU.add)
                rn_w = sdi.bitcast(F32)
                ra = p_sml.tile([128, 1], F32, tag="ra")
                for _ in range(2):
                    pool.tensor_mul(ra[:, :], rn_w[:, :], ss2[:, :])
                    pool.tensor_mul(ra[:, :], ra[:, :], rn_w[:, :])
                    pool.tensor_scalar(ra[:, :], ra[:, :], -0.5, 1.5,
                                            ALU.mult, ALU.add)
                    pool.tensor_mul(rn_w[:, :], rn_w[:, :], ra[:, :])
                rn_cur = p_rn.tile([128, 1], F32, tag="rn")
                pool.tensor_copy(rn_cur[:, :], rn_w[:, :])
                hm_cur = p_hm.tile([128, HH], F16, tag="hm")
                pool.tensor_scalar(hm_cur[:, :], hu_cur[:, :],
                                        rn_cur[:, :], None, ALU.mult)

            # fused z/g/q block
            zP = p_zps.tile([128, FW], F32, tag="z")
            for c in range(HC):
                lhs = hts(hT_cur, c)
                base = par * HC * BLK + c * BLK
                nc.tensor.matmul(zP[0:RP, :], lhs,
                                 rhs2[:, base:base + FW],
                                 start=(c == 0), stop=(c == HC - 1),
                                 skip_group_check=True)
                nc.tensor.matmul(zP[64:128, :], lhs,
                                 rhs2[:, base + FW:base + BLK],
                                 start=(c == 0), stop=(c == HC - 1),
                                 skip_group_check=True)

            # off-path work for t+1
            if t + 2 < nsteps:
                stage(t + 2, t % 2)
            if t + 1 < nsteps:
                bP = p_bps.tile([128, HH], F32, tag="b")
                emit_bias(bP, t + 1)

            # gate: masked reduce + sigmoid(rn*gpre + s.keys)
            gsc = p_sml.tile([128, BL], F16, tag="gsc")
            gpre = p_sml.tile([128, 1], F32, tag="gpre")
            mask_reduce(gsc[:, :], zP[:, HH:HH + 8], maskG[:, :],
                        gpre[:, :])
            gsig = p_sml.tile([128, 1], F32, tag="gsig")
            nc.scalar.activation(gsig[:, :], gpre[:, :], ACTF.Sigmoid,
                                 bias=sKall[:, t:t + 1], scale=rn_cur[:, :])

            # z = rn*zU + bias; cand = g*prelu(z)
            zs16 = p_e16.tile([128, HH], F16, tag="zs16")
            nc.vector.tensor_scalar(zs16[:, :], zP[:, 0:HH], rn_cur[:, :],
                                    None, ALU.mult)
            z16 = p_e16.tile([128, HH], F16, tag="z16")
            nc.vector.tensor_add(z16[:, :], zs16[:, :], b16_cur[:, :])
            cand = p_e16.tile([128, HH], F16, tag="cand")
            hu_new = p_hu.tile([128, HH], F16, tag="hu")
            if use_prelu:
                nc.scalar.activation(cand[:, :], z16[:, :], ACTF.Prelu,
                                     scale=gsig[:, :], alpha=alphav[:, :])
                # hu_{t+1} = h_t + cand
                nc.vector.tensor_add(hu_new[:, :], hm_cur[:, :], cand[:, :])
            else:
                # sim fallback: g*prelu(z) = g(1-a)/2*|z| + g(1+a)/2*z
                ca = p_sml.tile([128, 1], F32, tag="ca")
                cb = p_sml.tile([128, 1], F32, tag="cb")
                nc.vector.tensor_mul(ca[:, :], gsig[:, :], av_n[:, :])
                nc.vector.tensor_mul(cb[:, :], gsig[:, :], av_p[:, :])
                nc.scalar.activation(cand[:, :], z16[:, :], ACTF.Abs,
                                     scale=ca[:, :])
                bv = p_e16.tile([128, HH], F16, tag="bv")
                nc.vector.tensor_scalar(bv[:, :], z16[:, :], cb[:, :], None,
                                        ALU.mult)
                nc.vector.tensor_add(hu_new[:, :], hm_cur[:, :], cand[:, :])
                nc.vector.tensor_add(hu_new[:, :], hu_new[:, :], bv[:, :])

            # transposes -> huT_{t+1}
            tP = p_tps.tile([128, 3 * 128], F16, tag="t")
            emit_transposes(hu_new, tP)
            hT_new = p_hT.tile([128, 3 * 128], F16, tag="hT")
            nc.vector.tensor_copy(hT_new[:, :], tP[:, :])

            # norm accumulators for rn_{t+1}
            squ = p_e16.tile([128, HH], F16, tag="squ")
            ss_n = p_sml.tile([128, 1], F32, tag="ss")
            nc.scalar.activation(squ[:, :], hu_new[:, :], ACTF.Square,
                                 accum_out=ss_n[:, :])
            ss16_n = p_sml.tile([128, 1], F16, tag="ss16")
            pool.tensor_copy(ss16_n[:, :], ss_n[:, :])

            # bias16 for t+1 (ACT does the PSUM read; Pool has no PSUM port)
            if t + 1 < nsteps:
                b16_new = p_b16.tile([128, HH], F16, tag="b16")
                nc.scalar.copy(b16_new[:, :], bP[:, :])
            else:
                b16_new = b16_cur

            # score for step t-1: rn_t * masked-reduce(q-gram cols)
            if t > 0:
                qsc = p_sml.tile([128, BL], F16, tag="qsc")
                qtmp = p_sml.tile([128, 1], F32, tag="qtmp")
                mask_reduce(qsc[:, :], zP[:, HH + 8:FW], maskG[:, :],
                            qtmp[:, :])
                pool.tensor_scalar(scores[:, t - 1:t], qtmp[:, :],
                                        rn_cur[:, :], None, ALU.mult)

            if debug:
                dma(bass.AP(d_dhu, t * 128 * HH, [[HH, 128], [1, HH]]),
                    hu_new[:, :])
                dma(bass.AP(d_drn, t * 128, [[1, 128], [1, 1]]), rn_cur[:, :])
                dma(bass.AP(d_dg, t * 128, [[1, 128], [1, 1]]), gsig[:, :])
                dma(bass.AP(d_dz, t * 128 * HH, [[HH, 128], [1, HH]]),
                    z16[:, :])

            hu_cur, hT_cur, b16_cur = hu_new, hT_new, b16_new

        # ---- epilogue ----
        # rn_T
        ssp = p_sps.tile([128, 1], F32, tag="ssp")
        nc.tensor.matmul(ssp[:, :], P64h[:, :], ss16_n[:, :],
                         start=True, stop=True)
        sdi = p_sml.tile([128, 1], I32, tag="sdi")
        nc.vector.tensor_scalar(sdi[:, :], ssp.bitcast(I32)[:, :], 1,
                                None, ALU.logical_shift_right)
        ss2 = p_sml.tile([128, 1], F32, tag="ss2")
        nc.vector.tensor_copy(ss2[:, :], ssp[:, :])
        pool.tensor_scalar(sdi[:, :], sdi[:, :], -1, 0x5F3759DF,
                                ALU.mult, ALU.add)
        rn_w = sdi.bitcast(F32)
        ra = p_sml.tile([128, 1], F32, tag="ra")
        for _ in range(2):
            pool.tensor_mul(ra[:, :], rn_w[:, :], ss2[:, :])
            pool.tensor_mul(ra[:, :], ra[:, :], rn_w[:, :])
            pool.tensor_scalar(ra[:, :], ra[:, :], -0.5, 1.5,
                                    ALU.mult, ALU.add)
            pool.tensor_mul(rn_w[:, :], rn_w[:, :], ra[:, :])
        rn_T = p_rn.tile([128, 1], F32, tag="rn")
        pool.tensor_copy(rn_T[:, :], rn_w[:, :])

        # final q-gram: q_{T-1} . hu_T
        qPf = p_eps.tile([128, BL], F32, tag="qf")
        for c in range(HC):
            nc.tensor.matmul(
                qPf[0:RP, 0:BL], hts(hT_cur, c),
                qT[:, c * NR + (nsteps - 1) * 8:c * NR + nsteps * 8],
                start=(c == 0), stop=(c == HC - 1))
        qsc = p_sml.tile([128, BL], F16, tag="qsc")
        qtmp = p_sml.tile([128, 1], F32, tag="qtmp")
        mask_reduce(qsc[0:RP, :], qPf[0:RP, 0:BL], maskG[0:RP, :],
                    qtmp[0:RP, :])
        pool.tensor_scalar(scores[0:RP, nsteps - 1:nsteps],
                                qtmp[0:RP, :], rn_T[0:RP, :], None, ALU.mult)

        # output head
        pP = p_eps.tile([128, nsteps], F32, tag="pp")
        nc.tensor.matmul(pP[0:BL * L, :], Wsel[0:R, 0:BL * L],
                         scores[0:R, 0:nsteps], start=True, stop=True)
        osb = p_prm.tile([128, nsteps], F32, tag="osb")
        nc.vector.tensor_scalar(osb[0:BL * L, :], pP[0:BL * L, :],
                                bvec[0:BL * L, :], None, ALU.add)
        nc.sync.dma_start(bass.AP(d_out, 0, [[1, BL * L], [BL * L, nsteps]]),
                          osb[0:BL * L, :])
        if debug:
            dma(bass.AP(d_dsc, 0, [[nsteps, 128], [1, nsteps]]),
                scores[:, :])

    nc.compile()
    return nc


_CACHE = {}


def _get(nsteps, debug=False, use_prelu=True, **kw):
    key = (nsteps, debug, use_prelu) + tuple(sorted(kw.items()))
    if key not in _CACHE:
        _CACHE[key] = _build(nsteps, debug=debug, use_prelu=use_prelu, **kw)
    return _CACHE[key]


def _in_maps(inputs, nsteps):
    consts = _host_consts()
    fs = np.ascontiguousarray(np.asarray(inputs["features_sentence"], dtype=np.float32))
    fe = np.ascontiguousarray(np.asarray(inputs["features_entity"], dtype=np.float32))
    shared = {k: np.ascontiguousarray(np.asarray(inputs[k], dtype=np.float32))
              for k in ("keys", "U", "V", "W", "alpha", "W_out", "b_out")}
    shared.update(consts)
    in_maps = []
    for c in range(NC):
        m = dict(shared)
        m["features_sentence"] = np.ascontiguousarray(fs[:, c * BL:(c + 1) * BL, :])
        m["features_entity"] = np.ascontiguousarray(fe[:, c * BL:(c + 1) * BL, :])
        in_maps.append(m)
    return in_maps


def kernel(**inputs):
    nsteps = inputs["features_sentence"].shape[0]
    nc = _get(nsteps)
    res = run_bass_kernel_spmd(nc, _in_maps(inputs, nsteps),
                               core_ids=list(range(NC)))
    outs = [r["preds"].reshape(nsteps, BL, L) for r in res.results]
    return np.concatenate(outs, axis=1).reshape(nsteps * B, L)


# revision 22
# speedup vs baseline: 1.6182x; 1.6182x over previous
"""EntNetHead Trainium2 kernel (v2: deferred normalization).

Data-parallel over batch B=64 across 8 NeuronCores (8 batch rows per core);
T=256 recurrent steps run on-chip per core.

Key idea vs v1: the per-step L2 normalization is linear, so the state is
kept UNNORMALIZED (hu) together with rn = 1/||hu|| per packed row.  The
matmuls use huT directly; rn folds into (a) the sigmoid's per-partition
scale, (b) one DVE scale of the z PSUM, (c) a lazily-materialized
h = rn*hu used by the state update.  The whole square/fold/rsqrt chain
runs OFF the critical path (ACT/Pool/PE idle slots).

Per-step PE block is 12 matmuls: the g-gram (s_t . hu) and q-gram
(q_{t-1} . hu) are fused into the z matmuls as 16 extra rhs columns,
staged next to the U chunks by the (otherwise idle) DMA engines.  The
gate's content bias s_t.keys is precomputed for all t as a per-partition
column (sKall) and enters through the sigmoid's bias AP. The z bias
keyV + s_t@W accumulates in a separate PSUM bank off the critical path.

Engine split per step:
  PE:   12 fused z/g/q matmuls, 4 bias matmuls (t+1), 3 transposes,
        1 fp16 norm-fold matmul
  DVE:  gate reduce, z-scale, z-add(bias), hu update, hT copy,
        q reduce, rsqrt seed
  ACT:  sigmoid, prelu, square+accum, bias PSUM->SBUF copy
  Pool: Newton rsqrt, h materialize, score scale, ss fp16 cast
  DMA:  per-step staging of sT/qT slices into the fused rhs buffer
"""

import sys

sys.path.insert(0, "/opt/trn_rl_repo")

from contextlib import ExitStack

import numpy as np

import concourse.bacc as bacc
import concourse.bass as bass
import concourse.tile as tile
from concourse import mybir
from concourse.bass_utils import run_bass_kernel_spmd

F32 = mybir.dt.float32
F16 = mybir.dt.float16
I32 = mybir.dt.int32
ALU = mybir.AluOpType
ACTF = mybir.ActivationFunctionType

T, B, H, K, L = 256, 64, 768, 5, 3
NC = 8
BL = B // NC          # 8 batch rows per core
R = K * BL            # 40 (k,b) rows
RP = 64               # padded stationary width
HC = H // 128         # 6 contraction chunks
HH = H // 2           # 384
BLK = 2 * (HH + 16)   # 800: per-chunk fused rhs block [U_h0|sT|qT|U_h1|sT|qT]
FW = HH + 16          # 400: fused matmul width per half


def _host_consts():
    selK = np.zeros((K, RP), np.float16)
    for k in range(K):
        selK[k, k * BL:(k + 1) * BL] = 1.0
    selKb = np.zeros((K, 8 * RP), np.float16)
    for b in range(BL):
        for k in range(K):
            selKb[k, b * RP + k * BL + b] = 1.0
    selB = np.zeros((128, 16 * RP), np.float16)
    for m in range(16):
        for b in range(BL):
            for k in range(K):
                selB[m * BL + b, m * RP + k * BL + b] = 1.0
    maskG = np.zeros((128, BL), np.float16)
    for p in range(128):
        maskG[p, p % BL] = 1.0
    I64 = np.zeros((128, RP), np.float16)
    for j in range(RP):
        I64[j, j] = 1.0
        I64[64 + j, j] = 1.0
    I128 = np.eye(128, dtype=np.float32)
    mask24 = np.zeros((R, BL * L), np.float32)
    for k in range(K):
        for b in range(BL):
            mask24[k * BL + b, b * L:(b + 1) * L] = 1.0
    ones1x128 = np.ones((1, 128), np.float32)
    P64h = np.zeros((128, 128), np.float16)
    for i in range(128):
        P64h[i, i % 64] = 1.0
        P64h[i, i % 64 + 64] = 1.0
    selK32 = selK.astype(np.float32)
    return {
        "c_selK": selK, "c_selKb": selKb, "c_selB32": selB, "c_maskG": maskG,
        "c_I64": I64, "c_I128": I128, "c_selK32": selK32,
        "c_mask24": mask24, "c_ones": ones1x128, "c_P64h": P64h,
        "c_I128h": np.eye(128, dtype=np.float16),
    }


def _build(nsteps, debug=False, use_prelu=True,
           stage_dma=True, use_pool=True, use_ttr=False, newton_iters=1):
    # use_ttr: InstTensorTensorReduce wedges TRN2 here (NRT_EXEC_UNIT_
    # UNRECOVERABLE) despite passing CoreSim -- keep the 2-op fallback.
    nc = bacc.Bacc("TRN2", target_bir_lowering=False, debug=False)
    NR = nsteps * BL      # feature rows per core
    ntt = (NR + 127) // 128

    d_fs = nc.dram_tensor("features_sentence", [nsteps, BL, H], F32, kind="ExternalInput")
    d_fe = nc.dram_tensor("features_entity", [nsteps, BL, H], F32, kind="ExternalInput")
    d_keys = nc.dram_tensor("keys", [K, H], F32, kind="ExternalInput")
    d_U = nc.dram_tensor("U", [H, H], F32, kind="ExternalInput")
    d_V = nc.dram_tensor("V", [H, H], F32, kind="ExternalInput")
    d_W = nc.dram_tensor("W", [H, H], F32, kind="ExternalInput")
    d_alpha = nc.dram_tensor("alpha", [1], F32, kind="ExternalInput")
    d_Wout = nc.dram_tensor("W_out", [K, L], F32, kind="ExternalInput")
    d_bout = nc.dram_tensor("b_out", [L], F32, kind="ExternalInput")
    d_selK = nc.dram_tensor("c_selK", [K, RP], F16, kind="ExternalInput")
    d_selKb = nc.dram_tensor("c_selKb", [K, 8 * RP], F16, kind="ExternalInput")
    d_selB32 = nc.dram_tensor("c_selB32", [128, 16 * RP], F16, kind="ExternalInput")
    d_maskG = nc.dram_tensor("c_maskG", [128, BL], F16, kind="ExternalInput")
    d_I64 = nc.dram_tensor("c_I64", [128, RP], F16, kind="ExternalInput")
    d_I128 = nc.dram_tensor("c_I128", [128, 128], F32, kind="ExternalInput")
    d_selK32 = nc.dram_tensor("c_selK32", [K, RP], F32, kind="ExternalInput")
    d_mask24 = nc.dram_tensor("c_mask24", [R, BL * L], F32, kind="ExternalInput")
    d_ones = nc.dram_tensor("c_ones", [1, 128], F32, kind="ExternalInput")
    d_P64h = nc.dram_tensor("c_P64h", [128, 128], F16, kind="ExternalInput")
    d_I128h = nc.dram_tensor("c_I128h", [128, 128], F16, kind="ExternalInput")
    d_out = nc.dram_tensor("preds", [NR, L], F32, kind="ExternalOutput")
    if debug:
        d_dhu = nc.dram_tensor("dbg_hu", [nsteps + 1, 128, HH], F16, kind="ExternalOutput")
        d_drn = nc.dram_tensor("dbg_rn", [nsteps + 1, 128], F32, kind="ExternalOutput")
        d_dg = nc.dram_tensor("dbg_g", [nsteps, 128], F32, kind="ExternalOutput")
        d_dz = nc.dram_tensor("dbg_z", [nsteps, 128, HH], F16, kind="ExternalOutput")
        d_dsc = nc.dram_tensor("dbg_sc", [128, nsteps], F32, kind="ExternalOutput")

    with tile.TileContext(nc) as tc, ExitStack() as ctx:
        ep = ctx.enter_context
        pool = nc.gpsimd if use_pool else nc.vector

        p_sT = ep(tc.tile_pool(name="sT", bufs=1))
        p_qT = ep(tc.tile_pool(name="qT", bufs=1))
        p_sW = ep(tc.tile_pool(name="sW", bufs=1))
        p_rhs = ep(tc.tile_pool(name="rhs", bufs=1))
        p_prm = ep(tc.tile_pool(name="prm", bufs=1))
        p_hu = ep(tc.tile_pool(name="hu", bufs=2))
        p_hm = ep(tc.tile_pool(name="hm", bufs=2))
        p_hT = ep(tc.tile_pool(name="hT", bufs=2))
        p_rn = ep(tc.tile_pool(name="rn", bufs=2))
        p_b16 = ep(tc.tile_pool(name="b16", bufs=2))
        p_e16 = ep(tc.tile_pool(name="e16", bufs=2))
        p_sml = ep(tc.tile_pool(name="sml", bufs=3))

        sT = p_sT.tile([128, HC * NR], F16)   # [h%128, c*NR + t*8+b]
        qT = p_qT.tile([128, HC * NR], F16)
        sW = p_sW.tile([128, ntt * H], F16)   # [row%128, tile*768+h]
        rhs2 = p_rhs.tile([128, 2 * HC * BLK], F16)  # fused rhs, 2 parities
        keyV = p_prm.tile([128, H], F16, tag="keyV")
        keys16 = p_prm.tile([128, H], F16, tag="keys16")
        keysT = p_prm.tile([128, HC * 8], F16, tag="keysT")
        sK = p_prm.tile([128, NR], F16, tag="sK")
        sKall = p_prm.tile([128, nsteps], F32, tag="sKall")
        selK16 = p_prm.tile([128, RP], F16, tag="selK16")
        selKb16 = p_prm.tile([128, 8 * RP], F16, tag="selKb16")
        selB32 = p_prm.tile([128, 16 * RP], F16, tag="selB32")
        maskG = p_prm.tile([128, BL], F16, tag="maskG")
        I64 = p_prm.tile([128, RP], F16, tag="I64")
        I128 = p_prm.tile([128, 128], F32, tag="I128")
        selK32 = p_prm.tile([128, RP], F32, tag="selK32")
        mask24 = p_prm.tile([128, BL * L], F32, tag="mask24")
        ones_r = p_prm.tile([128, 128], F32, tag="ones_r")
        P64h = p_prm.tile([128, 128], F16, tag="P64h")
        I128h = p_prm.tile([128, 128], F16, tag="I128h")
        alphav = p_prm.tile([128, 1], F32, tag="alphav")
        av_n = p_prm.tile([128, 1], F32, tag="av_n")
        av_p = p_prm.tile([128, 1], F32, tag="av_p")
        Wsel = p_prm.tile([128, BL * L], F32, tag="Wsel")
        bvec = p_prm.tile([128, 1], F32, tag="bvec")
        scores = p_prm.tile([128, nsteps], F32, tag="scores")

        dma = nc.sync.dma_start

        dma(selK16[0:K, :], d_selK.ap())
        dma(selKb16[0:K, :], d_selKb.ap())
        dma(selB32[:, :], d_selB32.ap())
        dma(maskG[:, :], d_maskG.ap())
        dma(I64[:, :], d_I64.ap())
        dma(I128[:, :], d_I128.ap())
        dma(selK32[0:K, :], d_selK32.ap())
        dma(mask24[0:R, :], d_mask24.ap())
        dma(ones_r[0:1, :], d_ones.ap())
        dma(P64h[:, :], d_P64h.ap())
        dma(I128h[:, :], d_I128h.ap())
        for b in range(BL):
            dma(bvec[b * L:(b + 1) * L, 0:1], bass.AP(d_bout, 0, [[1, L], [1, 1]]))

        def hts(hT_tile, c):
            if c < 3:
                return hT_tile[:, c * 128:c * 128 + RP]
            return hT_tile[:, (c - 3) * 128 + RP:(c - 2) * 128]

        def emit_transposes(h_tile, out_psum):
            for cc in range(3):
                nc.tensor.transpose(out_psum[:, cc * 128:(cc + 1) * 128],
                                    h_tile[:, cc * 128:(cc + 1) * 128],
                                    I128h[:, :])

        def stage(t, par):
            """Copy sT_t and qT_{max(t-1,0)} slices into rhs2 parity par."""
            tq = max(t - 1, 0)
            mv = dma if stage_dma else pool.tensor_copy
            for i in range(2):
                mv(bass.AP(rhs2.tensor,
                           rhs2.offset + par * HC * BLK + i * FW + HH,
                           [[2 * HC * BLK, 128], [BLK, HC], [1, 8]]),
                   bass.AP(sT.tensor, sT.offset + t * 8,
                           [[HC * NR, 128], [NR, HC], [1, 8]]))
                mv(bass.AP(rhs2.tensor,
                           rhs2.offset + par * HC * BLK + i * FW + HH + 8,
                           [[2 * HC * BLK, 128], [BLK, HC], [1, 8]]),
                   bass.AP(qT.tensor, qT.offset + tq * 8,
                           [[HC * NR, 128], [NR, HC], [1, 8]]))

        def mask_reduce(out_sc, in_ap, msk, accum, eng=None):
            eng = eng or nc.vector
            if use_ttr:
                nc.vector.tensor_tensor_reduce(
                    out_sc, in_ap, msk, 1.0, 0.0, ALU.mult, ALU.add, accum)
            else:
                eng.tensor_mul(out_sc, in_ap, msk)
                eng.tensor_reduce(accum, out_sc,
                                  mybir.AxisListType.X, ALU.add)

        def emit_bias(bP, tn):
            """biasP for step tn: keyV broadcast + s_tn @ W broadcast."""
            m, tb = tn % 16, (tn * BL) // 128
            nc.tensor.matmul(bP[0:RP, :], selK16[0:K, :], keyV[0:K, 0:HH],
                             start=True, stop=False, skip_group_check=True)
            nc.tensor.matmul(bP[64:128, :], selK16[0:K, :], keyV[0:K, HH:H],
                             start=True, stop=False, skip_group_check=True)
            lsel = selB32[:, m * RP:(m + 1) * RP]
            nc.tensor.matmul(bP[0:RP, :], lsel, sW[:, tb * H:tb * H + HH],
                             start=False, stop=True, skip_group_check=True)
            nc.tensor.matmul(bP[64:128, :], lsel, sW[:, tb * H + HH:(tb + 1) * H],
                             start=False, stop=True, skip_group_check=True)

        with tc.tile_pool(name="pr32", bufs=2) as p32, \
             tc.tile_pool(name="prps", bufs=4, space="PSUM") as pps:

            def ptile(shape, dt):
                return pps.tile(shape, dt, tag="ps", name="ps")

            # keys
            kn = p32.tile([128, H], F32, tag="kn")
            dma(kn[0:K, :], d_keys.ap())
            nc.vector.tensor_copy(keys16[0:K, :], kn[0:K, :])
            # U (chunked fp32 load -> cast-copied into both parities of rhs2)
            for c in range(HC):
                u1 = p32.tile([128, H], F32, tag="u1")
                dma(u1[:, :], d_U.ap()[c * 128:(c + 1) * 128, :])
                for par in range(2):
                    for i in range(2):
                        dst = bass.AP(
                            rhs2.tensor,
                            rhs2.offset + par * HC * BLK + c * BLK + i * FW,
                            [[2 * HC * BLK, 128], [1, HH]])
                        nc.vector.tensor_copy(dst, u1[:, i * HH:(i + 1) * HH])
            # keysT via PE transpose
            tps_ = ptile([128, HC * 8], F16)
            for c in range(HC):
                nc.tensor.transpose(tps_[:, c * 8:c * 8 + K],
                                    keys16[0:K, c * 128:(c + 1) * 128],
                                    I64[0:K, 0:K])
            for c in range(HC):
                nc.vector.tensor_copy(keysT[:, c * 8:c * 8 + K],
                                      tps_[:, c * 8:c * 8 + K])
            # V -> keyV = keys @ V  (chunked)
            kvp0 = ptile([128, HH], F32)
            kvp1 = ptile([128, HH], F32)
            for c in range(HC):
                v1 = p32.tile([128, H], F32, tag="u1")
                dma(v1[:, :], d_V.ap()[c * 128:(c + 1) * 128, :])
                v16c = p32.tile([128, H], F16, tag="v16c")
                nc.vector.tensor_copy(v16c[:, :], v1[:, :])
                nc.tensor.matmul(kvp0[0:K, :], keysT[:, c * 8:c * 8 + K],
                                 v16c[:, 0:HH],
                                 start=(c == 0), stop=(c == HC - 1))
                nc.tensor.matmul(kvp1[0:K, :], keysT[:, c * 8:c * 8 + K],
                                 v16c[:, HH:H],
                                 start=(c == 0), stop=(c == HC - 1))
            nc.vector.tensor_copy(keyV[0:K, 0:HH], kvp0[0:K, :])
            nc.vector.tensor_copy(keyV[0:K, HH:H], kvp1[0:K, :])
            # W (fp16) for sW matmuls (chunked load)
            w16 = p32.tile([128, HC * H], F16, tag="big16w")
            for c in range(HC):
                w1 = p32.tile([128, H], F32, tag="u1")
                dma(w1[:, :], d_W.ap()[c * 128:(c + 1) * 128, :])
                nc.vector.tensor_copy(w16[:, c * H:(c + 1) * H], w1[:, :])

            # alpha -> all partitions
            asb = p32.tile([128, 1], F32, tag="asb")
            dma(asb[0:1, 0:1], bass.AP(d_alpha, 0, [[1, 1], [1, 1]]))
            alp = ptile([128, 1], F32)
            nc.tensor.matmul(alp[:, :], ones_r[0:1, :], asb[0:1, 0:1],
                             start=True, stop=True)
            nc.vector.tensor_copy(alphav[:, :], alp[:, :])
            nc.vector.tensor_scalar(av_n[:, :], alphav[:, :], -0.5, 0.5,
                                    ALU.mult, ALU.add)
            nc.vector.tensor_scalar(av_p[:, :], alphav[:, :], 0.5, 0.5,
                                    ALU.mult, ALU.add)

            # W_out -> Wsel
            wo = p32.tile([128, L], F32, tag="wo")
            dma(wo[0:K, :], d_Wout.ap())
            wrp = ptile([128, L], F32)
            nc.tensor.matmul(wrp[0:R, :], selK32[0:K, 0:R], wo[0:K, :],
                             start=True, stop=True)
            wrs = p32.tile([128, L], F32, tag="wrs")
            nc.vector.tensor_copy(wrs[0:R, :], wrp[0:R, :])
            for b in range(BL):
                nc.vector.tensor_mul(Wsel[0:R, b * L:(b + 1) * L],
                                     wrs[0:R, :], mask24[0:R, b * L:(b + 1) * L])

            # features: DMA, transpose (and sW for the sentence stream)
            def do_feat(dram, dstT, with_sw):
                for tb in range(ntt):
                    fn = p32.tile([128, H], F32, tag="fnat")
                    nrow = min(128, NR - tb * 128)
                    src = bass.AP(dram, tb * 128 * H, [[H, nrow], [1, H]])
                    dma(fn[0:nrow, :], src)
                    for grp in range(2):
                        tp = ptile([128, 3 * 128], F32)
                        for j in range(3):
                            c = grp * 3 + j
                            nc.tensor.transpose(
                                tp[:, j * nrow:(j + 1) * nrow],
                                fn[0:nrow, c * 128:(c + 1) * 128],
                                I128[0:nrow, 0:nrow])
                        dst = bass.AP(
                            dstT.tensor,
                            dstT.offset + (grp * 3) * NR + tb * 128,
                            [[HC * NR, 128], [NR, 3], [1, nrow]])
                        nc.vector.tensor_copy(
                            dst,
                            tp[:, 0:3 * nrow].rearrange("p (a b) -> p a b", a=3))
                    if with_sw:
                        for i in range(2):
                            swp = ptile([128, HH], F32)
                            for c in range(HC):
                                lhs = dstT[:, c * NR + tb * 128:
                                           c * NR + tb * 128 + nrow]
                                nc.tensor.matmul(
                                    swp[0:nrow, :], lhs,
                                    w16[:, c * H + i * HH:c * H + (i + 1) * HH],
                                    start=(c == 0), stop=(c == HC - 1))
                            nc.vector.tensor_copy(
                                sW[0:nrow, tb * H + i * HH:tb * H + (i + 1) * HH],
                                swp[0:nrow, :])

            if NR < 128:
                nc.vector.memset(sW[:, :], 0.0)
            do_feat(d_fs, sT, True)
            do_feat(d_fe, qT, False)

            # sK = keys @ s^T  [5, NR]
            for q in range((NR + 511) // 512):
                ncol = min(512, NR - q * 512)
                skp = ptile([128, 512], F32)
                for c in range(HC):
                    nc.tensor.matmul(
                        skp[0:K, 0:ncol], keysT[:, c * 8:c * 8 + K],
                        sT[:, c * NR + q * 512:c * NR + q * 512 + ncol],
                        start=(c == 0), stop=(c == HC - 1))
                nc.vector.tensor_copy(sK[0:K, q * 512:q * 512 + ncol],
                                      skp[0:K, 0:ncol])

            # sKall[p, t] = sK[k(p), t*8 + b(p)]  (gate content bias per row)
            skap = ptile([128, nsteps], F32)
            sKr = sK[:, :].rearrange("p (t b) -> p b t", b=BL)
            for b in range(BL):
                sel = selKb16[0:K, b * RP:(b + 1) * RP]
                src = sKr[0:K, b:b + 1, 0:nsteps]
                nc.tensor.matmul(skap[0:RP, :], sel, src,
                                 start=(b == 0), stop=(b == BL - 1),
                                 skip_group_check=True)
                nc.tensor.matmul(skap[64:128, :], sel, src,
                                 start=(b == 0), stop=(b == BL - 1),
                                 skip_group_check=True)
            nc.vector.tensor_copy(sKall[:, :], skap[:, :])

            # hu_0 = h0 = keys broadcast to (k,b) rows (zero into pad rows)
            h0p = ptile([128, HH], F32)
            nc.tensor.matmul(h0p[0:RP, :], selK16[0:K, :], keys16[0:K, 0:HH],
                             start=True, stop=True, skip_group_check=True)
            nc.tensor.matmul(h0p[64:128, :], selK16[0:K, :], keys16[0:K, HH:H],
                             start=True, stop=True, skip_group_check=True)
            hu_cur = p_hu.tile([128, HH], F16, tag="hu")
            nc.vector.tensor_copy(hu_cur[:, :], h0p[:, :])

            tp0 = ptile([128, 3 * 128], F16)
            emit_transposes(hu_cur, tp0)
            hT_cur = p_hT.tile([128, 3 * 128], F16, tag="hT")
            nc.vector.tensor_copy(hT_cur[:, :], tp0[:, :])

        # PSUM pools for the main loop (opened after the prologue pool frees)
        p_zps = ep(tc.tile_pool(name="zps", bufs=2, space="PSUM"))
        p_bps = ep(tc.tile_pool(name="bps", bufs=2, space="PSUM"))
        p_tps = ep(tc.tile_pool(name="tps", bufs=1, space="PSUM"))
        p_sps = ep(tc.tile_pool(name="sps", bufs=1, space="PSUM"))
        p_eps = ep(tc.tile_pool(name="eps", bufs=1, space="PSUM"))

        # rn_0 = 1, h_mat_0 = hu_0
        rn_cur = p_rn.tile([128, 1], F32, tag="rn")
        nc.vector.memset(rn_cur[:, :], 1.0)
        nc.vector.memset(scores[:, :], 0.0)
        hm_cur = p_hm.tile([128, HH], F16, tag="hm")
        nc.vector.tensor_copy(hm_cur[:, :], hu_cur[:, :])

        # stage steps 0 and 1; bias for step 0
        stage(0, 0)
        if nsteps > 1:
            stage(1, 1)
        bP0 = p_bps.tile([128, HH], F32, tag="b")
        emit_bias(bP0, 0)
        b16_cur = p_b16.tile([128, HH], F16, tag="b16")
        nc.scalar.copy(b16_cur[:, :], bP0[:, :])

        # ---- main loop ----
        rn_next = hm_next = None
        for t in range(nsteps):
            par = t % 2
            if t > 0:
                rn_cur, hm_cur = rn_next, hm_next

            # fused z/g/q block
            zP = p_zps.tile([128, FW], F32, tag="z")
            for c in range(HC):
                lhs = hts(hT_cur, c)
                base = par * HC * BLK + c * BLK
                nc.tensor.matmul(zP[0:RP, :], lhs,
                                 rhs2[:, base:base + FW],
                                 start=(c == 0), stop=(c == HC - 1),
                                 skip_group_check=True)
                nc.tensor.matmul(zP[64:128, :], lhs,
                                 rhs2[:, base + FW:base + BLK],
                                 start=(c == 0), stop=(c == HC - 1),
                                 skip_group_check=True)

            # off-path work for t+1
            if t + 2 < nsteps:
                stage(t + 2, t % 2)
            if t + 1 < nsteps:
                bP = p_bps.tile([128, HH], F32, tag="b")
                emit_bias(bP, t + 1)

            # gate: masked reduce + sigmoid(rn*gpre + s.keys)
            gsc = p_sml.tile([128, BL], F16, tag="gsc")
            gpre = p_sml.tile([128, 1], F32, tag="gpre")
            mask_reduce(gsc[:, :], zP[:, HH:HH + 8], maskG[:, :],
                        gpre[:, :])
            gsig = p_sml.tile([128, 1], F32, tag="gsig")
            nc.scalar.activation(gsig[:, :], gpre[:, :], ACTF.Sigmoid,
                                 bias=sKall[:, t:t + 1], scale=rn_cur[:, :])

            # z = rn*zU + bias; cand = g*prelu(z)
            zs16 = p_e16.tile([128, HH], F16, tag="zs16")
            nc.vector.tensor_scalar(zs16[:, :], zP[:, 0:HH], rn_cur[:, :],
                                    None, ALU.mult)
            z16 = p_e16.tile([128, HH], F16, tag="z16")
            nc.vector.tensor_add(z16[:, :], zs16[:, :], b16_cur[:, :])
            cand = p_e16.tile([128, HH], F16, tag="cand")
            hu_new = p_hu.tile([128, HH], F16, tag="hu")
            if use_prelu:
                nc.scalar.activation(cand[:, :], z16[:, :], ACTF.Prelu,
                                     scale=gsig[:, :], alpha=alphav[:, :])
                # hu_{t+1} = h_t + cand
                nc.vector.tensor_add(hu_new[:, :], hm_cur[:, :], cand[:, :])
            else:
                # sim fallback: g*prelu(z) = g(1-a)/2*|z| + g(1+a)/2*z
                ca = p_sml.tile([128, 1], F32, tag="ca")
                cb = p_sml.tile([128, 1], F32, tag="cb")
                nc.vector.tensor_mul(ca[:, :], gsig[:, :], av_n[:, :])
                nc.vector.tensor_mul(cb[:, :], gsig[:, :], av_p[:, :])
                nc.scalar.activation(cand[:, :], z16[:, :], ACTF.Abs,
                                     scale=ca[:, :])
                bv = p_e16.tile([128, HH], F16, tag="bv")
                nc.vector.tensor_scalar(bv[:, :], z16[:, :], cb[:, :], None,
                                        ALU.mult)
                nc.vector.tensor_add(hu_new[:, :], hm_cur[:, :], cand[:, :])
                nc.vector.tensor_add(hu_new[:, :], hu_new[:, :], bv[:, :])

            # transposes -> huT_{t+1}
            tP = p_tps.tile([128, 3 * 128], F16, tag="t")
            emit_transposes(hu_new, tP)
            hT_new = p_hT.tile([128, 3 * 128], F16, tag="hT")
            nc.vector.tensor_copy(hT_new[:, :], tP[:, :])

            # norm accumulators for rn_{t+1}
            squ = p_e16.tile([128, HH], F16, tag="squ")
            ss_n = p_sml.tile([128, 1], F32, tag="ss")
            nc.scalar.activation(squ[:, :], hu_new[:, :], ACTF.Square,
                                 accum_out=ss_n[:, :])
            ss16_n = p_sml.tile([128, 1], F16, tag="ss16")
            pool.tensor_copy(ss16_n[:, :], ss_n[:, :])

            # rn_{t+1} = rsqrt(fold(ss)): PE fold, DVE fast-inv-sqrt + 1 NR
            ssp = p_sps.tile([128, 1], F32, tag="ssp")
            nc.tensor.matmul(ssp[:, :], P64h[:, :], ss16_n[:, :],
                             start=True, stop=True)
            sdi = p_sml.tile([128, 1], I32, tag="sdi")
            nc.vector.tensor_scalar(sdi[:, :], ssp.bitcast(I32)[:, :], 1,
                                    None, ALU.logical_shift_right)
            nc.vector.tensor_scalar(sdi[:, :], sdi[:, :], -1, 0x5F3759DF,
                                    ALU.mult, ALU.add)
            rn_w = sdi.bitcast(F32)
            ra = p_sml.tile([128, 1], F32, tag="ra")
            rn_next = p_rn.tile([128, 1], F32, tag="rn")
            for it in range(newton_iters):
                nc.vector.tensor_mul(ra[:, :], rn_w[:, :], ssp[:, :])
                nc.vector.tensor_mul(ra[:, :], ra[:, :], rn_w[:, :])
                nc.vector.tensor_scalar(ra[:, :], ra[:, :], -0.5, 1.5,
                                        ALU.mult, ALU.add)
                dst = rn_next if it == newton_iters - 1 else rn_w
                nc.vector.tensor_mul(dst[:, :], rn_w[:, :], ra[:, :])
            hm_next = p_hm.tile([128, HH], F16, tag="hm")
            nc.vector.tensor_scalar(hm_next[:, :], hu_new[:, :],
                                    rn_next[:, :], None, ALU.mult)

            # bias16 for t+1 (ACT does the PSUM read; Pool has no PSUM port)
            if t + 1 < nsteps:
                b16_new = p_b16.tile([128, HH], F16, tag="b16")
                nc.scalar.copy(b16_new[:, :], bP[:, :])
            else:
                b16_new = b16_cur

            # score for step t-1: rn_t * masked-reduce(q-gram cols)
            if t > 0:
                qsc = p_sml.tile([128, BL], F16, tag="qsc")
                qtmp = p_sml.tile([128, 1], F32, tag="qtmp")
                mask_reduce(qsc[:, :], zP[:, HH + 8:FW], maskG[:, :],
                            qtmp[:, :])
                pool.tensor_scalar(scores[:, t - 1:t], qtmp[:, :],
                                        rn_cur[:, :], None, ALU.mult)

            if debug:
                dma(bass.AP(d_dhu, t * 128 * HH, [[HH, 128], [1, HH]]),
                    hu_new[:, :])
                dma(bass.AP(d_drn, t * 128, [[1, 128], [1, 1]]), rn_cur[:, :])
                dma(bass.AP(d_dg, t * 128, [[1, 128], [1, 1]]), gsig[:, :])
                dma(bass.AP(d_dz, t * 128 * HH, [[HH, 128], [1, HH]]),
                    z16[:, :])

            hu_cur, hT_cur, b16_cur = hu_new, hT_new, b16_new

        # ---- epilogue ----
        rn_T = rn_next

        # final q-gram: q_{T-1} . hu_T
        qPf = p_eps.tile([128, BL], F32, tag="qf")
        for c in range(HC):
            nc.tensor.matmul(
                qPf[0:RP, 0:BL], hts(hT_cur, c),
                qT[:, c * NR + (nsteps - 1) * 8:c * NR + nsteps * 8],
                start=(c == 0), stop=(c == HC - 1))
        qsc = p_sml.tile([128, BL], F16, tag="qsc")
        qtmp = p_sml.tile([128, 1], F32, tag="qtmp")
        mask_reduce(qsc[0:RP, :], qPf[0:RP, 0:BL], maskG[0:RP, :],
                    qtmp[0:RP, :])
        pool.tensor_scalar(scores[0:RP, nsteps - 1:nsteps],
                                qtmp[0:RP, :], rn_T[0:RP, :], None, ALU.mult)

        # output head
        pP = p_eps.tile([128, nsteps], F32, tag="pp")
        nc.tensor.matmul(pP[0:BL * L, :], Wsel[0:R, 0:BL * L],
                         scores[0:R, 0:nsteps], start=True, stop=True)
        osb = p_prm.tile([128, nsteps], F32, tag="osb")
        nc.vector.tensor_scalar(osb[0:BL * L, :], pP[0:BL * L, :],
                                bvec[0:BL * L, :], None, ALU.add)
        nc.sync.dma_start(bass.AP(d_out, 0, [[1, BL * L], [BL * L, nsteps]]),
                          osb[0:BL * L, :])
        if debug:
            dma(bass.AP(d_dsc, 0, [[nsteps, 128], [1, nsteps]]),
                scores[:, :])

    nc.compile()
    return nc


_CACHE = {}


def _get(nsteps, debug=False, use_prelu=True, **kw):
    key = (nsteps, debug, use_prelu) + tuple(sorted(kw.items()))
    if key not in _CACHE:
        _CACHE[key] = _build(nsteps, debug=debug, use_prelu=use_prelu, **kw)
    return _CACHE[key]


def _in_maps(inputs, nsteps):
    consts = _host_consts()
    fs = np.ascontiguousarray(np.asarray(inputs["features_sentence"], dtype=np.float32))
    fe = np.ascontiguousarray(np.asarray(inputs["features_entity"], dtype=np.float32))
    shared = {k: np.ascontiguousarray(np.asarray(inputs[k], dtype=np.float32))
              for k in ("keys", "U", "V", "W", "alpha", "W_out", "b_out")}
    shared.update(consts)
    in_maps = []
    for c in range(NC):
        m = dict(shared)
        m["features_sentence"] = np.ascontiguousarray(fs[:, c * BL:(c + 1) * BL, :])
        m["features_entity"] = np.ascontiguousarray(fe[:, c * BL:(c + 1) * BL, :])
        in_maps.append(m)
    return in_maps


def kernel(**inputs):
    nsteps = inputs["features_sentence"].shape[0]
    nc = _get(nsteps)
    res = run_bass_kernel_spmd(nc, _in_maps(inputs, nsteps),
                               core_ids=list(range(NC)))
    outs = [r["preds"].reshape(nsteps, BL, L) for r in res.results]
    return np.concatenate(outs, axis=1).reshape(nsteps * B, L)


# revision 23
# speedup vs baseline: 1.6449x; 1.0165x over previous
"""EntNetHead Trainium2 kernel (v2: deferred normalization).

Data-parallel over batch B=64 across 8 NeuronCores (8 batch rows per core);
T=256 recurrent steps run on-chip per core.

Key idea vs v1: the per-step L2 normalization is linear, so the state is
kept UNNORMALIZED (hu) together with rn = 1/||hu|| per packed row.  The
matmuls use huT directly; rn folds into (a) the sigmoid's per-partition
scale, (b) one DVE scale of the z PSUM, (c) a lazily-materialized
h = rn*hu used by the state update.  The whole square/fold/rsqrt chain
runs OFF the critical path (ACT/Pool/PE idle slots).

Per-step PE block is 12 matmuls: the g-gram (s_t . hu) and q-gram
(q_{t-1} . hu) are fused into the z matmuls as 16 extra rhs columns,
staged next to the U chunks by the (otherwise idle) DMA engines.  The
gate's content bias s_t.keys is precomputed for all t as a per-partition
column (sKall) and enters through the sigmoid's bias AP. The z bias
keyV + s_t@W accumulates in a separate PSUM bank off the critical path.

Engine split per step:
  PE:   12 fused z/g/q matmuls, 4 bias matmuls (t+1), 3 transposes,
        1 fp16 norm-fold matmul
  DVE:  gate reduce, z-scale, z-add(bias), hu update, hT copy,
        q reduce, rsqrt seed
  ACT:  sigmoid, prelu, square+accum, bias PSUM->SBUF copy
  Pool: Newton rsqrt, h materialize, score scale, ss fp16 cast
  DMA:  per-step staging of sT/qT slices into the fused rhs buffer
"""

import sys

sys.path.insert(0, "/opt/trn_rl_repo")

from contextlib import ExitStack

import numpy as np

import concourse.bacc as bacc
import concourse.bass as bass
import concourse.tile as tile
from concourse import mybir
from concourse.bass_utils import run_bass_kernel_spmd

F32 = mybir.dt.float32
F16 = mybir.dt.float16
I32 = mybir.dt.int32
ALU = mybir.AluOpType
ACTF = mybir.ActivationFunctionType

T, B, H, K, L = 256, 64, 768, 5, 3
NC = 8
BL = B // NC          # 8 batch rows per core
R = K * BL            # 40 (k,b) rows
RP = 64               # padded stationary width
HC = H // 128         # 6 contraction chunks
HH = H // 2           # 384
BLK = 2 * (HH + 16)   # 800: per-chunk fused rhs block [U_h0|sT|qT|U_h1|sT|qT]
FW = HH + 16          # 400: fused matmul width per half


def _host_consts():
    selK = np.zeros((K, RP), np.float16)
    for k in range(K):
        selK[k, k * BL:(k + 1) * BL] = 1.0
    selKb = np.zeros((K, 8 * RP), np.float16)
    for b in range(BL):
        for k in range(K):
            selKb[k, b * RP + k * BL + b] = 1.0
    selB = np.zeros((128, 16 * RP), np.float16)
    for m in range(16):
        for b in range(BL):
            for k in range(K):
                selB[m * BL + b, m * RP + k * BL + b] = 1.0
    maskG = np.zeros((128, BL), np.float16)
    for p in range(128):
        maskG[p, p % BL] = 1.0
    I64 = np.zeros((128, RP), np.float16)
    for j in range(RP):
        I64[j, j] = 1.0
        I64[64 + j, j] = 1.0
    I128 = np.eye(128, dtype=np.float32)
    mask24 = np.zeros((R, BL * L), np.float32)
    for k in range(K):
        for b in range(BL):
            mask24[k * BL + b, b * L:(b + 1) * L] = 1.0
    ones1x128 = np.ones((1, 128), np.float32)
    P64h = np.zeros((128, 128), np.float16)
    for i in range(128):
        P64h[i, i % 64] = 1.0
        P64h[i, i % 64 + 64] = 1.0
    selK32 = selK.astype(np.float32)
    return {
        "c_selK": selK, "c_selKb": selKb, "c_selB32": selB, "c_maskG": maskG,
        "c_I64": I64, "c_I128": I128, "c_selK32": selK32,
        "c_mask24": mask24, "c_ones": ones1x128, "c_P64h": P64h,
        "c_I128h": np.eye(128, dtype=np.float16),
    }


def _build(nsteps, debug=False, use_prelu=True,
           stage_dma=True, use_pool=True, use_ttr=False, newton_iters=1,
           alpha_const=None):
    # use_ttr: InstTensorTensorReduce wedges TRN2 here (NRT_EXEC_UNIT_
    # UNRECOVERABLE) despite passing CoreSim -- keep the 2-op fallback.
    nc = bacc.Bacc("TRN2", target_bir_lowering=False, debug=False)
    NR = nsteps * BL      # feature rows per core
    ntt = (NR + 127) // 128

    d_fs = nc.dram_tensor("features_sentence", [nsteps, BL, H], F32, kind="ExternalInput")
    d_fe = nc.dram_tensor("features_entity", [nsteps, BL, H], F32, kind="ExternalInput")
    d_keys = nc.dram_tensor("keys", [K, H], F32, kind="ExternalInput")
    d_U = nc.dram_tensor("U", [H, H], F32, kind="ExternalInput")
    d_V = nc.dram_tensor("V", [H, H], F32, kind="ExternalInput")
    d_W = nc.dram_tensor("W", [H, H], F32, kind="ExternalInput")
    d_alpha = nc.dram_tensor("alpha", [1], F32, kind="ExternalInput")
    d_Wout = nc.dram_tensor("W_out", [K, L], F32, kind="ExternalInput")
    d_bout = nc.dram_tensor("b_out", [L], F32, kind="ExternalInput")
    d_selK = nc.dram_tensor("c_selK", [K, RP], F16, kind="ExternalInput")
    d_selKb = nc.dram_tensor("c_selKb", [K, 8 * RP], F16, kind="ExternalInput")
    d_selB32 = nc.dram_tensor("c_selB32", [128, 16 * RP], F16, kind="ExternalInput")
    d_maskG = nc.dram_tensor("c_maskG", [128, BL], F16, kind="ExternalInput")
    d_I64 = nc.dram_tensor("c_I64", [128, RP], F16, kind="ExternalInput")
    d_I128 = nc.dram_tensor("c_I128", [128, 128], F32, kind="ExternalInput")
    d_selK32 = nc.dram_tensor("c_selK32", [K, RP], F32, kind="ExternalInput")
    d_mask24 = nc.dram_tensor("c_mask24", [R, BL * L], F32, kind="ExternalInput")
    d_ones = nc.dram_tensor("c_ones", [1, 128], F32, kind="ExternalInput")
    d_P64h = nc.dram_tensor("c_P64h", [128, 128], F16, kind="ExternalInput")
    d_I128h = nc.dram_tensor("c_I128h", [128, 128], F16, kind="ExternalInput")
    d_out = nc.dram_tensor("preds", [NR, L], F32, kind="ExternalOutput")
    if debug:
        d_dhu = nc.dram_tensor("dbg_hu", [nsteps + 1, 128, HH], F16, kind="ExternalOutput")
        d_drn = nc.dram_tensor("dbg_rn", [nsteps + 1, 128], F32, kind="ExternalOutput")
        d_dg = nc.dram_tensor("dbg_g", [nsteps, 128], F32, kind="ExternalOutput")
        d_dz = nc.dram_tensor("dbg_z", [nsteps, 128, HH], F16, kind="ExternalOutput")
        d_dsc = nc.dram_tensor("dbg_sc", [128, nsteps], F32, kind="ExternalOutput")

    with tile.TileContext(nc) as tc, ExitStack() as ctx:
        ep = ctx.enter_context
        pool = nc.gpsimd if use_pool else nc.vector

        p_sT = ep(tc.tile_pool(name="sT", bufs=1))
        p_qT = ep(tc.tile_pool(name="qT", bufs=1))
        p_sW = ep(tc.tile_pool(name="sW", bufs=1))
        p_rhs = ep(tc.tile_pool(name="rhs", bufs=1))
        p_prm = ep(tc.tile_pool(name="prm", bufs=1))
        p_hu = ep(tc.tile_pool(name="hu", bufs=2))
        p_hm = ep(tc.tile_pool(name="hm", bufs=2))
        p_hT = ep(tc.tile_pool(name="hT", bufs=2))
        p_rn = ep(tc.tile_pool(name="rn", bufs=2))
        p_b16 = ep(tc.tile_pool(name="b16", bufs=2))
        p_e16 = ep(tc.tile_pool(name="e16", bufs=2))
        p_sml = ep(tc.tile_pool(name="sml", bufs=3))

        sT = p_sT.tile([128, HC * NR], F16)   # [h%128, c*NR + t*8+b]
        qT = p_qT.tile([128, HC * NR], F16)
        sW = p_sW.tile([128, ntt * H], F16)   # [row%128, tile*768+h]
        rhs2 = p_rhs.tile([128, 2 * HC * BLK], F16)  # fused rhs, 2 parities
        keyV = p_prm.tile([128, H], F16, tag="keyV")
        keys16 = p_prm.tile([128, H], F16, tag="keys16")
        keysT = p_prm.tile([128, HC * 8], F16, tag="keysT")
        sK = p_prm.tile([128, NR], F16, tag="sK")
        sKall = p_prm.tile([128, nsteps], F32, tag="sKall")
        selK16 = p_prm.tile([128, RP], F16, tag="selK16")
        selKb16 = p_prm.tile([128, 8 * RP], F16, tag="selKb16")
        selB32 = p_prm.tile([128, 16 * RP], F16, tag="selB32")
        maskG = p_prm.tile([128, BL], F16, tag="maskG")
        I64 = p_prm.tile([128, RP], F16, tag="I64")
        I128 = p_prm.tile([128, 128], F32, tag="I128")
        selK32 = p_prm.tile([128, RP], F32, tag="selK32")
        mask24 = p_prm.tile([128, BL * L], F32, tag="mask24")
        ones_r = p_prm.tile([128, 128], F32, tag="ones_r")
        P64h = p_prm.tile([128, 128], F16, tag="P64h")
        I128h = p_prm.tile([128, 128], F16, tag="I128h")
        alphav = p_prm.tile([128, 1], F32, tag="alphav")
        av_n = p_prm.tile([128, 1], F32, tag="av_n")
        av_p = p_prm.tile([128, 1], F32, tag="av_p")
        Wsel = p_prm.tile([128, BL * L], F32, tag="Wsel")
        bvec = p_prm.tile([128, 1], F32, tag="bvec")
        scores = p_prm.tile([128, nsteps], F32, tag="scores")

        dma = nc.sync.dma_start

        dma(selK16[0:K, :], d_selK.ap())
        dma(selKb16[0:K, :], d_selKb.ap())
        dma(selB32[:, :], d_selB32.ap())
        dma(maskG[:, :], d_maskG.ap())
        dma(I64[:, :], d_I64.ap())
        dma(I128[:, :], d_I128.ap())
        dma(selK32[0:K, :], d_selK32.ap())
        dma(mask24[0:R, :], d_mask24.ap())
        dma(ones_r[0:1, :], d_ones.ap())
        dma(P64h[:, :], d_P64h.ap())
        dma(I128h[:, :], d_I128h.ap())
        for b in range(BL):
            dma(bvec[b * L:(b + 1) * L, 0:1], bass.AP(d_bout, 0, [[1, L], [1, 1]]))

        def hts(hT_tile, c):
            if c < 3:
                return hT_tile[:, c * 128:c * 128 + RP]
            return hT_tile[:, (c - 3) * 128 + RP:(c - 2) * 128]

        def emit_transposes(h_tile, out_psum):
            for cc in range(3):
                nc.tensor.transpose(out_psum[:, cc * 128:(cc + 1) * 128],
                                    h_tile[:, cc * 128:(cc + 1) * 128],
                                    I128h[:, :])

        def stage(t, par):
            """Copy sT_t and qT_{max(t-1,0)} slices into rhs2 parity par."""
            tq = max(t - 1, 0)
            mv = dma if stage_dma else pool.tensor_copy
            for i in range(2):
                mv(bass.AP(rhs2.tensor,
                           rhs2.offset + par * HC * BLK + i * FW + HH,
                           [[2 * HC * BLK, 128], [BLK, HC], [1, 8]]),
                   bass.AP(sT.tensor, sT.offset + t * 8,
                           [[HC * NR, 128], [NR, HC], [1, 8]]))
                mv(bass.AP(rhs2.tensor,
                           rhs2.offset + par * HC * BLK + i * FW + HH + 8,
                           [[2 * HC * BLK, 128], [BLK, HC], [1, 8]]),
                   bass.AP(qT.tensor, qT.offset + tq * 8,
                           [[HC * NR, 128], [NR, HC], [1, 8]]))

        def mask_reduce(out_sc, in_ap, msk, accum, eng=None):
            eng = eng or nc.vector
            if use_ttr:
                nc.vector.tensor_tensor_reduce(
                    out_sc, in_ap, msk, 1.0, 0.0, ALU.mult, ALU.add, accum)
            else:
                eng.tensor_mul(out_sc, in_ap, msk)
                eng.tensor_reduce(accum, out_sc,
                                  mybir.AxisListType.X, ALU.add)

        def emit_bias(bP, tn):
            """biasP for step tn: keyV broadcast + s_tn @ W broadcast."""
            m, tb = tn % 16, (tn * BL) // 128
            nc.tensor.matmul(bP[0:RP, :], selK16[0:K, :], keyV[0:K, 0:HH],
                             start=True, stop=False, skip_group_check=True)
            nc.tensor.matmul(bP[64:128, :], selK16[0:K, :], keyV[0:K, HH:H],
                             start=True, stop=False, skip_group_check=True)
            lsel = selB32[:, m * RP:(m + 1) * RP]
            nc.tensor.matmul(bP[0:RP, :], lsel, sW[:, tb * H:tb * H + HH],
                             start=False, stop=True, skip_group_check=True)
            nc.tensor.matmul(bP[64:128, :], lsel, sW[:, tb * H + HH:(tb + 1) * H],
                             start=False, stop=True, skip_group_check=True)

        with tc.tile_pool(name="pr32", bufs=2) as p32, \
             tc.tile_pool(name="prps", bufs=4, space="PSUM") as pps:

            def ptile(shape, dt):
                return pps.tile(shape, dt, tag="ps", name="ps")

            # keys
            kn = p32.tile([128, H], F32, tag="kn")
            dma(kn[0:K, :], d_keys.ap())
            nc.vector.tensor_copy(keys16[0:K, :], kn[0:K, :])
            # U (chunked fp32 load -> cast-copied into both parities of rhs2)
            for c in range(HC):
                u1 = p32.tile([128, H], F32, tag="u1")
                dma(u1[:, :], d_U.ap()[c * 128:(c + 1) * 128, :])
                for par in range(2):
                    for i in range(2):
                        dst = bass.AP(
                            rhs2.tensor,
                            rhs2.offset + par * HC * BLK + c * BLK + i * FW,
                            [[2 * HC * BLK, 128], [1, HH]])
                        nc.vector.tensor_copy(dst, u1[:, i * HH:(i + 1) * HH])
            # keysT via PE transpose
            tps_ = ptile([128, HC * 8], F16)
            for c in range(HC):
                nc.tensor.transpose(tps_[:, c * 8:c * 8 + K],
                                    keys16[0:K, c * 128:(c + 1) * 128],
                                    I64[0:K, 0:K])
            for c in range(HC):
                nc.vector.tensor_copy(keysT[:, c * 8:c * 8 + K],
                                      tps_[:, c * 8:c * 8 + K])
            # V -> keyV = keys @ V  (chunked)
            kvp0 = ptile([128, HH], F32)
            kvp1 = ptile([128, HH], F32)
            for c in range(HC):
                v1 = p32.tile([128, H], F32, tag="u1")
                dma(v1[:, :], d_V.ap()[c * 128:(c + 1) * 128, :])
                v16c = p32.tile([128, H], F16, tag="v16c")
                nc.vector.tensor_copy(v16c[:, :], v1[:, :])
                nc.tensor.matmul(kvp0[0:K, :], keysT[:, c * 8:c * 8 + K],
                                 v16c[:, 0:HH],
                                 start=(c == 0), stop=(c == HC - 1))
                nc.tensor.matmul(kvp1[0:K, :], keysT[:, c * 8:c * 8 + K],
                                 v16c[:, HH:H],
                                 start=(c == 0), stop=(c == HC - 1))
            nc.vector.tensor_copy(keyV[0:K, 0:HH], kvp0[0:K, :])
            nc.vector.tensor_copy(keyV[0:K, HH:H], kvp1[0:K, :])
            # W (fp16) for sW matmuls (chunked load)
            w16 = p32.tile([128, HC * H], F16, tag="big16w")
            for c in range(HC):
                w1 = p32.tile([128, H], F32, tag="u1")
                dma(w1[:, :], d_W.ap()[c * 128:(c + 1) * 128, :])
                nc.vector.tensor_copy(w16[:, c * H:(c + 1) * H], w1[:, :])

            # alpha -> all partitions
            asb = p32.tile([128, 1], F32, tag="asb")
            dma(asb[0:1, 0:1], bass.AP(d_alpha, 0, [[1, 1], [1, 1]]))
            alp = ptile([128, 1], F32)
            nc.tensor.matmul(alp[:, :], ones_r[0:1, :], asb[0:1, 0:1],
                             start=True, stop=True)
            nc.vector.tensor_copy(alphav[:, :], alp[:, :])
            nc.vector.tensor_scalar(av_n[:, :], alphav[:, :], -0.5, 0.5,
                                    ALU.mult, ALU.add)
            nc.vector.tensor_scalar(av_p[:, :], alphav[:, :], 0.5, 0.5,
                                    ALU.mult, ALU.add)

            # W_out -> Wsel
            wo = p32.tile([128, L], F32, tag="wo")
            dma(wo[0:K, :], d_Wout.ap())
            wrp = ptile([128, L], F32)
            nc.tensor.matmul(wrp[0:R, :], selK32[0:K, 0:R], wo[0:K, :],
                             start=True, stop=True)
            wrs = p32.tile([128, L], F32, tag="wrs")
            nc.vector.tensor_copy(wrs[0:R, :], wrp[0:R, :])
            for b in range(BL):
                nc.vector.tensor_mul(Wsel[0:R, b * L:(b + 1) * L],
                                     wrs[0:R, :], mask24[0:R, b * L:(b + 1) * L])

            # features: DMA, transpose (and sW for the sentence stream)
            def do_feat(dram, dstT, with_sw):
                for tb in range(ntt):
                    fn = p32.tile([128, H], F32, tag="fnat")
                    nrow = min(128, NR - tb * 128)
                    src = bass.AP(dram, tb * 128 * H, [[H, nrow], [1, H]])
                    dma(fn[0:nrow, :], src)
                    for grp in range(2):
                        tp = ptile([128, 3 * 128], F32)
                        for j in range(3):
                            c = grp * 3 + j
                            nc.tensor.transpose(
                                tp[:, j * nrow:(j + 1) * nrow],
                                fn[0:nrow, c * 128:(c + 1) * 128],
                                I128[0:nrow, 0:nrow])
                        dst = bass.AP(
                            dstT.tensor,
                            dstT.offset + (grp * 3) * NR + tb * 128,
                            [[HC * NR, 128], [NR, 3], [1, nrow]])
                        nc.vector.tensor_copy(
                            dst,
                            tp[:, 0:3 * nrow].rearrange("p (a b) -> p a b", a=3))
                    if with_sw:
                        for i in range(2):
                            swp = ptile([128, HH], F32)
                            for c in range(HC):
                                lhs = dstT[:, c * NR + tb * 128:
                                           c * NR + tb * 128 + nrow]
                                nc.tensor.matmul(
                                    swp[0:nrow, :], lhs,
                                    w16[:, c * H + i * HH:c * H + (i + 1) * HH],
                                    start=(c == 0), stop=(c == HC - 1))
                            nc.vector.tensor_copy(
                                sW[0:nrow, tb * H + i * HH:tb * H + (i + 1) * HH],
                                swp[0:nrow, :])

            if NR < 128:
                nc.vector.memset(sW[:, :], 0.0)
            do_feat(d_fs, sT, True)
            do_feat(d_fe, qT, False)

            # sK = keys @ s^T  [5, NR]
            for q in range((NR + 511) // 512):
                ncol = min(512, NR - q * 512)
                skp = ptile([128, 512], F32)
                for c in range(HC):
                    nc.tensor.matmul(
                        skp[0:K, 0:ncol], keysT[:, c * 8:c * 8 + K],
                        sT[:, c * NR + q * 512:c * NR + q * 512 + ncol],
                        start=(c == 0), stop=(c == HC - 1))
                nc.vector.tensor_copy(sK[0:K, q * 512:q * 512 + ncol],
                                      skp[0:K, 0:ncol])

            # sKall[p, t] = sK[k(p), t*8 + b(p)]  (gate content bias per row)
            skap = ptile([128, nsteps], F32)
            sKr = sK[:, :].rearrange("p (t b) -> p b t", b=BL)
            for b in range(BL):
                sel = selKb16[0:K, b * RP:(b + 1) * RP]
                src = sKr[0:K, b:b + 1, 0:nsteps]
                nc.tensor.matmul(skap[0:RP, :], sel, src,
                                 start=(b == 0), stop=(b == BL - 1),
                                 skip_group_check=True)
                nc.tensor.matmul(skap[64:128, :], sel, src,
                                 start=(b == 0), stop=(b == BL - 1),
                                 skip_group_check=True)
            nc.vector.tensor_copy(sKall[:, :], skap[:, :])

            # hu_0 = h0 = keys broadcast to (k,b) rows (zero into pad rows)
            h0p = ptile([128, HH], F32)
            nc.tensor.matmul(h0p[0:RP, :], selK16[0:K, :], keys16[0:K, 0:HH],
                             start=True, stop=True, skip_group_check=True)
            nc.tensor.matmul(h0p[64:128, :], selK16[0:K, :], keys16[0:K, HH:H],
                             start=True, stop=True, skip_group_check=True)
            hu_cur = p_hu.tile([128, HH], F16, tag="hu")
            nc.vector.tensor_copy(hu_cur[:, :], h0p[:, :])

            tp0 = ptile([128, 3 * 128], F16)
            emit_transposes(hu_cur, tp0)
            hT_cur = p_hT.tile([128, 3 * 128], F16, tag="hT")
            nc.vector.tensor_copy(hT_cur[:, :], tp0[:, :])

        # PSUM pools for the main loop (opened after the prologue pool frees)
        p_zps = ep(tc.tile_pool(name="zps", bufs=2, space="PSUM"))
        p_bps = ep(tc.tile_pool(name="bps", bufs=2, space="PSUM"))
        p_tps = ep(tc.tile_pool(name="tps", bufs=1, space="PSUM"))
        p_sps = ep(tc.tile_pool(name="sps", bufs=1, space="PSUM"))
        p_eps = ep(tc.tile_pool(name="eps", bufs=1, space="PSUM"))

        # rn_0 = 1, h_mat_0 = hu_0
        rn_cur = p_rn.tile([128, 1], F32, tag="rn")
        nc.vector.memset(rn_cur[:, :], 1.0)
        nc.vector.memset(scores[:, :], 0.0)
        hm_cur = p_hm.tile([128, HH], F16, tag="hm")
        nc.vector.tensor_copy(hm_cur[:, :], hu_cur[:, :])

        # stage steps 0 and 1; bias for step 0
        stage(0, 0)
        if nsteps > 1:
            stage(1, 1)
        bP0 = p_bps.tile([128, HH], F32, tag="b")
        emit_bias(bP0, 0)
        b16_cur = p_b16.tile([128, HH], F16, tag="b16")
        nc.scalar.copy(b16_cur[:, :], bP0[:, :])

        # ---- main loop ----
        rn_next = hm_next = None
        for t in range(nsteps):
            par = t % 2
            if t > 0:
                rn_cur, hm_cur = rn_next, hm_next

            # bias matmuls for t+1 run in the post-block PE idle window
            if t + 1 < nsteps:
                bP = p_bps.tile([128, HH], F32, tag="b")
                emit_bias(bP, t + 1)

            # fused z/g/q block
            zP = p_zps.tile([128, FW], F32, tag="z")
            for c in range(HC):
                lhs = hts(hT_cur, c)
                base = par * HC * BLK + c * BLK
                nc.tensor.matmul(zP[0:RP, :], lhs,
                                 rhs2[:, base:base + FW],
                                 start=(c == 0), stop=(c == HC - 1),
                                 skip_group_check=True)
                nc.tensor.matmul(zP[64:128, :], lhs,
                                 rhs2[:, base + FW:base + BLK],
                                 start=(c == 0), stop=(c == HC - 1),
                                 skip_group_check=True)

            # off-path work for t+1
            if t + 2 < nsteps:
                stage(t + 2, t % 2)

            # gate: masked reduce + sigmoid(rn*gpre + s.keys)
            gsc = p_sml.tile([128, BL], F16, tag="gsc")
            gpre = p_sml.tile([128, 1], F32, tag="gpre")
            mask_reduce(gsc[:, :], zP[:, HH:HH + 8], maskG[:, :],
                        gpre[:, :])
            gsig = p_sml.tile([128, 1], F32, tag="gsig")
            nc.scalar.activation(gsig[:, :], gpre[:, :], ACTF.Sigmoid,
                                 bias=sKall[:, t:t + 1], scale=rn_cur[:, :])

            # z = rn*zU + bias; cand = g*prelu(z)
            zs16 = p_e16.tile([128, HH], F16, tag="zs16")
            nc.vector.tensor_scalar(zs16[:, :], zP[:, 0:HH], rn_cur[:, :],
                                    None, ALU.mult)
            z16 = p_e16.tile([128, HH], F16, tag="z16")
            nc.vector.tensor_add(z16[:, :], zs16[:, :], b16_cur[:, :])
            cand = p_e16.tile([128, HH], F16, tag="cand")
            hu_new = p_hu.tile([128, HH], F16, tag="hu")
            if use_prelu:
                al = alphav[:, :] if alpha_const is None else float(alpha_const)
                nc.scalar.activation(cand[:, :], z16[:, :], ACTF.Prelu,
                                     scale=gsig[:, :], alpha=al)
                # hu_{t+1} = h_t + cand
                nc.vector.tensor_add(hu_new[:, :], hm_cur[:, :], cand[:, :])
            else:
                # sim fallback: g*prelu(z) = g(1-a)/2*|z| + g(1+a)/2*z
                ca = p_sml.tile([128, 1], F32, tag="ca")
                cb = p_sml.tile([128, 1], F32, tag="cb")
                nc.vector.tensor_mul(ca[:, :], gsig[:, :], av_n[:, :])
                nc.vector.tensor_mul(cb[:, :], gsig[:, :], av_p[:, :])
                nc.scalar.activation(cand[:, :], z16[:, :], ACTF.Abs,
                                     scale=ca[:, :])
                bv = p_e16.tile([128, HH], F16, tag="bv")
                nc.vector.tensor_scalar(bv[:, :], z16[:, :], cb[:, :], None,
                                        ALU.mult)
                nc.vector.tensor_add(hu_new[:, :], hm_cur[:, :], cand[:, :])
                nc.vector.tensor_add(hu_new[:, :], hu_new[:, :], bv[:, :])

            # transposes -> huT_{t+1}
            tP = p_tps.tile([128, 3 * 128], F16, tag="t")
            emit_transposes(hu_new, tP)
            hT_new = p_hT.tile([128, 3 * 128], F16, tag="hT")
            nc.vector.tensor_copy(hT_new[:, :], tP[:, :])

            # norm accumulators for rn_{t+1}
            squ = p_e16.tile([128, HH], F16, tag="squ")
            ss_n = p_sml.tile([128, 1], F32, tag="ss")
            nc.scalar.activation(squ[:, :], hu_new[:, :], ACTF.Square,
                                 accum_out=ss_n[:, :])
            ss16_n = p_sml.tile([128, 1], F16, tag="ss16")
            pool.tensor_copy(ss16_n[:, :], ss_n[:, :])

            # rn_{t+1} = rsqrt(fold(ss)): PE fold, DVE fast-inv-sqrt + 1 NR
            ssp = p_sps.tile([128, 1], F32, tag="ssp")
            nc.tensor.matmul(ssp[:, :], P64h[:, :], ss16_n[:, :],
                             start=True, stop=True)
            sdi = p_sml.tile([128, 1], I32, tag="sdi")
            nc.vector.tensor_scalar(sdi[:, :], ssp.bitcast(I32)[:, :], 1,
                                    None, ALU.logical_shift_right)
            nc.vector.tensor_scalar(sdi[:, :], sdi[:, :], -1, 0x5F3759DF,
                                    ALU.mult, ALU.add)
            rn_w = sdi.bitcast(F32)
            ra = p_sml.tile([128, 1], F32, tag="ra")
            rn_next = p_rn.tile([128, 1], F32, tag="rn")
            for it in range(newton_iters):
                nc.vector.tensor_mul(ra[:, :], rn_w[:, :], ssp[:, :])
                nc.vector.tensor_mul(ra[:, :], ra[:, :], rn_w[:, :])
                nc.vector.tensor_scalar(ra[:, :], ra[:, :], -0.5, 1.5,
                                        ALU.mult, ALU.add)
                dst = rn_next if it == newton_iters - 1 else rn_w
                nc.vector.tensor_mul(dst[:, :], rn_w[:, :], ra[:, :])
            hm_next = p_hm.tile([128, HH], F16, tag="hm")
            nc.vector.tensor_scalar(hm_next[:, :], hu_new[:, :],
                                    rn_next[:, :], None, ALU.mult)

            # bias16 for t+1 (ACT does the PSUM read; Pool has no PSUM port)
            if t + 1 < nsteps:
                b16_new = p_b16.tile([128, HH], F16, tag="b16")
                nc.scalar.copy(b16_new[:, :], bP[:, :])
            else:
                b16_new = b16_cur

            # score for step t-1: rn_t * masked-reduce(q-gram cols)
            if t > 0:
                qsc = p_sml.tile([128, BL], F16, tag="qsc")
                qtmp = p_sml.tile([128, 1], F32, tag="qtmp")
                mask_reduce(qsc[:, :], zP[:, HH + 8:FW], maskG[:, :],
                            qtmp[:, :])
                pool.tensor_scalar(scores[:, t - 1:t], qtmp[:, :],
                                        rn_cur[:, :], None, ALU.mult)

            if debug:
                dma(bass.AP(d_dhu, t * 128 * HH, [[HH, 128], [1, HH]]),
                    hu_new[:, :])
                dma(bass.AP(d_drn, t * 128, [[1, 128], [1, 1]]), rn_cur[:, :])
                dma(bass.AP(d_dg, t * 128, [[1, 128], [1, 1]]), gsig[:, :])
                dma(bass.AP(d_dz, t * 128 * HH, [[HH, 128], [1, HH]]),
                    z16[:, :])

            hu_cur, hT_cur, b16_cur = hu_new, hT_new, b16_new

        # ---- epilogue ----
        rn_T = rn_next

        # final q-gram: q_{T-1} . hu_T
        qPf = p_eps.tile([128, BL], F32, tag="qf")
        for c in range(HC):
            nc.tensor.matmul(
                qPf[0:RP, 0:BL], hts(hT_cur, c),
                qT[:, c * NR + (nsteps - 1) * 8:c * NR + nsteps * 8],
                start=(c == 0), stop=(c == HC - 1))
        qsc = p_sml.tile([128, BL], F16, tag="qsc")
        qtmp = p_sml.tile([128, 1], F32, tag="qtmp")
        mask_reduce(qsc[0:RP, :], qPf[0:RP, 0:BL], maskG[0:RP, :],
                    qtmp[0:RP, :])
        pool.tensor_scalar(scores[0:RP, nsteps - 1:nsteps],
                                qtmp[0:RP, :], rn_T[0:RP, :], None, ALU.mult)

        # output head
        pP = p_eps.tile([128, nsteps], F32, tag="pp")
        nc.tensor.matmul(pP[0:BL * L, :], Wsel[0:R, 0:BL * L],
                         scores[0:R, 0:nsteps], start=True, stop=True)
        osb = p_prm.tile([128, nsteps], F32, tag="osb")
        nc.vector.tensor_scalar(osb[0:BL * L, :], pP[0:BL * L, :],
                                bvec[0:BL * L, :], None, ALU.add)
        nc.sync.dma_start(bass.AP(d_out, 0, [[1, BL * L], [BL * L, nsteps]]),
                          osb[0:BL * L, :])
        if debug:
            dma(bass.AP(d_dsc, 0, [[nsteps, 128], [1, nsteps]]),
                scores[:, :])

    nc.compile()
    return nc


_CACHE = {}


def _get(nsteps, debug=False, use_prelu=True, **kw):
    key = (nsteps, debug, use_prelu) + tuple(sorted(kw.items()))
    if key not in _CACHE:
        _CACHE[key] = _build(nsteps, debug=debug, use_prelu=use_prelu, **kw)
    return _CACHE[key]


def _in_maps(inputs, nsteps):
    consts = _host_consts()
    fs = np.ascontiguousarray(np.asarray(inputs["features_sentence"], dtype=np.float32))
    fe = np.ascontiguousarray(np.asarray(inputs["features_entity"], dtype=np.float32))
    shared = {k: np.ascontiguousarray(np.asarray(inputs[k], dtype=np.float32))
              for k in ("keys", "U", "V", "W", "alpha", "W_out", "b_out")}
    shared.update(consts)
    in_maps = []
    for c in range(NC):
        m = dict(shared)
        m["features_sentence"] = np.ascontiguousarray(fs[:, c * BL:(c + 1) * BL, :])
        m["features_entity"] = np.ascontiguousarray(fe[:, c * BL:(c + 1) * BL, :])
        in_maps.append(m)
    return in_maps


def kernel(**inputs):
    nsteps = inputs["features_sentence"].shape[0]
    nc = _get(nsteps, alpha_const=float(np.asarray(inputs["alpha"]).ravel()[0]))
    res = run_bass_kernel_spmd(nc, _in_maps(inputs, nsteps),
                               core_ids=list(range(NC)))
    outs = [r["preds"].reshape(nsteps, BL, L) for r in res.results]
    return np.concatenate(outs, axis=1).reshape(nsteps * B, L)


# revision 25
# speedup vs baseline: 1.6496x; 1.0029x over previous
"""EntNetHead Trainium2 kernel (v2: deferred normalization).

Data-parallel over batch B=64 across 8 NeuronCores (8 batch rows per core);
T=256 recurrent steps run on-chip per core.

Key idea vs v1: the per-step L2 normalization is linear, so the state is
kept UNNORMALIZED (hu) together with rn = 1/||hu|| per packed row.  The
matmuls use huT directly; rn folds into (a) the sigmoid's per-partition
scale, (b) one DVE scale of the z PSUM, (c) a lazily-materialized
h = rn*hu used by the state update.  The whole square/fold/rsqrt chain
runs OFF the critical path (ACT/Pool/PE idle slots).

Per-step PE block is 12 matmuls: the g-gram (s_t . hu) and q-gram
(q_{t-1} . hu) are fused into the z matmuls as 16 extra rhs columns,
staged next to the U chunks by the (otherwise idle) DMA engines.  The
gate's content bias s_t.keys is precomputed for all t as a per-partition
column (sKall) and enters through the sigmoid's bias AP. The z bias
keyV + s_t@W accumulates in a separate PSUM bank off the critical path.

Engine split per step:
  PE:   12 fused z/g/q matmuls, 4 bias matmuls (t+1), 3 transposes,
        1 fp16 norm-fold matmul
  DVE:  gate reduce, z-scale, z-add(bias), hu update, hT copy,
        q reduce, rsqrt seed
  ACT:  sigmoid, prelu, square+accum, bias PSUM->SBUF copy
  Pool: Newton rsqrt, h materialize, score scale, ss fp16 cast
  DMA:  per-step staging of sT/qT slices into the fused rhs buffer
"""

import sys

sys.path.insert(0, "/opt/trn_rl_repo")

from contextlib import ExitStack

import numpy as np

import concourse.bacc as bacc
import concourse.bass as bass
import concourse.tile as tile
from concourse import mybir
from concourse.bass_utils import run_bass_kernel_spmd

F32 = mybir.dt.float32
F16 = mybir.dt.float16
I32 = mybir.dt.int32
ALU = mybir.AluOpType
ACTF = mybir.ActivationFunctionType

T, B, H, K, L = 256, 64, 768, 5, 3
NC = 8
BL = B // NC          # 8 batch rows per core
R = K * BL            # 40 (k,b) rows
RP = 64               # padded stationary width
HC = H // 128         # 6 contraction chunks
HH = H // 2           # 384
BLK = 2 * (HH + 16)   # 800: per-chunk fused rhs block [U_h0|sT|qT|U_h1|sT|qT]
FW = HH + 16          # 400: fused matmul width per half


def _host_consts():
    selK = np.zeros((K, RP), np.float16)
    for k in range(K):
        selK[k, k * BL:(k + 1) * BL] = 1.0
    selKb = np.zeros((K, 8 * RP), np.float16)
    for b in range(BL):
        for k in range(K):
            selKb[k, b * RP + k * BL + b] = 1.0
    selB = np.zeros((128, 16 * RP), np.float16)
    for m in range(16):
        for b in range(BL):
            for k in range(K):
                selB[m * BL + b, m * RP + k * BL + b] = 1.0
    maskG = np.zeros((128, BL), np.float16)
    for p in range(128):
        maskG[p, p % BL] = 1.0
    I64 = np.zeros((128, RP), np.float16)
    for j in range(RP):
        I64[j, j] = 1.0
        I64[64 + j, j] = 1.0
    I128 = np.eye(128, dtype=np.float32)
    mask24 = np.zeros((R, BL * L), np.float32)
    for k in range(K):
        for b in range(BL):
            mask24[k * BL + b, b * L:(b + 1) * L] = 1.0
    ones1x128 = np.ones((1, 128), np.float32)
    P64h = np.zeros((128, 128), np.float16)
    for i in range(128):
        P64h[i, i % 64] = 1.0
        P64h[i, i % 64 + 64] = 1.0
    selK32 = selK.astype(np.float32)
    return {
        "c_selK": selK, "c_selKb": selKb, "c_selB32": selB, "c_maskG": maskG,
        "c_I64": I64, "c_I128": I128, "c_selK32": selK32,
        "c_mask24": mask24, "c_ones": ones1x128, "c_P64h": P64h,
        "c_I128h": np.eye(128, dtype=np.float16),
    }


def _build(nsteps, debug=False, use_prelu=True,
           stage_dma=True, use_pool=True, use_ttr=False, newton_iters=1,
           alpha_const=None):
    # use_ttr: InstTensorTensorReduce wedges TRN2 here (NRT_EXEC_UNIT_
    # UNRECOVERABLE) despite passing CoreSim -- keep the 2-op fallback.
    nc = bacc.Bacc("TRN2", target_bir_lowering=False, debug=False)
    NR = nsteps * BL      # feature rows per core
    ntt = (NR + 127) // 128

    d_fs = nc.dram_tensor("features_sentence", [nsteps, BL, H], F32, kind="ExternalInput")
    d_fe = nc.dram_tensor("features_entity", [nsteps, BL, H], F32, kind="ExternalInput")
    d_keys = nc.dram_tensor("keys", [K, H], F32, kind="ExternalInput")
    d_U = nc.dram_tensor("U", [H, H], F32, kind="ExternalInput")
    d_V = nc.dram_tensor("V", [H, H], F32, kind="ExternalInput")
    d_W = nc.dram_tensor("W", [H, H], F32, kind="ExternalInput")
    d_alpha = nc.dram_tensor("alpha", [1], F32, kind="ExternalInput")
    d_Wout = nc.dram_tensor("W_out", [K, L], F32, kind="ExternalInput")
    d_bout = nc.dram_tensor("b_out", [L], F32, kind="ExternalInput")
    d_selK = nc.dram_tensor("c_selK", [K, RP], F16, kind="ExternalInput")
    d_selKb = nc.dram_tensor("c_selKb", [K, 8 * RP], F16, kind="ExternalInput")
    d_selB32 = nc.dram_tensor("c_selB32", [128, 16 * RP], F16, kind="ExternalInput")
    d_maskG = nc.dram_tensor("c_maskG", [128, BL], F16, kind="ExternalInput")
    d_I64 = nc.dram_tensor("c_I64", [128, RP], F16, kind="ExternalInput")
    d_I128 = nc.dram_tensor("c_I128", [128, 128], F32, kind="ExternalInput")
    d_selK32 = nc.dram_tensor("c_selK32", [K, RP], F32, kind="ExternalInput")
    d_mask24 = nc.dram_tensor("c_mask24", [R, BL * L], F32, kind="ExternalInput")
    d_ones = nc.dram_tensor("c_ones", [1, 128], F32, kind="ExternalInput")
    d_P64h = nc.dram_tensor("c_P64h", [128, 128], F16, kind="ExternalInput")
    d_I128h = nc.dram_tensor("c_I128h", [128, 128], F16, kind="ExternalInput")
    d_out = nc.dram_tensor("preds", [NR, L], F32, kind="ExternalOutput")
    if debug:
        d_dhu = nc.dram_tensor("dbg_hu", [nsteps + 1, 128, HH], F16, kind="ExternalOutput")
        d_drn = nc.dram_tensor("dbg_rn", [nsteps + 1, 128], F32, kind="ExternalOutput")
        d_dg = nc.dram_tensor("dbg_g", [nsteps, 128], F32, kind="ExternalOutput")
        d_dz = nc.dram_tensor("dbg_z", [nsteps, 128, HH], F16, kind="ExternalOutput")
        d_dsc = nc.dram_tensor("dbg_sc", [128, nsteps], F32, kind="ExternalOutput")

    with tile.TileContext(nc) as tc, ExitStack() as ctx:
        ep = ctx.enter_context
        pool = nc.gpsimd if use_pool else nc.vector

        p_sT = ep(tc.tile_pool(name="sT", bufs=1))
        p_qT = ep(tc.tile_pool(name="qT", bufs=1))
        p_sW = ep(tc.tile_pool(name="sW", bufs=1))
        p_rhs = ep(tc.tile_pool(name="rhs", bufs=1))
        p_prm = ep(tc.tile_pool(name="prm", bufs=1))
        p_hu = ep(tc.tile_pool(name="hu", bufs=2))
        p_hm = ep(tc.tile_pool(name="hm", bufs=2))
        p_hT = ep(tc.tile_pool(name="hT", bufs=2))
        p_rn = ep(tc.tile_pool(name="rn", bufs=2))
        p_b16 = ep(tc.tile_pool(name="b16", bufs=2))
        p_e16 = ep(tc.tile_pool(name="e16", bufs=2))
        p_sml = ep(tc.tile_pool(name="sml", bufs=3))

        sT = p_sT.tile([128, HC * NR], F16)   # [h%128, c*NR + t*8+b]
        qT = p_qT.tile([128, HC * NR], F16)
        sW = p_sW.tile([128, ntt * H], F16)   # [row%128, tile*768+h]
        rhs2 = p_rhs.tile([128, 2 * HC * BLK], F16)  # fused rhs, 2 parities
        keyV = p_prm.tile([128, H], F16, tag="keyV")
        keys16 = p_prm.tile([128, H], F16, tag="keys16")
        keysT = p_prm.tile([128, HC * 8], F16, tag="keysT")
        sK = p_prm.tile([128, NR], F16, tag="sK")
        sKall = p_prm.tile([128, nsteps], F32, tag="sKall")
        selK16 = p_prm.tile([128, RP], F16, tag="selK16")
        selKb16 = p_prm.tile([128, 8 * RP], F16, tag="selKb16")
        selB32 = p_prm.tile([128, 16 * RP], F16, tag="selB32")
        maskG = p_prm.tile([128, BL], F16, tag="maskG")
        I64 = p_prm.tile([128, RP], F16, tag="I64")
        I128 = p_prm.tile([128, 128], F32, tag="I128")
        selK32 = p_prm.tile([128, RP], F32, tag="selK32")
        mask24 = p_prm.tile([128, BL * L], F32, tag="mask24")
        ones_r = p_prm.tile([128, 128], F32, tag="ones_r")
        P64h = p_prm.tile([128, 128], F16, tag="P64h")
        I128h = p_prm.tile([128, 128], F16, tag="I128h")
        alphav = p_prm.tile([128, 1], F32, tag="alphav")
        av_n = p_prm.tile([128, 1], F32, tag="av_n")
        av_p = p_prm.tile([128, 1], F32, tag="av_p")
        Wsel = p_prm.tile([128, BL * L], F32, tag="Wsel")
        bvec = p_prm.tile([128, 1], F32, tag="bvec")
        scores = p_prm.tile([128, nsteps], F32, tag="scores")

        dma = nc.sync.dma_start

        dma(selK16[0:K, :], d_selK.ap())
        dma(selKb16[0:K, :], d_selKb.ap())
        dma(selB32[:, :], d_selB32.ap())
        dma(maskG[:, :], d_maskG.ap())
        dma(I64[:, :], d_I64.ap())
        dma(I128[:, :], d_I128.ap())
        dma(selK32[0:K, :], d_selK32.ap())
        dma(mask24[0:R, :], d_mask24.ap())
        dma(ones_r[0:1, :], d_ones.ap())
        dma(P64h[:, :], d_P64h.ap())
        dma(I128h[:, :], d_I128h.ap())
        for b in range(BL):
            dma(bvec[b * L:(b + 1) * L, 0:1], bass.AP(d_bout, 0, [[1, L], [1, 1]]))

        def hts(hT_tile, c):
            if c < 3:
                return hT_tile[:, c * 128:c * 128 + RP]
            return hT_tile[:, (c - 3) * 128 + RP:(c - 2) * 128]

        def emit_transposes(h_tile, out_psum):
            for cc in range(3):
                nc.tensor.transpose(out_psum[:, cc * 128:(cc + 1) * 128],
                                    h_tile[:, cc * 128:(cc + 1) * 128],
                                    I128h[:, :])

        def stage(t, par):
            """Copy sT_t and qT_{max(t-1,0)} slices into rhs2 parity par."""
            tq = max(t - 1, 0)
            mv = dma if stage_dma else pool.tensor_copy
            for i in range(2):
                mv(bass.AP(rhs2.tensor,
                           rhs2.offset + par * HC * BLK + i * FW + HH,
                           [[2 * HC * BLK, 128], [BLK, HC], [1, 8]]),
                   bass.AP(sT.tensor, sT.offset + t * 8,
                           [[HC * NR, 128], [NR, HC], [1, 8]]))
                mv(bass.AP(rhs2.tensor,
                           rhs2.offset + par * HC * BLK + i * FW + HH + 8,
                           [[2 * HC * BLK, 128], [BLK, HC], [1, 8]]),
                   bass.AP(qT.tensor, qT.offset + tq * 8,
                           [[HC * NR, 128], [NR, HC], [1, 8]]))

        def mask_reduce(out_sc, in_ap, msk, accum, eng=None):
            eng = eng or nc.vector
            if use_ttr:
                nc.vector.tensor_tensor_reduce(
                    out_sc, in_ap, msk, 1.0, 0.0, ALU.mult, ALU.add, accum)
            else:
                eng.tensor_mul(out_sc, in_ap, msk)
                eng.tensor_reduce(accum, out_sc,
                                  mybir.AxisListType.X, ALU.add)

        def emit_bias(bP, tn):
            """biasP for step tn: keyV broadcast + s_tn @ W broadcast."""
            m, tb = tn % 16, (tn * BL) // 128
            nc.tensor.matmul(bP[0:RP, :], selK16[0:K, :], keyV[0:K, 0:HH],
                             start=True, stop=False, skip_group_check=True)
            nc.tensor.matmul(bP[64:128, :], selK16[0:K, :], keyV[0:K, HH:H],
                             start=True, stop=False, skip_group_check=True)
            lsel = selB32[:, m * RP:(m + 1) * RP]
            nc.tensor.matmul(bP[0:RP, :], lsel, sW[:, tb * H:tb * H + HH],
                             start=False, stop=True, skip_group_check=True)
            nc.tensor.matmul(bP[64:128, :], lsel, sW[:, tb * H + HH:(tb + 1) * H],
                             start=False, stop=True, skip_group_check=True)

        with tc.tile_pool(name="pr32", bufs=2) as p32, \
             tc.tile_pool(name="prps", bufs=4, space="PSUM") as pps:

            def ptile(shape, dt):
                return pps.tile(shape, dt, tag="ps", name="ps")

            # keys
            kn = p32.tile([128, H], F32, tag="kn")
            dma(kn[0:K, :], d_keys.ap())
            nc.vector.tensor_copy(keys16[0:K, :], kn[0:K, :])
            # U (chunked fp32 load -> cast-copied into both parities of rhs2)
            for c in range(HC):
                u1 = p32.tile([128, H], F32, tag="u1")
                dma(u1[:, :], d_U.ap()[c * 128:(c + 1) * 128, :])
                for par in range(2):
                    for i in range(2):
                        dst = bass.AP(
                            rhs2.tensor,
                            rhs2.offset + par * HC * BLK + c * BLK + i * FW,
                            [[2 * HC * BLK, 128], [1, HH]])
                        nc.vector.tensor_copy(dst, u1[:, i * HH:(i + 1) * HH])
            # keysT via PE transpose
            tps_ = ptile([128, HC * 8], F16)
            for c in range(HC):
                nc.tensor.transpose(tps_[:, c * 8:c * 8 + K],
                                    keys16[0:K, c * 128:(c + 1) * 128],
                                    I64[0:K, 0:K])
            for c in range(HC):
                nc.vector.tensor_copy(keysT[:, c * 8:c * 8 + K],
                                      tps_[:, c * 8:c * 8 + K])
            # V -> keyV = keys @ V  (chunked)
            kvp0 = ptile([128, HH], F32)
            kvp1 = ptile([128, HH], F32)
            for c in range(HC):
                v1 = p32.tile([128, H], F32, tag="u1")
                dma(v1[:, :], d_V.ap()[c * 128:(c + 1) * 128, :])
                v16c = p32.tile([128, H], F16, tag="v16c")
                nc.vector.tensor_copy(v16c[:, :], v1[:, :])
                nc.tensor.matmul(kvp0[0:K, :], keysT[:, c * 8:c * 8 + K],
                                 v16c[:, 0:HH],
                                 start=(c == 0), stop=(c == HC - 1))
                nc.tensor.matmul(kvp1[0:K, :], keysT[:, c * 8:c * 8 + K],
                                 v16c[:, HH:H],
                                 start=(c == 0), stop=(c == HC - 1))
            nc.vector.tensor_copy(keyV[0:K, 0:HH], kvp0[0:K, :])
            nc.vector.tensor_copy(keyV[0:K, HH:H], kvp1[0:K, :])
            # W (fp16) for sW matmuls (chunked load)
            w16 = p32.tile([128, HC * H], F16, tag="big16w")
            for c in range(HC):
                w1 = p32.tile([128, H], F32, tag="u1")
                dma(w1[:, :], d_W.ap()[c * 128:(c + 1) * 128, :])
                nc.vector.tensor_copy(w16[:, c * H:(c + 1) * H], w1[:, :])

            # alpha -> all partitions
            asb = p32.tile([128, 1], F32, tag="asb")
            dma(asb[0:1, 0:1], bass.AP(d_alpha, 0, [[1, 1], [1, 1]]))
            alp = ptile([128, 1], F32)
            nc.tensor.matmul(alp[:, :], ones_r[0:1, :], asb[0:1, 0:1],
                             start=True, stop=True)
            nc.vector.tensor_copy(alphav[:, :], alp[:, :])
            nc.vector.tensor_scalar(av_n[:, :], alphav[:, :], -0.5, 0.5,
                                    ALU.mult, ALU.add)
            nc.vector.tensor_scalar(av_p[:, :], alphav[:, :], 0.5, 0.5,
                                    ALU.mult, ALU.add)

            # W_out -> Wsel
            wo = p32.tile([128, L], F32, tag="wo")
            dma(wo[0:K, :], d_Wout.ap())
            wrp = ptile([128, L], F32)
            nc.tensor.matmul(wrp[0:R, :], selK32[0:K, 0:R], wo[0:K, :],
                             start=True, stop=True)
            wrs = p32.tile([128, L], F32, tag="wrs")
            nc.vector.tensor_copy(wrs[0:R, :], wrp[0:R, :])
            for b in range(BL):
                nc.vector.tensor_mul(Wsel[0:R, b * L:(b + 1) * L],
                                     wrs[0:R, :], mask24[0:R, b * L:(b + 1) * L])

            # features: DMA, transpose (and sW for the sentence stream)
            def do_feat(dram, dstT, with_sw):
                for tb in range(ntt):
                    fn = p32.tile([128, H], F32, tag="fnat")
                    nrow = min(128, NR - tb * 128)
                    src = bass.AP(dram, tb * 128 * H, [[H, nrow], [1, H]])
                    dma(fn[0:nrow, :], src)
                    for grp in range(2):
                        tp = ptile([128, 3 * 128], F32)
                        for j in range(3):
                            c = grp * 3 + j
                            nc.tensor.transpose(
                                tp[:, j * nrow:(j + 1) * nrow],
                                fn[0:nrow, c * 128:(c + 1) * 128],
                                I128[0:nrow, 0:nrow])
                        dst = bass.AP(
                            dstT.tensor,
                            dstT.offset + (grp * 3) * NR + tb * 128,
                            [[HC * NR, 128], [NR, 3], [1, nrow]])
                        nc.vector.tensor_copy(
                            dst,
                            tp[:, 0:3 * nrow].rearrange("p (a b) -> p a b", a=3))
                    if with_sw:
                        for i in range(2):
                            swp = ptile([128, HH], F32)
                            for c in range(HC):
                                lhs = dstT[:, c * NR + tb * 128:
                                           c * NR + tb * 128 + nrow]
                                nc.tensor.matmul(
                                    swp[0:nrow, :], lhs,
                                    w16[:, c * H + i * HH:c * H + (i + 1) * HH],
                                    start=(c == 0), stop=(c == HC - 1))
                            nc.vector.tensor_copy(
                                sW[0:nrow, tb * H + i * HH:tb * H + (i + 1) * HH],
                                swp[0:nrow, :])

            if NR < 128:
                nc.vector.memset(sW[:, :], 0.0)
            do_feat(d_fs, sT, True)
            do_feat(d_fe, qT, False)

            # sK = keys @ s^T  [5, NR]
            for q in range((NR + 511) // 512):
                ncol = min(512, NR - q * 512)
                skp = ptile([128, 512], F32)
                for c in range(HC):
                    nc.tensor.matmul(
                        skp[0:K, 0:ncol], keysT[:, c * 8:c * 8 + K],
                        sT[:, c * NR + q * 512:c * NR + q * 512 + ncol],
                        start=(c == 0), stop=(c == HC - 1))
                nc.vector.tensor_copy(sK[0:K, q * 512:q * 512 + ncol],
                                      skp[0:K, 0:ncol])

            # sKall[p, t] = sK[k(p), t*8 + b(p)]  (gate content bias per row)
            skap = ptile([128, nsteps], F32)
            sKr = sK[:, :].rearrange("p (t b) -> p b t", b=BL)
            for b in range(BL):
                sel = selKb16[0:K, b * RP:(b + 1) * RP]
                src = sKr[0:K, b:b + 1, 0:nsteps]
                nc.tensor.matmul(skap[0:RP, :], sel, src,
                                 start=(b == 0), stop=(b == BL - 1),
                                 skip_group_check=True)
                nc.tensor.matmul(skap[64:128, :], sel, src,
                                 start=(b == 0), stop=(b == BL - 1),
                                 skip_group_check=True)
            nc.vector.tensor_copy(sKall[:, :], skap[:, :])

            # hu_0 = h0 = keys broadcast to (k,b) rows (zero into pad rows)
            h0p = ptile([128, HH], F32)
            nc.tensor.matmul(h0p[0:RP, :], selK16[0:K, :], keys16[0:K, 0:HH],
                             start=True, stop=True, skip_group_check=True)
            nc.tensor.matmul(h0p[64:128, :], selK16[0:K, :], keys16[0:K, HH:H],
                             start=True, stop=True, skip_group_check=True)
            hu_cur = p_hu.tile([128, HH], F16, tag="hu")
            nc.vector.tensor_copy(hu_cur[:, :], h0p[:, :])

            tp0 = ptile([128, 3 * 128], F16)
            emit_transposes(hu_cur, tp0)
            hT_cur = p_hT.tile([128, 3 * 128], F16, tag="hT")
            nc.vector.tensor_copy(hT_cur[:, :], tp0[:, :])

        # PSUM pools for the main loop (opened after the prologue pool frees)
        p_zps = ep(tc.tile_pool(name="zps", bufs=2, space="PSUM"))
        p_bps = ep(tc.tile_pool(name="bps", bufs=2, space="PSUM"))
        p_tps = ep(tc.tile_pool(name="tps", bufs=1, space="PSUM"))
        p_sps = ep(tc.tile_pool(name="sps", bufs=1, space="PSUM"))
        p_eps = ep(tc.tile_pool(name="eps", bufs=1, space="PSUM"))

        # rn_0 = 1, h_mat_0 = hu_0
        rn_cur = p_rn.tile([128, 1], F32, tag="rn")
        nc.vector.memset(rn_cur[:, :], 1.0)
        nc.vector.memset(scores[:, :], 0.0)
        hm_cur = p_hm.tile([128, HH], F16, tag="hm")
        nc.vector.tensor_copy(hm_cur[:, :], hu_cur[:, :])

        # stage steps 0 and 1; bias for step 0
        stage(0, 0)
        if nsteps > 1:
            stage(1, 1)
        bP0 = p_bps.tile([128, HH], F32, tag="b")
        emit_bias(bP0, 0)
        b16_cur = p_b16.tile([128, HH], F16, tag="b16")
        nc.scalar.copy(b16_cur[:, :], bP0[:, :])
        if nsteps > 1:
            bP_next = p_bps.tile([128, HH], F32, tag="b")
            emit_bias(bP_next, 1)
        else:
            bP_next = None

        # ---- main loop ----
        rn_next = hm_next = None
        for t in range(nsteps):
            par = t % 2
            if t > 0:
                rn_cur, hm_cur = rn_next, hm_next

            # fused z/g/q block
            zP = p_zps.tile([128, FW], F32, tag="z")
            for c in range(HC):
                lhs = hts(hT_cur, c)
                base = par * HC * BLK + c * BLK
                nc.tensor.matmul(zP[0:RP, :], lhs,
                                 rhs2[:, base:base + FW],
                                 start=(c == 0), stop=(c == HC - 1),
                                 skip_group_check=True)
                nc.tensor.matmul(zP[64:128, :], lhs,
                                 rhs2[:, base + FW:base + BLK],
                                 start=(c == 0), stop=(c == HC - 1),
                                 skip_group_check=True)

            # off-path work: bias t+2 fills the post-block PE idle window;
            # the PSUM->SBUF copy of bias_{t+1} runs early in the ACT queue
            if t + 2 < nsteps:
                stage(t + 2, t % 2)
                bP_next2 = p_bps.tile([128, HH], F32, tag="b")
                emit_bias(bP_next2, t + 2)
            else:
                bP_next2 = None
            if t + 1 < nsteps:
                b16_new = p_b16.tile([128, HH], F16, tag="b16")
                nc.scalar.copy(b16_new[:, :], bP_next[:, :])
            else:
                b16_new = b16_cur
            bP_next = bP_next2

            # gate: masked reduce + sigmoid(rn*gpre + s.keys)
            gsc = p_sml.tile([128, BL], F16, tag="gsc")
            gpre = p_sml.tile([128, 1], F32, tag="gpre")
            mask_reduce(gsc[:, :], zP[:, HH:HH + 8], maskG[:, :],
                        gpre[:, :])
            gsig = p_sml.tile([128, 1], F32, tag="gsig")
            nc.scalar.activation(gsig[:, :], gpre[:, :], ACTF.Sigmoid,
                                 bias=sKall[:, t:t + 1], scale=rn_cur[:, :])

            # z = rn*zU + bias; cand = g*prelu(z)
            zs16 = p_e16.tile([128, HH], F16, tag="zs16")
            nc.vector.tensor_scalar(zs16[:, :], zP[:, 0:HH], rn_cur[:, :],
                                    None, ALU.mult)
            z16 = p_e16.tile([128, HH], F16, tag="z16")
            nc.vector.tensor_add(z16[:, :], zs16[:, :], b16_cur[:, :])
            cand = p_e16.tile([128, HH], F16, tag="cand")
            hu_new = p_hu.tile([128, HH], F16, tag="hu")
            if use_prelu:
                al = alphav[:, :] if alpha_const is None else float(alpha_const)
                nc.scalar.activation(cand[:, :], z16[:, :], ACTF.Prelu,
                                     scale=gsig[:, :], alpha=al)
                # hu_{t+1} = h_t + cand
                nc.vector.tensor_add(hu_new[:, :], hm_cur[:, :], cand[:, :])
            else:
                # sim fallback: g*prelu(z) = g(1-a)/2*|z| + g(1+a)/2*z
                ca = p_sml.tile([128, 1], F32, tag="ca")
                cb = p_sml.tile([128, 1], F32, tag="cb")
                nc.vector.tensor_mul(ca[:, :], gsig[:, :], av_n[:, :])
                nc.vector.tensor_mul(cb[:, :], gsig[:, :], av_p[:, :])
                nc.scalar.activation(cand[:, :], z16[:, :], ACTF.Abs,
                                     scale=ca[:, :])
                bv = p_e16.tile([128, HH], F16, tag="bv")
                nc.vector.tensor_scalar(bv[:, :], z16[:, :], cb[:, :], None,
                                        ALU.mult)
                nc.vector.tensor_add(hu_new[:, :], hm_cur[:, :], cand[:, :])
                nc.vector.tensor_add(hu_new[:, :], hu_new[:, :], bv[:, :])

            # transposes -> huT_{t+1}
            tP = p_tps.tile([128, 3 * 128], F16, tag="t")
            emit_transposes(hu_new, tP)
            hT_new = p_hT.tile([128, 3 * 128], F16, tag="hT")
            nc.vector.tensor_copy(hT_new[:, :], tP[:, :])

            # norm accumulators for rn_{t+1}
            squ = p_e16.tile([128, HH], F16, tag="squ")
            ss_n = p_sml.tile([128, 1], F32, tag="ss")
            nc.scalar.activation(squ[:, :], hu_new[:, :], ACTF.Square,
                                 accum_out=ss_n[:, :])
            ss16_n = p_sml.tile([128, 1], F16, tag="ss16")
            pool.tensor_copy(ss16_n[:, :], ss_n[:, :])

            # rn_{t+1} = rsqrt(fold(ss)): PE fold, DVE fast-inv-sqrt + 1 NR
            ssp = p_sps.tile([128, 1], F32, tag="ssp")
            nc.tensor.matmul(ssp[:, :], P64h[:, :], ss16_n[:, :],
                             start=True, stop=True)
            sdi = p_sml.tile([128, 1], I32, tag="sdi")
            nc.vector.tensor_scalar(sdi[:, :], ssp.bitcast(I32)[:, :], 1,
                                    None, ALU.logical_shift_right)
            nc.vector.tensor_scalar(sdi[:, :], sdi[:, :], -1, 0x5F3759DF,
                                    ALU.mult, ALU.add)
            rn_w = sdi.bitcast(F32)
            ra = p_sml.tile([128, 1], F32, tag="ra")
            rn_next = p_rn.tile([128, 1], F32, tag="rn")
            for it in range(newton_iters):
                nc.vector.tensor_mul(ra[:, :], rn_w[:, :], ssp[:, :])
                nc.vector.tensor_mul(ra[:, :], ra[:, :], rn_w[:, :])
                nc.vector.tensor_scalar(ra[:, :], ra[:, :], -0.5, 1.5,
                                        ALU.mult, ALU.add)
                dst = rn_next if it == newton_iters - 1 else rn_w
                nc.vector.tensor_mul(dst[:, :], rn_w[:, :], ra[:, :])
            hm_next = p_hm.tile([128, HH], F16, tag="hm")
            nc.vector.tensor_scalar(hm_next[:, :], hu_new[:, :],
                                    rn_next[:, :], None, ALU.mult)

            # score for step t-1: rn_t * masked-reduce(q-gram cols)
            if t > 0:
                qsc = p_sml.tile([128, BL], F16, tag="qsc")
                qtmp = p_sml.tile([128, 1], F32, tag="qtmp")
                mask_reduce(qsc[:, :], zP[:, HH + 8:FW], maskG[:, :],
                            qtmp[:, :])
                pool.tensor_scalar(scores[:, t - 1:t], qtmp[:, :],
                                        rn_cur[:, :], None, ALU.mult)

            if debug:
                dma(bass.AP(d_dhu, t * 128 * HH, [[HH, 128], [1, HH]]),
                    hu_new[:, :])
                dma(bass.AP(d_drn, t * 128, [[1, 128], [1, 1]]), rn_cur[:, :])
                dma(bass.AP(d_dg, t * 128, [[1, 128], [1, 1]]), gsig[:, :])
                dma(bass.AP(d_dz, t * 128 * HH, [[HH, 128], [1, HH]]),
                    z16[:, :])

            hu_cur, hT_cur, b16_cur = hu_new, hT_new, b16_new

        # ---- epilogue ----
        rn_T = rn_next

        # final q-gram: q_{T-1} . hu_T
        qPf = p_eps.tile([128, BL], F32, tag="qf")
        for c in range(HC):
            nc.tensor.matmul(
                qPf[0:RP, 0:BL], hts(hT_cur, c),
                qT[:, c * NR + (nsteps - 1) * 8:c * NR + nsteps * 8],
                start=(c == 0), stop=(c == HC - 1))
        qsc = p_sml.tile([128, BL], F16, tag="qsc")
        qtmp = p_sml.tile([128, 1], F32, tag="qtmp")
        mask_reduce(qsc[0:RP, :], qPf[0:RP, 0:BL], maskG[0:RP, :],
                    qtmp[0:RP, :])
        pool.tensor_scalar(scores[0:RP, nsteps - 1:nsteps],
                                qtmp[0:RP, :], rn_T[0:RP, :], None, ALU.mult)

        # output head
        pP = p_eps.tile([128, nsteps], F32, tag="pp")
        nc.tensor.matmul(pP[0:BL * L, :], Wsel[0:R, 0:BL * L],
                         scores[0:R, 0:nsteps], start=True, stop=True)
        osb = p_prm.tile([128, nsteps], F32, tag="osb")
        nc.vector.tensor_scalar(osb[0:BL * L, :], pP[0:BL * L, :],
                                bvec[0:BL * L, :], None, ALU.add)
        nc.sync.dma_start(bass.AP(d_out, 0, [[1, BL * L], [BL * L, nsteps]]),
                          osb[0:BL * L, :])
        if debug:
            dma(bass.AP(d_dsc, 0, [[nsteps, 128], [1, nsteps]]),
                scores[:, :])

    nc.compile()
    return nc


_CACHE = {}


def _get(nsteps, debug=False, use_prelu=True, **kw):
    key = (nsteps, debug, use_prelu) + tuple(sorted(kw.items()))
    if key not in _CACHE:
        _CACHE[key] = _build(nsteps, debug=debug, use_prelu=use_prelu, **kw)
    return _CACHE[key]


def _in_maps(inputs, nsteps):
    consts = _host_consts()
    fs = np.ascontiguousarray(np.asarray(inputs["features_sentence"], dtype=np.float32))
    fe = np.ascontiguousarray(np.asarray(inputs["features_entity"], dtype=np.float32))
    shared = {k: np.ascontiguousarray(np.asarray(inputs[k], dtype=np.float32))
              for k in ("keys", "U", "V", "W", "alpha", "W_out", "b_out")}
    shared.update(consts)
    in_maps = []
    for c in range(NC):
        m = dict(shared)
        m["features_sentence"] = np.ascontiguousarray(fs[:, c * BL:(c + 1) * BL, :])
        m["features_entity"] = np.ascontiguousarray(fe[:, c * BL:(c + 1) * BL, :])
        in_maps.append(m)
    return in_maps


def kernel(**inputs):
    nsteps = inputs["features_sentence"].shape[0]
    nc = _get(nsteps, alpha_const=float(np.asarray(inputs["alpha"]).ravel()[0]))
    res = run_bass_kernel_spmd(nc, _in_maps(inputs, nsteps),
                               core_ids=list(range(NC)))
    outs = [r["preds"].reshape(nsteps, BL, L) for r in res.results]
    return np.concatenate(outs, axis=1).reshape(nsteps * B, L)


# revision 26
# speedup vs baseline: 1.6665x; 1.0102x over previous
"""EntNetHead Trainium2 kernel (v2: deferred normalization).

Data-parallel over batch B=64 across 8 NeuronCores (8 batch rows per core);
T=256 recurrent steps run on-chip per core.

Key idea vs v1: the per-step L2 normalization is linear, so the state is
kept UNNORMALIZED (hu) together with rn = 1/||hu|| per packed row.  The
matmuls use huT directly; rn folds into (a) the sigmoid's per-partition
scale, (b) one DVE scale of the z PSUM, (c) a lazily-materialized
h = rn*hu used by the state update.  The whole square/fold/rsqrt chain
runs OFF the critical path (ACT/Pool/PE idle slots).

Per-step PE block is 12 matmuls: the g-gram (s_t . hu) and q-gram
(q_{t-1} . hu) are fused into the z matmuls as 16 extra rhs columns,
staged next to the U chunks by the (otherwise idle) DMA engines.  The
gate's content bias s_t.keys is precomputed for all t as a per-partition
column (sKall) and enters through the sigmoid's bias AP. The z bias
keyV + s_t@W accumulates in a separate PSUM bank off the critical path.

Engine split per step:
  PE:   12 fused z/g/q matmuls, 4 bias matmuls (t+1), 3 transposes,
        1 fp16 norm-fold matmul
  DVE:  gate reduce, z-scale, z-add(bias), hu update, hT copy,
        q reduce, rsqrt seed
  ACT:  sigmoid, prelu, square+accum, bias PSUM->SBUF copy
  Pool: Newton rsqrt, h materialize, score scale, ss fp16 cast
  DMA:  per-step staging of sT/qT slices into the fused rhs buffer
"""

import sys

sys.path.insert(0, "/opt/trn_rl_repo")

from contextlib import ExitStack

import numpy as np

import concourse.bacc as bacc
import concourse.bass as bass
import concourse.tile as tile
from concourse import mybir
from concourse.bass_utils import run_bass_kernel_spmd

F32 = mybir.dt.float32
F16 = mybir.dt.float16
I32 = mybir.dt.int32
ALU = mybir.AluOpType
ACTF = mybir.ActivationFunctionType

T, B, H, K, L = 256, 64, 768, 5, 3
NC = 8
BL = B // NC          # 8 batch rows per core
R = K * BL            # 40 (k,b) rows
RP = 64               # padded stationary width
HC = H // 128         # 6 contraction chunks
HH = H // 2           # 384
BLK = 2 * (HH + 16)   # 800: per-chunk fused rhs block, 2 halves of FW
FW = HH + 16          # 400: fused rhs width per half: [U 0:192|sT|qT|U 192:384]
GA = 208              # group-A matmul width (192 z cols + 8 g + 8 q)
GZ = 192              # z cols in group A


def _host_consts():
    selK = np.zeros((K, RP), np.float16)
    for k in range(K):
        selK[k, k * BL:(k + 1) * BL] = 1.0
    selKb = np.zeros((K, 8 * RP), np.float16)
    for b in range(BL):
        for k in range(K):
            selKb[k, b * RP + k * BL + b] = 1.0
    selB = np.zeros((128, 16 * RP), np.float16)
    for m in range(16):
        for b in range(BL):
            for k in range(K):
                selB[m * BL + b, m * RP + k * BL + b] = 1.0
    maskG = np.zeros((128, BL), np.float16)
    for p in range(128):
        maskG[p, p % BL] = 1.0
    I64 = np.zeros((128, RP), np.float16)
    for j in range(RP):
        I64[j, j] = 1.0
        I64[64 + j, j] = 1.0
    I128 = np.eye(128, dtype=np.float32)
    mask24 = np.zeros((R, BL * L), np.float32)
    for k in range(K):
        for b in range(BL):
            mask24[k * BL + b, b * L:(b + 1) * L] = 1.0
    ones1x128 = np.ones((1, 128), np.float32)
    P64h = np.zeros((128, 128), np.float16)
    for i in range(128):
        P64h[i, i % 64] = 1.0
        P64h[i, i % 64 + 64] = 1.0
    selK32 = selK.astype(np.float32)
    return {
        "c_selK": selK, "c_selKb": selKb, "c_selB32": selB, "c_maskG": maskG,
        "c_I64": I64, "c_I128": I128, "c_selK32": selK32,
        "c_mask24": mask24, "c_ones": ones1x128, "c_P64h": P64h,
        "c_I128h": np.eye(128, dtype=np.float16),
    }


def _build(nsteps, debug=False, use_prelu=True,
           stage_dma=True, use_pool=True, use_ttr=False, newton_iters=1,
           alpha_const=None):
    # use_ttr: InstTensorTensorReduce wedges TRN2 here (NRT_EXEC_UNIT_
    # UNRECOVERABLE) despite passing CoreSim -- keep the 2-op fallback.
    nc = bacc.Bacc("TRN2", target_bir_lowering=False, debug=False)
    NR = nsteps * BL      # feature rows per core
    ntt = (NR + 127) // 128

    d_fs = nc.dram_tensor("features_sentence", [nsteps, BL, H], F32, kind="ExternalInput")
    d_fe = nc.dram_tensor("features_entity", [nsteps, BL, H], F32, kind="ExternalInput")
    d_keys = nc.dram_tensor("keys", [K, H], F32, kind="ExternalInput")
    d_U = nc.dram_tensor("U", [H, H], F32, kind="ExternalInput")
    d_V = nc.dram_tensor("V", [H, H], F32, kind="ExternalInput")
    d_W = nc.dram_tensor("W", [H, H], F32, kind="ExternalInput")
    d_alpha = nc.dram_tensor("alpha", [1], F32, kind="ExternalInput")
    d_Wout = nc.dram_tensor("W_out", [K, L], F32, kind="ExternalInput")
    d_bout = nc.dram_tensor("b_out", [L], F32, kind="ExternalInput")
    d_selK = nc.dram_tensor("c_selK", [K, RP], F16, kind="ExternalInput")
    d_selKb = nc.dram_tensor("c_selKb", [K, 8 * RP], F16, kind="ExternalInput")
    d_selB32 = nc.dram_tensor("c_selB32", [128, 16 * RP], F16, kind="ExternalInput")
    d_maskG = nc.dram_tensor("c_maskG", [128, BL], F16, kind="ExternalInput")
    d_I64 = nc.dram_tensor("c_I64", [128, RP], F16, kind="ExternalInput")
    d_I128 = nc.dram_tensor("c_I128", [128, 128], F32, kind="ExternalInput")
    d_selK32 = nc.dram_tensor("c_selK32", [K, RP], F32, kind="ExternalInput")
    d_mask24 = nc.dram_tensor("c_mask24", [R, BL * L], F32, kind="ExternalInput")
    d_ones = nc.dram_tensor("c_ones", [1, 128], F32, kind="ExternalInput")
    d_P64h = nc.dram_tensor("c_P64h", [128, 128], F16, kind="ExternalInput")
    d_I128h = nc.dram_tensor("c_I128h", [128, 128], F16, kind="ExternalInput")
    d_out = nc.dram_tensor("preds", [NR, L], F32, kind="ExternalOutput")
    if debug:
        d_dhu = nc.dram_tensor("dbg_hu", [nsteps + 1, 128, HH], F16, kind="ExternalOutput")
        d_drn = nc.dram_tensor("dbg_rn", [nsteps + 1, 128], F32, kind="ExternalOutput")
        d_dg = nc.dram_tensor("dbg_g", [nsteps, 128], F32, kind="ExternalOutput")
        d_dz = nc.dram_tensor("dbg_z", [nsteps, 128, HH], F16, kind="ExternalOutput")
        d_dsc = nc.dram_tensor("dbg_sc", [128, nsteps], F32, kind="ExternalOutput")

    with tile.TileContext(nc) as tc, ExitStack() as ctx:
        ep = ctx.enter_context
        pool = nc.gpsimd if use_pool else nc.vector

        p_sT = ep(tc.tile_pool(name="sT", bufs=1))
        p_qT = ep(tc.tile_pool(name="qT", bufs=1))
        p_sW = ep(tc.tile_pool(name="sW", bufs=1))
        p_rhs = ep(tc.tile_pool(name="rhs", bufs=1))
        p_prm = ep(tc.tile_pool(name="prm", bufs=1))
        p_hu = ep(tc.tile_pool(name="hu", bufs=2))
        p_hm = ep(tc.tile_pool(name="hm", bufs=2))
        p_hT = ep(tc.tile_pool(name="hT", bufs=2))
        p_rn = ep(tc.tile_pool(name="rn", bufs=2))
        p_b16 = ep(tc.tile_pool(name="b16", bufs=2))
        p_e16 = ep(tc.tile_pool(name="e16", bufs=2))
        p_sml = ep(tc.tile_pool(name="sml", bufs=3))

        sT = p_sT.tile([128, HC * NR], F16)   # [h%128, c*NR + t*8+b]
        qT = p_qT.tile([128, HC * NR], F16)
        sW = p_sW.tile([128, ntt * H], F16)   # [row%128, tile*768+h]
        rhs2 = p_rhs.tile([128, 2 * HC * BLK], F16)  # fused rhs, 2 parities
        keyV = p_prm.tile([128, H], F16, tag="keyV")
        keys16 = p_prm.tile([128, H], F16, tag="keys16")
        keysT = p_prm.tile([128, HC * 8], F16, tag="keysT")
        sK = p_prm.tile([128, NR], F16, tag="sK")
        sKall = p_prm.tile([128, nsteps], F32, tag="sKall")
        selK16 = p_prm.tile([128, RP], F16, tag="selK16")
        selKb16 = p_prm.tile([128, 8 * RP], F16, tag="selKb16")
        selB32 = p_prm.tile([128, 16 * RP], F16, tag="selB32")
        maskG = p_prm.tile([128, BL], F16, tag="maskG")
        I64 = p_prm.tile([128, RP], F16, tag="I64")
        I128 = p_prm.tile([128, 128], F32, tag="I128")
        selK32 = p_prm.tile([128, RP], F32, tag="selK32")
        mask24 = p_prm.tile([128, BL * L], F32, tag="mask24")
        ones_r = p_prm.tile([128, 128], F32, tag="ones_r")
        P64h = p_prm.tile([128, 128], F16, tag="P64h")
        I128h = p_prm.tile([128, 128], F16, tag="I128h")
        alphav = p_prm.tile([128, 1], F32, tag="alphav")
        av_n = p_prm.tile([128, 1], F32, tag="av_n")
        av_p = p_prm.tile([128, 1], F32, tag="av_p")
        Wsel = p_prm.tile([128, BL * L], F32, tag="Wsel")
        bvec = p_prm.tile([128, 1], F32, tag="bvec")
        scores = p_prm.tile([128, nsteps], F32, tag="scores")

        dma = nc.sync.dma_start

        dma(selK16[0:K, :], d_selK.ap())
        dma(selKb16[0:K, :], d_selKb.ap())
        dma(selB32[:, :], d_selB32.ap())
        dma(maskG[:, :], d_maskG.ap())
        dma(I64[:, :], d_I64.ap())
        dma(I128[:, :], d_I128.ap())
        dma(selK32[0:K, :], d_selK32.ap())
        dma(mask24[0:R, :], d_mask24.ap())
        dma(ones_r[0:1, :], d_ones.ap())
        dma(P64h[:, :], d_P64h.ap())
        dma(I128h[:, :], d_I128h.ap())
        for b in range(BL):
            dma(bvec[b * L:(b + 1) * L, 0:1], bass.AP(d_bout, 0, [[1, L], [1, 1]]))

        def hts(hT_tile, c):
            if c < 3:
                return hT_tile[:, c * 128:c * 128 + RP]
            return hT_tile[:, (c - 3) * 128 + RP:(c - 2) * 128]

        def emit_transposes(h_tile, out_psum):
            for cc in range(3):
                nc.tensor.transpose(out_psum[:, cc * 128:(cc + 1) * 128],
                                    h_tile[:, cc * 128:(cc + 1) * 128],
                                    I128h[:, :])

        def stage(t, par):
            """Copy sT_t and qT_{max(t-1,0)} slices into rhs2 parity par."""
            tq = max(t - 1, 0)
            mv = dma if stage_dma else pool.tensor_copy
            for i in range(2):
                mv(bass.AP(rhs2.tensor,
                           rhs2.offset + par * HC * BLK + i * FW + GZ,
                           [[2 * HC * BLK, 128], [BLK, HC], [1, 8]]),
                   bass.AP(sT.tensor, sT.offset + t * 8,
                           [[HC * NR, 128], [NR, HC], [1, 8]]))
                mv(bass.AP(rhs2.tensor,
                           rhs2.offset + par * HC * BLK + i * FW + GZ + 8,
                           [[2 * HC * BLK, 128], [BLK, HC], [1, 8]]),
                   bass.AP(qT.tensor, qT.offset + tq * 8,
                           [[HC * NR, 128], [NR, HC], [1, 8]]))

        def mask_reduce(out_sc, in_ap, msk, accum, eng=None):
            eng = eng or nc.vector
            if use_ttr:
                nc.vector.tensor_tensor_reduce(
                    out_sc, in_ap, msk, 1.0, 0.0, ALU.mult, ALU.add, accum)
            else:
                eng.tensor_mul(out_sc, in_ap, msk)
                eng.tensor_reduce(accum, out_sc,
                                  mybir.AxisListType.X, ALU.add)

        def emit_bias(bP, tn):
            """biasP for step tn: keyV broadcast + s_tn @ W broadcast."""
            m, tb = tn % 16, (tn * BL) // 128
            nc.tensor.matmul(bP[0:RP, :], selK16[0:K, :], keyV[0:K, 0:HH],
                             start=True, stop=False, skip_group_check=True)
            nc.tensor.matmul(bP[64:128, :], selK16[0:K, :], keyV[0:K, HH:H],
                             start=True, stop=False, skip_group_check=True)
            lsel = selB32[:, m * RP:(m + 1) * RP]
            nc.tensor.matmul(bP[0:RP, :], lsel, sW[:, tb * H:tb * H + HH],
                             start=False, stop=True, skip_group_check=True)
            nc.tensor.matmul(bP[64:128, :], lsel, sW[:, tb * H + HH:(tb + 1) * H],
                             start=False, stop=True, skip_group_check=True)

        with tc.tile_pool(name="pr32", bufs=2) as p32, \
             tc.tile_pool(name="prps", bufs=4, space="PSUM") as pps:

            def ptile(shape, dt):
                return pps.tile(shape, dt, tag="ps", name="ps")

            # keys
            kn = p32.tile([128, H], F32, tag="kn")
            dma(kn[0:K, :], d_keys.ap())
            nc.vector.tensor_copy(keys16[0:K, :], kn[0:K, :])
            # U (chunked fp32 load -> cast-copied into both parities of rhs2)
            for c in range(HC):
                u1 = p32.tile([128, H], F32, tag="u1")
                dma(u1[:, :], d_U.ap()[c * 128:(c + 1) * 128, :])
                for par in range(2):
                    for i in range(2):
                        base = par * HC * BLK + c * BLK + i * FW
                        dst = bass.AP(rhs2.tensor, rhs2.offset + base,
                                      [[2 * HC * BLK, 128], [1, GZ]])
                        nc.vector.tensor_copy(
                            dst, u1[:, i * HH:i * HH + GZ])
                        dst = bass.AP(rhs2.tensor, rhs2.offset + base + GA,
                                      [[2 * HC * BLK, 128], [1, HH - GZ]])
                        nc.vector.tensor_copy(
                            dst, u1[:, i * HH + GZ:(i + 1) * HH])
            # keysT via PE transpose
            tps_ = ptile([128, HC * 8], F16)
            for c in range(HC):
                nc.tensor.transpose(tps_[:, c * 8:c * 8 + K],
                                    keys16[0:K, c * 128:(c + 1) * 128],
                                    I64[0:K, 0:K])
            for c in range(HC):
                nc.vector.tensor_copy(keysT[:, c * 8:c * 8 + K],
                                      tps_[:, c * 8:c * 8 + K])
            # V -> keyV = keys @ V  (chunked)
            kvp0 = ptile([128, HH], F32)
            kvp1 = ptile([128, HH], F32)
            for c in range(HC):
                v1 = p32.tile([128, H], F32, tag="u1")
                dma(v1[:, :], d_V.ap()[c * 128:(c + 1) * 128, :])
                v16c = p32.tile([128, H], F16, tag="v16c")
                nc.vector.tensor_copy(v16c[:, :], v1[:, :])
                nc.tensor.matmul(kvp0[0:K, :], keysT[:, c * 8:c * 8 + K],
                                 v16c[:, 0:HH],
                                 start=(c == 0), stop=(c == HC - 1))
                nc.tensor.matmul(kvp1[0:K, :], keysT[:, c * 8:c * 8 + K],
                                 v16c[:, HH:H],
                                 start=(c == 0), stop=(c == HC - 1))
            nc.vector.tensor_copy(keyV[0:K, 0:HH], kvp0[0:K, :])
            nc.vector.tensor_copy(keyV[0:K, HH:H], kvp1[0:K, :])
            # W (fp16) for sW matmuls (chunked load)
            w16 = p32.tile([128, HC * H], F16, tag="big16w")
            for c in range(HC):
                w1 = p32.tile([128, H], F32, tag="u1")
                dma(w1[:, :], d_W.ap()[c * 128:(c + 1) * 128, :])
                nc.vector.tensor_copy(w16[:, c * H:(c + 1) * H], w1[:, :])

            # alpha -> all partitions
            asb = p32.tile([128, 1], F32, tag="asb")
            dma(asb[0:1, 0:1], bass.AP(d_alpha, 0, [[1, 1], [1, 1]]))
            alp = ptile([128, 1], F32)
            nc.tensor.matmul(alp[:, :], ones_r[0:1, :], asb[0:1, 0:1],
                             start=True, stop=True)
            nc.vector.tensor_copy(alphav[:, :], alp[:, :])
            nc.vector.tensor_scalar(av_n[:, :], alphav[:, :], -0.5, 0.5,
                                    ALU.mult, ALU.add)
            nc.vector.tensor_scalar(av_p[:, :], alphav[:, :], 0.5, 0.5,
                                    ALU.mult, ALU.add)

            # W_out -> Wsel
            wo = p32.tile([128, L], F32, tag="wo")
            dma(wo[0:K, :], d_Wout.ap())
            wrp = ptile([128, L], F32)
            nc.tensor.matmul(wrp[0:R, :], selK32[0:K, 0:R], wo[0:K, :],
                             start=True, stop=True)
            wrs = p32.tile([128, L], F32, tag="wrs")
            nc.vector.tensor_copy(wrs[0:R, :], wrp[0:R, :])
            for b in range(BL):
                nc.vector.tensor_mul(Wsel[0:R, b * L:(b + 1) * L],
                                     wrs[0:R, :], mask24[0:R, b * L:(b + 1) * L])

            # features: DMA, transpose (and sW for the sentence stream)
            def do_feat(dram, dstT, with_sw):
                for tb in range(ntt):
                    fn = p32.tile([128, H], F32, tag="fnat")
                    nrow = min(128, NR - tb * 128)
                    src = bass.AP(dram, tb * 128 * H, [[H, nrow], [1, H]])
                    dma(fn[0:nrow, :], src)
                    for grp in range(2):
                        tp = ptile([128, 3 * 128], F32)
                        for j in range(3):
                            c = grp * 3 + j
                            nc.tensor.transpose(
                                tp[:, j * nrow:(j + 1) * nrow],
                                fn[0:nrow, c * 128:(c + 1) * 128],
                                I128[0:nrow, 0:nrow])
                        dst = bass.AP(
                            dstT.tensor,
                            dstT.offset + (grp * 3) * NR + tb * 128,
                            [[HC * NR, 128], [NR, 3], [1, nrow]])
                        nc.vector.tensor_copy(
                            dst,
                            tp[:, 0:3 * nrow].rearrange("p (a b) -> p a b", a=3))
                    if with_sw:
                        for i in range(2):
                            swp = ptile([128, HH], F32)
                            for c in range(HC):
                                lhs = dstT[:, c * NR + tb * 128:
                                           c * NR + tb * 128 + nrow]
                                nc.tensor.matmul(
                                    swp[0:nrow, :], lhs,
                                    w16[:, c * H + i * HH:c * H + (i + 1) * HH],
                                    start=(c == 0), stop=(c == HC - 1))
                            nc.vector.tensor_copy(
                                sW[0:nrow, tb * H + i * HH:tb * H + (i + 1) * HH],
                                swp[0:nrow, :])

            if NR < 128:
                nc.vector.memset(sW[:, :], 0.0)
            do_feat(d_fs, sT, True)
            do_feat(d_fe, qT, False)

            # sK = keys @ s^T  [5, NR]
            for q in range((NR + 511) // 512):
                ncol = min(512, NR - q * 512)
                skp = ptile([128, 512], F32)
                for c in range(HC):
                    nc.tensor.matmul(
                        skp[0:K, 0:ncol], keysT[:, c * 8:c * 8 + K],
                        sT[:, c * NR + q * 512:c * NR + q * 512 + ncol],
                        start=(c == 0), stop=(c == HC - 1))
                nc.vector.tensor_copy(sK[0:K, q * 512:q * 512 + ncol],
                                      skp[0:K, 0:ncol])

            # sKall[p, t] = sK[k(p), t*8 + b(p)]  (gate content bias per row)
            skap = ptile([128, nsteps], F32)
            sKr = sK[:, :].rearrange("p (t b) -> p b t", b=BL)
            for b in range(BL):
                sel = selKb16[0:K, b * RP:(b + 1) * RP]
                src = sKr[0:K, b:b + 1, 0:nsteps]
                nc.tensor.matmul(skap[0:RP, :], sel, src,
                                 start=(b == 0), stop=(b == BL - 1),
                                 skip_group_check=True)
                nc.tensor.matmul(skap[64:128, :], sel, src,
                                 start=(b == 0), stop=(b == BL - 1),
                                 skip_group_check=True)
            nc.vector.tensor_copy(sKall[:, :], skap[:, :])

            # hu_0 = h0 = keys broadcast to (k,b) rows (zero into pad rows)
            h0p = ptile([128, HH], F32)
            nc.tensor.matmul(h0p[0:RP, :], selK16[0:K, :], keys16[0:K, 0:HH],
                             start=True, stop=True, skip_group_check=True)
            nc.tensor.matmul(h0p[64:128, :], selK16[0:K, :], keys16[0:K, HH:H],
                             start=True, stop=True, skip_group_check=True)
            hu_cur = p_hu.tile([128, HH], F16, tag="hu")
            nc.vector.tensor_copy(hu_cur[:, :], h0p[:, :])

            tp0 = ptile([128, 3 * 128], F16)
            emit_transposes(hu_cur, tp0)
            hT_cur = p_hT.tile([128, 3 * 128], F16, tag="hT")
            nc.vector.tensor_copy(hT_cur[:, :], tp0[:, :])

        # PSUM pools for the main loop (opened after the prologue pool frees)
        p_zps = ep(tc.tile_pool(name="zps", bufs=2, space="PSUM"))
        p_bps = ep(tc.tile_pool(name="bps", bufs=2, space="PSUM"))
        p_tps = ep(tc.tile_pool(name="tps", bufs=1, space="PSUM"))
        p_sps = ep(tc.tile_pool(name="sps", bufs=1, space="PSUM"))
        p_eps = ep(tc.tile_pool(name="eps", bufs=1, space="PSUM"))

        # rn_0 = 1, h_mat_0 = hu_0
        rn_cur = p_rn.tile([128, 1], F32, tag="rn")
        nc.vector.memset(rn_cur[:, :], 1.0)
        nc.vector.memset(scores[:, :], 0.0)
        hm_cur = p_hm.tile([128, HH], F16, tag="hm")
        nc.vector.tensor_copy(hm_cur[:, :], hu_cur[:, :])

        # stage steps 0 and 1; bias for step 0
        stage(0, 0)
        if nsteps > 1:
            stage(1, 1)
        bP0 = p_bps.tile([128, HH], F32, tag="b")
        emit_bias(bP0, 0)
        b16_cur = p_b16.tile([128, HH], F16, tag="b16")
        nc.scalar.copy(b16_cur[:, :], bP0[:, :])
        if nsteps > 1:
            bP_next = p_bps.tile([128, HH], F32, tag="b")
            emit_bias(bP_next, 1)
        else:
            bP_next = None

        # ---- main loop ----
        rn_next = hm_next = None
        for t in range(nsteps):
            par = t % 2
            if t > 0:
                rn_cur, hm_cur = rn_next, hm_next

            # fused z/g/q block: group A (z 0:192 + g + q) first so its
            # combine/prelu overlaps group B's matmuls
            zP = p_zps.tile([128, FW], F32, tag="z")
            for c in range(HC):
                lhs = hts(hT_cur, c)
                base = par * HC * BLK + c * BLK
                nc.tensor.matmul(zP[0:RP, 0:GA], lhs,
                                 rhs2[:, base:base + GA],
                                 start=(c == 0), stop=(c == HC - 1),
                                 skip_group_check=True)
                nc.tensor.matmul(zP[64:128, 0:GA], lhs,
                                 rhs2[:, base + FW:base + FW + GA],
                                 start=(c == 0), stop=(c == HC - 1),
                                 skip_group_check=True)
            for c in range(HC):
                lhs = hts(hT_cur, c)
                base = par * HC * BLK + c * BLK
                nc.tensor.matmul(zP[0:RP, GA:FW], lhs,
                                 rhs2[:, base + GA:base + FW],
                                 start=(c == 0), stop=(c == HC - 1),
                                 skip_group_check=True)
                nc.tensor.matmul(zP[64:128, GA:FW], lhs,
                                 rhs2[:, base + FW + GA:base + BLK],
                                 start=(c == 0), stop=(c == HC - 1),
                                 skip_group_check=True)

            # off-path work: bias t+2 fills the post-block PE idle window;
            # the PSUM->SBUF copy of bias_{t+1} runs early in the ACT queue
            if t + 2 < nsteps:
                stage(t + 2, t % 2)
                bP_next2 = p_bps.tile([128, HH], F32, tag="b")
                emit_bias(bP_next2, t + 2)
            else:
                bP_next2 = None
            if t + 1 < nsteps:
                b16_new = p_b16.tile([128, HH], F16, tag="b16")
                nc.scalar.copy(b16_new[:, :], bP_next[:, :])
            else:
                b16_new = b16_cur
            bP_next = bP_next2

            # gate: masked reduce + sigmoid(rn*gpre + s.keys)
            gsc = p_sml.tile([128, BL], F16, tag="gsc")
            gpre = p_sml.tile([128, 1], F32, tag="gpre")
            mask_reduce(gsc[:, :], zP[:, GZ:GZ + 8], maskG[:, :],
                        gpre[:, :])
            gsig = p_sml.tile([128, 1], F32, tag="gsig")
            nc.scalar.activation(gsig[:, :], gpre[:, :], ACTF.Sigmoid,
                                 bias=sKall[:, t:t + 1], scale=rn_cur[:, :])

            # z = rn*zU + bias; cand = g*prelu(z)
            # zP col j<GZ -> z col j; col GA+j -> z col GZ+j
            zs16 = p_e16.tile([128, HH], F16, tag="zs16")
            z16 = p_e16.tile([128, HH], F16, tag="z16")
            nc.vector.tensor_scalar(zs16[:, 0:GZ], zP[:, 0:GZ], rn_cur[:, :],
                                    None, ALU.mult)
            nc.vector.tensor_add(z16[:, 0:GZ], zs16[:, 0:GZ],
                                 b16_cur[:, 0:GZ])
            nc.vector.tensor_scalar(zs16[:, GZ:HH], zP[:, GA:FW],
                                    rn_cur[:, :], None, ALU.mult)
            nc.vector.tensor_add(z16[:, GZ:HH], zs16[:, GZ:HH],
                                 b16_cur[:, GZ:HH])
            cand = p_e16.tile([128, HH], F16, tag="cand")
            hu_new = p_hu.tile([128, HH], F16, tag="hu")
            if use_prelu:
                al = alphav[:, :] if alpha_const is None else float(alpha_const)
                nc.scalar.activation(cand[:, 0:GZ], z16[:, 0:GZ], ACTF.Prelu,
                                     scale=gsig[:, :], alpha=al)
                nc.scalar.activation(cand[:, GZ:HH], z16[:, GZ:HH],
                                     ACTF.Prelu, scale=gsig[:, :], alpha=al)
                # hu_{t+1} = h_t + cand
                nc.vector.tensor_add(hu_new[:, :], hm_cur[:, :], cand[:, :])
            else:
                # sim fallback: g*prelu(z) = g(1-a)/2*|z| + g(1+a)/2*z
                ca = p_sml.tile([128, 1], F32, tag="ca")
                cb = p_sml.tile([128, 1], F32, tag="cb")
                nc.vector.tensor_mul(ca[:, :], gsig[:, :], av_n[:, :])
                nc.vector.tensor_mul(cb[:, :], gsig[:, :], av_p[:, :])
                nc.scalar.activation(cand[:, :], z16[:, :], ACTF.Abs,
                                     scale=ca[:, :])
                bv = p_e16.tile([128, HH], F16, tag="bv")
                nc.vector.tensor_scalar(bv[:, :], z16[:, :], cb[:, :], None,
                                        ALU.mult)
                nc.vector.tensor_add(hu_new[:, :], hm_cur[:, :], cand[:, :])
                nc.vector.tensor_add(hu_new[:, :], hu_new[:, :], bv[:, :])

            # transposes -> huT_{t+1}
            tP = p_tps.tile([128, 3 * 128], F16, tag="t")
            emit_transposes(hu_new, tP)
            hT_new = p_hT.tile([128, 3 * 128], F16, tag="hT")
            nc.vector.tensor_copy(hT_new[:, :], tP[:, :])

            # norm accumulators for rn_{t+1}
            squ = p_e16.tile([128, HH], F16, tag="squ")
            ss_n = p_sml.tile([128, 1], F32, tag="ss")
            nc.scalar.activation(squ[:, :], hu_new[:, :], ACTF.Square,
                                 accum_out=ss_n[:, :])
            ss16_n = p_sml.tile([128, 1], F16, tag="ss16")
            pool.tensor_copy(ss16_n[:, :], ss_n[:, :])

            # rn_{t+1} = rsqrt(fold(ss)): PE fold, DVE fast-inv-sqrt + 1 NR
            ssp = p_sps.tile([128, 1], F32, tag="ssp")
            nc.tensor.matmul(ssp[:, :], P64h[:, :], ss16_n[:, :],
                             start=True, stop=True)
            sdi = p_sml.tile([128, 1], I32, tag="sdi")
            nc.vector.tensor_scalar(sdi[:, :], ssp.bitcast(I32)[:, :], 1,
                                    None, ALU.logical_shift_right)
            nc.vector.tensor_scalar(sdi[:, :], sdi[:, :], -1, 0x5F3759DF,
                                    ALU.mult, ALU.add)
            rn_w = sdi.bitcast(F32)
            ra = p_sml.tile([128, 1], F32, tag="ra")
            rn_next = p_rn.tile([128, 1], F32, tag="rn")
            for it in range(newton_iters):
                nc.vector.tensor_mul(ra[:, :], rn_w[:, :], ssp[:, :])
                nc.vector.tensor_mul(ra[:, :], ra[:, :], rn_w[:, :])
                nc.vector.tensor_scalar(ra[:, :], ra[:, :], -0.5, 1.5,
                                        ALU.mult, ALU.add)
                dst = rn_next if it == newton_iters - 1 else rn_w
                nc.vector.tensor_mul(dst[:, :], rn_w[:, :], ra[:, :])
            hm_next = p_hm.tile([128, HH], F16, tag="hm")
            nc.vector.tensor_scalar(hm_next[:, :], hu_new[:, :],
                                    rn_next[:, :], None, ALU.mult)

            # score for step t-1: rn_t * masked-reduce(q-gram cols)
            if t > 0:
                qsc = p_sml.tile([128, BL], F16, tag="qsc")
                qtmp = p_sml.tile([128, 1], F32, tag="qtmp")
                mask_reduce(qsc[:, :], zP[:, GZ + 8:GA], maskG[:, :],
                            qtmp[:, :])
                pool.tensor_scalar(scores[:, t - 1:t], qtmp[:, :],
                                        rn_cur[:, :], None, ALU.mult)

            if debug:
                dma(bass.AP(d_dhu, t * 128 * HH, [[HH, 128], [1, HH]]),
                    hu_new[:, :])
                dma(bass.AP(d_drn, t * 128, [[1, 128], [1, 1]]), rn_cur[:, :])
                dma(bass.AP(d_dg, t * 128, [[1, 128], [1, 1]]), gsig[:, :])
                dma(bass.AP(d_dz, t * 128 * HH, [[HH, 128], [1, HH]]),
                    z16[:, :])

            hu_cur, hT_cur, b16_cur = hu_new, hT_new, b16_new

        # ---- epilogue ----
        rn_T = rn_next

        # final q-gram: q_{T-1} . hu_T
        qPf = p_eps.tile([128, BL], F32, tag="qf")
        for c in range(HC):
            nc.tensor.matmul(
                qPf[0:RP, 0:BL], hts(hT_cur, c),
                qT[:, c * NR + (nsteps - 1) * 8:c * NR + nsteps * 8],
                start=(c == 0), stop=(c == HC - 1))
        qsc = p_sml.tile([128, BL], F16, tag="qsc")
        qtmp = p_sml.tile([128, 1], F32, tag="qtmp")
        mask_reduce(qsc[0:RP, :], qPf[0:RP, 0:BL], maskG[0:RP, :],
                    qtmp[0:RP, :])
        pool.tensor_scalar(scores[0:RP, nsteps - 1:nsteps],
                                qtmp[0:RP, :], rn_T[0:RP, :], None, ALU.mult)

        # output head
        pP = p_eps.tile([128, nsteps], F32, tag="pp")
        nc.tensor.matmul(pP[0:BL * L, :], Wsel[0:R, 0:BL * L],
                         scores[0:R, 0:nsteps], start=True, stop=True)
        osb = p_prm.tile([128, nsteps], F32, tag="osb")
        nc.vector.tensor_scalar(osb[0:BL * L, :], pP[0:BL * L, :],
                                bvec[0:BL * L, :], None, ALU.add)
        nc.sync.dma_start(bass.AP(d_out, 0, [[1, BL * L], [BL * L, nsteps]]),
                          osb[0:BL * L, :])
        if debug:
            dma(bass.AP(d_dsc, 0, [[nsteps, 128], [1, nsteps]]),
                scores[:, :])

    nc.compile()
    return nc


_CACHE = {}


def _get(nsteps, debug=False, use_prelu=True, **kw):
    key = (nsteps, debug, use_prelu) + tuple(sorted(kw.items()))
    if key not in _CACHE:
        _CACHE[key] = _build(nsteps, debug=debug, use_prelu=use_prelu, **kw)
    return _CACHE[key]


def _in_maps(inputs, nsteps):
    consts = _host_consts()
    fs = np.ascontiguousarray(np.asarray(inputs["features_sentence"], dtype=np.float32))
    fe = np.ascontiguousarray(np.asarray(inputs["features_entity"], dtype=np.float32))
    shared = {k: np.ascontiguousarray(np.asarray(inputs[k], dtype=np.float32))
              for k in ("keys", "U", "V", "W", "alpha", "W_out", "b_out")}
    shared.update(consts)
    in_maps = []
    for c in range(NC):
        m = dict(shared)
        m["features_sentence"] = np.ascontiguousarray(fs[:, c * BL:(c + 1) * BL, :])
        m["features_entity"] = np.ascontiguousarray(fe[:, c * BL:(c + 1) * BL, :])
        in_maps.append(m)
    return in_maps


def kernel(**inputs):
    nsteps = inputs["features_sentence"].shape[0]
    nc = _get(nsteps, alpha_const=float(np.asarray(inputs["alpha"]).ravel()[0]))
    res = run_bass_kernel_spmd(nc, _in_maps(inputs, nsteps),
                               core_ids=list(range(NC)))
    outs = [r["preds"].reshape(nsteps, BL, L) for r in res.results]
    return np.concatenate(outs, axis=1).reshape(nsteps * B, L)


# revision 27
# speedup vs baseline: 1.7771x; 1.0664x over previous
"""EntNetHead Trainium2 kernel (v2: deferred normalization).

Data-parallel over batch B=64 across 8 NeuronCores (8 batch rows per core);
T=256 recurrent steps run on-chip per core.

Key idea vs v1: the per-step L2 normalization is linear, so the state is
kept UNNORMALIZED (hu) together with rn = 1/||hu|| per packed row.  The
matmuls use huT directly; rn folds into (a) the sigmoid's per-partition
scale, (b) one DVE scale of the z PSUM, (c) a lazily-materialized
h = rn*hu used by the state update.  The whole square/fold/rsqrt chain
runs OFF the critical path (ACT/Pool/PE idle slots).

Per-step PE block is 12 matmuls: the g-gram (s_t . hu) and q-gram
(q_{t-1} . hu) are fused into the z matmuls as 16 extra rhs columns,
staged next to the U chunks by the (otherwise idle) DMA engines.  The
gate's content bias s_t.keys is precomputed for all t as a per-partition
column (sKall) and enters through the sigmoid's bias AP. The z bias
keyV + s_t@W accumulates in a separate PSUM bank off the critical path.

Engine split per step:
  PE:   12 fused z/g/q matmuls, 4 bias matmuls (t+1), 3 transposes,
        1 fp16 norm-fold matmul
  DVE:  gate reduce, z-scale, z-add(bias), hu update, hT copy,
        q reduce, rsqrt seed
  ACT:  sigmoid, prelu, square+accum, bias PSUM->SBUF copy
  Pool: Newton rsqrt, h materialize, score scale, ss fp16 cast
  DMA:  per-step staging of sT/qT slices into the fused rhs buffer
"""

import sys

sys.path.insert(0, "/opt/trn_rl_repo")

from contextlib import ExitStack

import numpy as np

import concourse.bacc as bacc
import concourse.bass as bass
import concourse.tile as tile
from concourse import mybir
from concourse.bass_utils import run_bass_kernel_spmd

F32 = mybir.dt.float32
F16 = mybir.dt.float16
I32 = mybir.dt.int32
ALU = mybir.AluOpType
ACTF = mybir.ActivationFunctionType

T, B, H, K, L = 256, 64, 768, 5, 3
NC = 8
BL = B // NC          # 8 batch rows per core
R = K * BL            # 40 (k,b) rows
RP = 64               # padded stationary width
HC = H // 128         # 6 contraction chunks
HH = H // 2           # 384
BLK = 2 * (HH + 16)   # 800: per-chunk fused rhs block, 2 halves of FW
FW = HH + 16          # 400: fused rhs width per half: [U 0:192|sT|qT|U 192:384]
GA = 208              # group-A matmul width (192 z cols + 8 g + 8 q)
GZ = 192              # z cols in group A


def _host_consts():
    selK = np.zeros((K, RP), np.float16)
    for k in range(K):
        selK[k, k * BL:(k + 1) * BL] = 1.0
    selKb = np.zeros((K, 8 * RP), np.float16)
    for b in range(BL):
        for k in range(K):
            selKb[k, b * RP + k * BL + b] = 1.0
    selB = np.zeros((128, 16 * RP), np.float16)
    for m in range(16):
        for b in range(BL):
            for k in range(K):
                selB[m * BL + b, m * RP + k * BL + b] = 1.0
    maskG = np.zeros((128, BL), np.float16)
    for p in range(128):
        maskG[p, p % BL] = 1.0
    I64 = np.zeros((128, RP), np.float16)
    for j in range(RP):
        I64[j, j] = 1.0
        I64[64 + j, j] = 1.0
    I128 = np.eye(128, dtype=np.float32)
    mask24 = np.zeros((R, BL * L), np.float32)
    for k in range(K):
        for b in range(BL):
            mask24[k * BL + b, b * L:(b + 1) * L] = 1.0
    ones1x128 = np.ones((1, 128), np.float32)
    P64h = np.zeros((128, 128), np.float16)
    for i in range(128):
        P64h[i, i % 64] = 1.0
        P64h[i, i % 64 + 64] = 1.0
    selK32 = selK.astype(np.float32)
    return {
        "c_selK": selK, "c_selKb": selKb, "c_selB32": selB, "c_maskG": maskG,
        "c_I64": I64, "c_I128": I128, "c_selK32": selK32,
        "c_mask24": mask24, "c_ones": ones1x128, "c_P64h": P64h,
        "c_I128h": np.eye(128, dtype=np.float16),
    }


def _build(nsteps, debug=False, use_prelu=True,
           stage_dma=True, use_pool=True, use_ttr=False, newton_iters=1,
           alpha_const=None):
    # use_ttr: InstTensorTensorReduce wedges TRN2 here (NRT_EXEC_UNIT_
    # UNRECOVERABLE) despite passing CoreSim -- keep the 2-op fallback.
    nc = bacc.Bacc("TRN2", target_bir_lowering=False, debug=False)
    NR = nsteps * BL      # feature rows per core
    ntt = (NR + 127) // 128

    d_fs = nc.dram_tensor("features_sentence", [nsteps, BL, H], F32, kind="ExternalInput")
    d_fe = nc.dram_tensor("features_entity", [nsteps, BL, H], F32, kind="ExternalInput")
    d_keys = nc.dram_tensor("keys", [K, H], F32, kind="ExternalInput")
    d_U = nc.dram_tensor("U", [H, H], F32, kind="ExternalInput")
    d_V = nc.dram_tensor("V", [H, H], F32, kind="ExternalInput")
    d_W = nc.dram_tensor("W", [H, H], F32, kind="ExternalInput")
    d_alpha = nc.dram_tensor("alpha", [1], F32, kind="ExternalInput")
    d_Wout = nc.dram_tensor("W_out", [K, L], F32, kind="ExternalInput")
    d_bout = nc.dram_tensor("b_out", [L], F32, kind="ExternalInput")
    d_selK = nc.dram_tensor("c_selK", [K, RP], F16, kind="ExternalInput")
    d_selKb = nc.dram_tensor("c_selKb", [K, 8 * RP], F16, kind="ExternalInput")
    d_selB32 = nc.dram_tensor("c_selB32", [128, 16 * RP], F16, kind="ExternalInput")
    d_maskG = nc.dram_tensor("c_maskG", [128, BL], F16, kind="ExternalInput")
    d_I64 = nc.dram_tensor("c_I64", [128, RP], F16, kind="ExternalInput")
    d_I128 = nc.dram_tensor("c_I128", [128, 128], F32, kind="ExternalInput")
    d_selK32 = nc.dram_tensor("c_selK32", [K, RP], F32, kind="ExternalInput")
    d_mask24 = nc.dram_tensor("c_mask24", [R, BL * L], F32, kind="ExternalInput")
    d_ones = nc.dram_tensor("c_ones", [1, 128], F32, kind="ExternalInput")
    d_P64h = nc.dram_tensor("c_P64h", [128, 128], F16, kind="ExternalInput")
    d_I128h = nc.dram_tensor("c_I128h", [128, 128], F16, kind="ExternalInput")
    d_out = nc.dram_tensor("preds", [NR, L], F32, kind="ExternalOutput")
    if debug:
        d_dhu = nc.dram_tensor("dbg_hu", [nsteps + 1, 128, HH], F16, kind="ExternalOutput")
        d_drn = nc.dram_tensor("dbg_rn", [nsteps + 1, 128], F32, kind="ExternalOutput")
        d_dg = nc.dram_tensor("dbg_g", [nsteps, 128], F32, kind="ExternalOutput")
        d_dz = nc.dram_tensor("dbg_z", [nsteps, 128, HH], F16, kind="ExternalOutput")
        d_dsc = nc.dram_tensor("dbg_sc", [128, nsteps], F32, kind="ExternalOutput")

    with tile.TileContext(nc) as tc, ExitStack() as ctx:
        ep = ctx.enter_context
        pool = nc.gpsimd if use_pool else nc.vector

        p_sT = ep(tc.tile_pool(name="sT", bufs=1))
        p_qT = ep(tc.tile_pool(name="qT", bufs=1))
        p_sW = ep(tc.tile_pool(name="sW", bufs=1))
        p_rhs = ep(tc.tile_pool(name="rhs", bufs=1))
        p_prm = ep(tc.tile_pool(name="prm", bufs=1))
        p_hu = ep(tc.tile_pool(name="hu", bufs=2))
        p_hm = ep(tc.tile_pool(name="hm", bufs=2))
        p_hT = ep(tc.tile_pool(name="hT", bufs=2))
        p_rn = ep(tc.tile_pool(name="rn", bufs=2))
        p_b16 = ep(tc.tile_pool(name="b16", bufs=2))
        p_e16 = ep(tc.tile_pool(name="e16", bufs=2))
        p_sml = ep(tc.tile_pool(name="sml", bufs=3))

        sT = p_sT.tile([128, HC * NR], F16)   # [h%128, c*NR + t*8+b]
        qT = p_qT.tile([128, HC * NR], F16)
        sW = p_sW.tile([128, ntt * H], F16)   # [row%128, tile*768+h]
        rhs2 = p_rhs.tile([128, 2 * HC * BLK], F16)  # fused rhs, 2 parities
        keyV = p_prm.tile([128, H], F16, tag="keyV")
        keys16 = p_prm.tile([128, H], F16, tag="keys16")
        keysT = p_prm.tile([128, HC * 8], F16, tag="keysT")
        sK = p_prm.tile([128, NR], F16, tag="sK")
        sKall = p_prm.tile([128, nsteps], F32, tag="sKall")
        selK16 = p_prm.tile([128, RP], F16, tag="selK16")
        selKb16 = p_prm.tile([128, 8 * RP], F16, tag="selKb16")
        selB32 = p_prm.tile([128, 16 * RP], F16, tag="selB32")
        maskG = p_prm.tile([128, BL], F16, tag="maskG")
        I64 = p_prm.tile([128, RP], F16, tag="I64")
        I128 = p_prm.tile([128, 128], F32, tag="I128")
        selK32 = p_prm.tile([128, RP], F32, tag="selK32")
        mask24 = p_prm.tile([128, BL * L], F32, tag="mask24")
        ones_r = p_prm.tile([128, 128], F32, tag="ones_r")
        P64h = p_prm.tile([128, 128], F16, tag="P64h")
        I128h = p_prm.tile([128, 128], F16, tag="I128h")
        alphav = p_prm.tile([128, 1], F32, tag="alphav")
        av_n = p_prm.tile([128, 1], F32, tag="av_n")
        av_p = p_prm.tile([128, 1], F32, tag="av_p")
        Wsel = p_prm.tile([128, BL * L], F32, tag="Wsel")
        bvec = p_prm.tile([128, 1], F32, tag="bvec")
        scores = p_prm.tile([128, nsteps], F32, tag="scores")

        dma = nc.sync.dma_start

        dma(selK16[0:K, :], d_selK.ap())
        dma(selKb16[0:K, :], d_selKb.ap())
        dma(selB32[:, :], d_selB32.ap())
        dma(maskG[:, :], d_maskG.ap())
        dma(I64[:, :], d_I64.ap())
        dma(I128[:, :], d_I128.ap())
        dma(selK32[0:K, :], d_selK32.ap())
        dma(mask24[0:R, :], d_mask24.ap())
        dma(ones_r[0:1, :], d_ones.ap())
        dma(P64h[:, :], d_P64h.ap())
        dma(I128h[:, :], d_I128h.ap())
        for b in range(BL):
            dma(bvec[b * L:(b + 1) * L, 0:1], bass.AP(d_bout, 0, [[1, L], [1, 1]]))

        def hts(hT_tile, c):
            if c < 3:
                return hT_tile[:, c * 128:c * 128 + RP]
            return hT_tile[:, (c - 3) * 128 + RP:(c - 2) * 128]

        def emit_transposes(h_tile, out_psum):
            for cc in range(3):
                nc.tensor.transpose(out_psum[:, cc * 128:(cc + 1) * 128],
                                    h_tile[:, cc * 128:(cc + 1) * 128],
                                    I128h[:, :])

        def stage(t, par):
            """Copy sT_t and qT_{max(t-1,0)} slices into rhs2 parity par."""
            tq = max(t - 1, 0)
            mv = dma if stage_dma else pool.tensor_copy
            for i in range(2):
                mv(bass.AP(rhs2.tensor,
                           rhs2.offset + par * HC * BLK + i * FW + GZ,
                           [[2 * HC * BLK, 128], [BLK, HC], [1, 8]]),
                   bass.AP(sT.tensor, sT.offset + t * 8,
                           [[HC * NR, 128], [NR, HC], [1, 8]]))
                mv(bass.AP(rhs2.tensor,
                           rhs2.offset + par * HC * BLK + i * FW + GZ + 8,
                           [[2 * HC * BLK, 128], [BLK, HC], [1, 8]]),
                   bass.AP(qT.tensor, qT.offset + tq * 8,
                           [[HC * NR, 128], [NR, HC], [1, 8]]))

        def mask_reduce(out_sc, in_ap, msk, accum, eng=None):
            eng = eng or nc.vector
            if use_ttr:
                nc.vector.tensor_tensor_reduce(
                    out_sc, in_ap, msk, 1.0, 0.0, ALU.mult, ALU.add, accum)
            else:
                eng.tensor_mul(out_sc, in_ap, msk)
                eng.tensor_reduce(accum, out_sc,
                                  mybir.AxisListType.X, ALU.add)

        def emit_bias(bP, tn):
            """biasP for step tn: keyV broadcast + s_tn @ W broadcast."""
            m, tb = tn % 16, (tn * BL) // 128
            nc.tensor.matmul(bP[0:RP, :], selK16[0:K, :], keyV[0:K, 0:HH],
                             start=True, stop=False, skip_group_check=True)
            nc.tensor.matmul(bP[64:128, :], selK16[0:K, :], keyV[0:K, HH:H],
                             start=True, stop=False, skip_group_check=True)
            lsel = selB32[:, m * RP:(m + 1) * RP]
            nc.tensor.matmul(bP[0:RP, :], lsel, sW[:, tb * H:tb * H + HH],
                             start=False, stop=True, skip_group_check=True)
            nc.tensor.matmul(bP[64:128, :], lsel, sW[:, tb * H + HH:(tb + 1) * H],
                             start=False, stop=True, skip_group_check=True)

        with tc.tile_pool(name="pr32", bufs=2) as p32, \
             tc.tile_pool(name="prps", bufs=4, space="PSUM") as pps:

            def ptile(shape, dt):
                return pps.tile(shape, dt, tag="ps", name="ps")

            # keys
            kn = p32.tile([128, H], F32, tag="kn")
            dma(kn[0:K, :], d_keys.ap())
            nc.vector.tensor_copy(keys16[0:K, :], kn[0:K, :])
            # U (chunked fp32 load -> cast-copied into both parities of rhs2)
            for c in range(HC):
                u1 = p32.tile([128, H], F32, tag="u1")
                dma(u1[:, :], d_U.ap()[c * 128:(c + 1) * 128, :])
                for par in range(2):
                    for i in range(2):
                        base = par * HC * BLK + c * BLK + i * FW
                        dst = bass.AP(rhs2.tensor, rhs2.offset + base,
                                      [[2 * HC * BLK, 128], [1, GZ]])
                        nc.vector.tensor_copy(
                            dst, u1[:, i * HH:i * HH + GZ])
                        dst = bass.AP(rhs2.tensor, rhs2.offset + base + GA,
                                      [[2 * HC * BLK, 128], [1, HH - GZ]])
                        nc.vector.tensor_copy(
                            dst, u1[:, i * HH + GZ:(i + 1) * HH])
            # keysT via PE transpose
            tps_ = ptile([128, HC * 8], F16)
            for c in range(HC):
                nc.tensor.transpose(tps_[:, c * 8:c * 8 + K],
                                    keys16[0:K, c * 128:(c + 1) * 128],
                                    I64[0:K, 0:K])
            for c in range(HC):
                nc.vector.tensor_copy(keysT[:, c * 8:c * 8 + K],
                                      tps_[:, c * 8:c * 8 + K])
            # V -> keyV = keys @ V  (chunked)
            kvp0 = ptile([128, HH], F32)
            kvp1 = ptile([128, HH], F32)
            for c in range(HC):
                v1 = p32.tile([128, H], F32, tag="u1")
                dma(v1[:, :], d_V.ap()[c * 128:(c + 1) * 128, :])
                v16c = p32.tile([128, H], F16, tag="v16c")
                nc.vector.tensor_copy(v16c[:, :], v1[:, :])
                nc.tensor.matmul(kvp0[0:K, :], keysT[:, c * 8:c * 8 + K],
                                 v16c[:, 0:HH],
                                 start=(c == 0), stop=(c == HC - 1))
                nc.tensor.matmul(kvp1[0:K, :], keysT[:, c * 8:c * 8 + K],
                                 v16c[:, HH:H],
                                 start=(c == 0), stop=(c == HC - 1))
            nc.vector.tensor_copy(keyV[0:K, 0:HH], kvp0[0:K, :])
            nc.vector.tensor_copy(keyV[0:K, HH:H], kvp1[0:K, :])
            # W (fp16) for sW matmuls (chunked load)
            w16 = p32.tile([128, HC * H], F16, tag="big16w")
            for c in range(HC):
                w1 = p32.tile([128, H], F32, tag="u1")
                dma(w1[:, :], d_W.ap()[c * 128:(c + 1) * 128, :])
                nc.vector.tensor_copy(w16[:, c * H:(c + 1) * H], w1[:, :])

            # alpha -> all partitions
            asb = p32.tile([128, 1], F32, tag="asb")
            dma(asb[0:1, 0:1], bass.AP(d_alpha, 0, [[1, 1], [1, 1]]))
            alp = ptile([128, 1], F32)
            nc.tensor.matmul(alp[:, :], ones_r[0:1, :], asb[0:1, 0:1],
                             start=True, stop=True)
            nc.vector.tensor_copy(alphav[:, :], alp[:, :])
            nc.vector.tensor_scalar(av_n[:, :], alphav[:, :], -0.5, 0.5,
                                    ALU.mult, ALU.add)
            nc.vector.tensor_scalar(av_p[:, :], alphav[:, :], 0.5, 0.5,
                                    ALU.mult, ALU.add)

            # W_out -> Wsel
            wo = p32.tile([128, L], F32, tag="wo")
            dma(wo[0:K, :], d_Wout.ap())
            wrp = ptile([128, L], F32)
            nc.tensor.matmul(wrp[0:R, :], selK32[0:K, 0:R], wo[0:K, :],
                             start=True, stop=True)
            wrs = p32.tile([128, L], F32, tag="wrs")
            nc.vector.tensor_copy(wrs[0:R, :], wrp[0:R, :])
            for b in range(BL):
                nc.vector.tensor_mul(Wsel[0:R, b * L:(b + 1) * L],
                                     wrs[0:R, :], mask24[0:R, b * L:(b + 1) * L])

            # features: DMA, transpose (and sW for the sentence stream)
            def do_feat(dram, dstT, with_sw):
                for tb in range(ntt):
                    fn = p32.tile([128, H], F32, tag="fnat")
                    nrow = min(128, NR - tb * 128)
                    src = bass.AP(dram, tb * 128 * H, [[H, nrow], [1, H]])
                    dma(fn[0:nrow, :], src)
                    for grp in range(2):
                        tp = ptile([128, 3 * 128], F32)
                        for j in range(3):
                            c = grp * 3 + j
                            nc.tensor.transpose(
                                tp[:, j * nrow:(j + 1) * nrow],
                                fn[0:nrow, c * 128:(c + 1) * 128],
                                I128[0:nrow, 0:nrow])
                        dst = bass.AP(
                            dstT.tensor,
                            dstT.offset + (grp * 3) * NR + tb * 128,
                            [[HC * NR, 128], [NR, 3], [1, nrow]])
                        nc.vector.tensor_copy(
                            dst,
                            tp[:, 0:3 * nrow].rearrange("p (a b) -> p a b", a=3))
                    if with_sw:
                        for i in range(2):
                            swp = ptile([128, HH], F32)
                            for c in range(HC):
                                lhs = dstT[:, c * NR + tb * 128:
                                           c * NR + tb * 128 + nrow]
                                nc.tensor.matmul(
                                    swp[0:nrow, :], lhs,
                                    w16[:, c * H + i * HH:c * H + (i + 1) * HH],
                                    start=(c == 0), stop=(c == HC - 1))
                            nc.vector.tensor_copy(
                                sW[0:nrow, tb * H + i * HH:tb * H + (i + 1) * HH],
                                swp[0:nrow, :])

            if NR < 128:
                nc.vector.memset(sW[:, :], 0.0)
            do_feat(d_fs, sT, True)
            do_feat(d_fe, qT, False)

            # sK = keys @ s^T  [5, NR]
            for q in range((NR + 511) // 512):
                ncol = min(512, NR - q * 512)
                skp = ptile([128, 512], F32)
                for c in range(HC):
                    nc.tensor.matmul(
                        skp[0:K, 0:ncol], keysT[:, c * 8:c * 8 + K],
                        sT[:, c * NR + q * 512:c * NR + q * 512 + ncol],
                        start=(c == 0), stop=(c == HC - 1))
                nc.vector.tensor_copy(sK[0:K, q * 512:q * 512 + ncol],
                                      skp[0:K, 0:ncol])

            # sKall[p, t] = sK[k(p), t*8 + b(p)]  (gate content bias per row)
            skap = ptile([128, nsteps], F32)
            sKr = sK[:, :].rearrange("p (t b) -> p b t", b=BL)
            for b in range(BL):
                sel = selKb16[0:K, b * RP:(b + 1) * RP]
                src = sKr[0:K, b:b + 1, 0:nsteps]
                nc.tensor.matmul(skap[0:RP, :], sel, src,
                                 start=(b == 0), stop=(b == BL - 1),
                                 skip_group_check=True)
                nc.tensor.matmul(skap[64:128, :], sel, src,
                                 start=(b == 0), stop=(b == BL - 1),
                                 skip_group_check=True)
            nc.vector.tensor_copy(sKall[:, :], skap[:, :])

            # hu_0 = h0 = keys broadcast to (k,b) rows (zero into pad rows)
            h0p = ptile([128, HH], F32)
            nc.tensor.matmul(h0p[0:RP, :], selK16[0:K, :], keys16[0:K, 0:HH],
                             start=True, stop=True, skip_group_check=True)
            nc.tensor.matmul(h0p[64:128, :], selK16[0:K, :], keys16[0:K, HH:H],
                             start=True, stop=True, skip_group_check=True)
            hu_cur = p_hu.tile([128, HH], F16, tag="hu")
            nc.vector.tensor_copy(hu_cur[:, :], h0p[:, :])

            tp0 = ptile([128, 3 * 128], F16)
            emit_transposes(hu_cur, tp0)
            hT_cur = p_hT.tile([128, 3 * 128], F16, tag="hT")
            nc.vector.tensor_copy(hT_cur[:, :], tp0[:, :])

        # PSUM pools for the main loop (opened after the prologue pool frees)
        lctx = ExitStack()
        lep = lctx.enter_context
        p_zpa = lep(tc.tile_pool(name="zpa", bufs=2, space="PSUM"))
        p_zpb = lep(tc.tile_pool(name="zpb", bufs=2, space="PSUM"))
        p_bps = lep(tc.tile_pool(name="bps", bufs=2, space="PSUM"))
        p_tps = lep(tc.tile_pool(name="tps", bufs=1, space="PSUM"))
        p_sps = lep(tc.tile_pool(name="sps", bufs=1, space="PSUM"))

        # rn_0 = 1, h_mat_0 = hu_0
        rn_cur = p_rn.tile([128, 1], F32, tag="rn")
        nc.vector.memset(rn_cur[:, :], 1.0)
        nc.vector.memset(scores[:, :], 0.0)
        hm_cur = p_hm.tile([128, HH], F16, tag="hm")
        nc.vector.tensor_copy(hm_cur[:, :], hu_cur[:, :])

        # stage steps 0 and 1; bias for step 0
        stage(0, 0)
        if nsteps > 1:
            stage(1, 1)
        bP0 = p_bps.tile([128, HH], F32, tag="b")
        emit_bias(bP0, 0)
        b16_cur = p_b16.tile([128, HH], F16, tag="b16")
        nc.scalar.copy(b16_cur[:, :], bP0[:, :])
        if nsteps > 1:
            bP_next = p_bps.tile([128, HH], F32, tag="b")
            emit_bias(bP_next, 1)
        else:
            bP_next = None

        # ---- main loop ----
        rn_next = hm_next = None
        for t in range(nsteps):
            par = t % 2
            if t > 0:
                rn_cur, hm_cur = rn_next, hm_next

            # fused z/g/q block: group A (z 0:192 + g + q) first so its
            # combine/prelu overlaps group B's matmuls
            zPA = p_zpa.tile([128, GA], F32, tag="za")
            zPB = p_zpb.tile([128, HH - GZ], F32, tag="zb")
            for c in range(HC):
                lhs = hts(hT_cur, c)
                base = par * HC * BLK + c * BLK
                nc.tensor.matmul(zPA[0:RP, :], lhs,
                                 rhs2[:, base:base + GA],
                                 start=(c == 0), stop=(c == HC - 1),
                                 skip_group_check=True)
                nc.tensor.matmul(zPA[64:128, :], lhs,
                                 rhs2[:, base + FW:base + FW + GA],
                                 start=(c == 0), stop=(c == HC - 1),
                                 skip_group_check=True)
            for c in range(HC):
                lhs = hts(hT_cur, c)
                base = par * HC * BLK + c * BLK
                nc.tensor.matmul(zPB[0:RP, :], lhs,
                                 rhs2[:, base + GA:base + FW],
                                 start=(c == 0), stop=(c == HC - 1),
                                 skip_group_check=True)
                nc.tensor.matmul(zPB[64:128, :], lhs,
                                 rhs2[:, base + FW + GA:base + BLK],
                                 start=(c == 0), stop=(c == HC - 1),
                                 skip_group_check=True)

            # off-path work: bias t+2 fills the post-block PE idle window;
            # the PSUM->SBUF copy of bias_{t+1} runs early in the ACT queue
            if t + 2 < nsteps:
                stage(t + 2, t % 2)
                bP_next2 = p_bps.tile([128, HH], F32, tag="b")
                emit_bias(bP_next2, t + 2)
            else:
                bP_next2 = None
            if t + 1 < nsteps:
                b16_new = p_b16.tile([128, HH], F16, tag="b16")
                nc.scalar.copy(b16_new[:, :], bP_next[:, :])
            else:
                b16_new = b16_cur
            bP_next = bP_next2

            # gate: masked reduce + sigmoid(rn*gpre + s.keys)
            gsc = p_sml.tile([128, BL], F16, tag="gsc")
            gpre = p_sml.tile([128, 1], F32, tag="gpre")
            mask_reduce(gsc[:, :], zPA[:, GZ:GZ + 8], maskG[:, :],
                        gpre[:, :])
            gsig = p_sml.tile([128, 1], F32, tag="gsig")
            nc.scalar.activation(gsig[:, :], gpre[:, :], ACTF.Sigmoid,
                                 bias=sKall[:, t:t + 1], scale=rn_cur[:, :])

            # z = rn*zU + bias; cand = g*prelu(z)
            # zP col j<GZ -> z col j; col GA+j -> z col GZ+j
            zs16 = p_e16.tile([128, HH], F16, tag="zs16")
            z16 = p_e16.tile([128, HH], F16, tag="z16")
            nc.vector.tensor_scalar(zs16[:, 0:GZ], zPA[:, 0:GZ], rn_cur[:, :],
                                    None, ALU.mult)
            nc.vector.tensor_add(z16[:, 0:GZ], zs16[:, 0:GZ],
                                 b16_cur[:, 0:GZ])
            nc.vector.tensor_scalar(zs16[:, GZ:HH], zPB[:, :],
                                    rn_cur[:, :], None, ALU.mult)
            nc.vector.tensor_add(z16[:, GZ:HH], zs16[:, GZ:HH],
                                 b16_cur[:, GZ:HH])
            cand = p_e16.tile([128, HH], F16, tag="cand")
            hu_new = p_hu.tile([128, HH], F16, tag="hu")
            if use_prelu:
                al = alphav[:, :] if alpha_const is None else float(alpha_const)
                nc.scalar.activation(cand[:, 0:GZ], z16[:, 0:GZ], ACTF.Prelu,
                                     scale=gsig[:, :], alpha=al)
                nc.scalar.activation(cand[:, GZ:HH], z16[:, GZ:HH],
                                     ACTF.Prelu, scale=gsig[:, :], alpha=al)
                # hu_{t+1} = h_t + cand
                nc.vector.tensor_add(hu_new[:, :], hm_cur[:, :], cand[:, :])
            else:
                # sim fallback: g*prelu(z) = g(1-a)/2*|z| + g(1+a)/2*z
                ca = p_sml.tile([128, 1], F32, tag="ca")
                cb = p_sml.tile([128, 1], F32, tag="cb")
                nc.vector.tensor_mul(ca[:, :], gsig[:, :], av_n[:, :])
                nc.vector.tensor_mul(cb[:, :], gsig[:, :], av_p[:, :])
                nc.scalar.activation(cand[:, :], z16[:, :], ACTF.Abs,
                                     scale=ca[:, :])
                bv = p_e16.tile([128, HH], F16, tag="bv")
                nc.vector.tensor_scalar(bv[:, :], z16[:, :], cb[:, :], None,
                                        ALU.mult)
                nc.vector.tensor_add(hu_new[:, :], hm_cur[:, :], cand[:, :])
                nc.vector.tensor_add(hu_new[:, :], hu_new[:, :], bv[:, :])

            # transposes -> huT_{t+1}
            tP = p_tps.tile([128, 3 * 128], F16, tag="t")
            emit_transposes(hu_new, tP)
            hT_new = p_hT.tile([128, 3 * 128], F16, tag="hT")
            nc.vector.tensor_copy(hT_new[:, :], tP[:, :])

            # norm accumulators for rn_{t+1}
            squ = p_e16.tile([128, HH], F16, tag="squ")
            ss_n = p_sml.tile([128, 1], F32, tag="ss")
            nc.scalar.activation(squ[:, :], hu_new[:, :], ACTF.Square,
                                 accum_out=ss_n[:, :])
            ss16_n = p_sml.tile([128, 1], F16, tag="ss16")
            pool.tensor_copy(ss16_n[:, :], ss_n[:, :])

            # rn_{t+1} = rsqrt(fold(ss)): PE fold, DVE fast-inv-sqrt + 1 NR
            ssp = p_sps.tile([128, 1], F32, tag="ssp")
            nc.tensor.matmul(ssp[:, :], P64h[:, :], ss16_n[:, :],
                             start=True, stop=True)
            sdi = p_sml.tile([128, 1], I32, tag="sdi")
            nc.vector.tensor_scalar(sdi[:, :], ssp.bitcast(I32)[:, :], 1,
                                    None, ALU.logical_shift_right)
            nc.vector.tensor_scalar(sdi[:, :], sdi[:, :], -1, 0x5F3759DF,
                                    ALU.mult, ALU.add)
            rn_w = sdi.bitcast(F32)
            ra = p_sml.tile([128, 1], F32, tag="ra")
            rn_next = p_rn.tile([128, 1], F32, tag="rn")
            for it in range(newton_iters):
                nc.vector.tensor_mul(ra[:, :], rn_w[:, :], ssp[:, :])
                nc.vector.tensor_mul(ra[:, :], ra[:, :], rn_w[:, :])
                nc.vector.tensor_scalar(ra[:, :], ra[:, :], -0.5, 1.5,
                                        ALU.mult, ALU.add)
                dst = rn_next if it == newton_iters - 1 else rn_w
                nc.vector.tensor_mul(dst[:, :], rn_w[:, :], ra[:, :])
            hm_next = p_hm.tile([128, HH], F16, tag="hm")
            nc.vector.tensor_scalar(hm_next[:, :], hu_new[:, :],
                                    rn_next[:, :], None, ALU.mult)

            # score for step t-1: rn_t * masked-reduce(q-gram cols)
            if t > 0:
                qsc = p_sml.tile([128, BL], F16, tag="qsc")
                qtmp = p_sml.tile([128, 1], F32, tag="qtmp")
                mask_reduce(qsc[:, :], zPA[:, GZ + 8:GA], maskG[:, :],
                            qtmp[:, :])
                pool.tensor_scalar(scores[:, t - 1:t], qtmp[:, :],
                                        rn_cur[:, :], None, ALU.mult)

            if debug:
                dma(bass.AP(d_dhu, t * 128 * HH, [[HH, 128], [1, HH]]),
                    hu_new[:, :])
                dma(bass.AP(d_drn, t * 128, [[1, 128], [1, 1]]), rn_cur[:, :])
                dma(bass.AP(d_dg, t * 128, [[1, 128], [1, 1]]), gsig[:, :])
                dma(bass.AP(d_dz, t * 128 * HH, [[HH, 128], [1, HH]]),
                    z16[:, :])

            hu_cur, hT_cur, b16_cur = hu_new, hT_new, b16_new

        # ---- epilogue ----
        lctx.close()
        p_eps = ep(tc.tile_pool(name="eps", bufs=1, space="PSUM"))
        rn_T = rn_next

        # final q-gram: q_{T-1} . hu_T
        qPf = p_eps.tile([128, BL], F32, tag="qf")
        for c in range(HC):
            nc.tensor.matmul(
                qPf[0:RP, 0:BL], hts(hT_cur, c),
                qT[:, c * NR + (nsteps - 1) * 8:c * NR + nsteps * 8],
                start=(c == 0), stop=(c == HC - 1))
        qsc = p_sml.tile([128, BL], F16, tag="qsc")
        qtmp = p_sml.tile([128, 1], F32, tag="qtmp")
        mask_reduce(qsc[0:RP, :], qPf[0:RP, 0:BL], maskG[0:RP, :],
                    qtmp[0:RP, :])
        pool.tensor_scalar(scores[0:RP, nsteps - 1:nsteps],
                                qtmp[0:RP, :], rn_T[0:RP, :], None, ALU.mult)

        # output head
        pP = p_eps.tile([128, nsteps], F32, tag="pp")
        nc.tensor.matmul(pP[0:BL * L, :], Wsel[0:R, 0:BL * L],
                         scores[0:R, 0:nsteps], start=True, stop=True)
        osb = p_prm.tile([128, nsteps], F32, tag="osb")
        nc.vector.tensor_scalar(osb[0:BL * L, :], pP[0:BL * L, :],
                                bvec[0:BL * L, :], None, ALU.add)
        nc.sync.dma_start(bass.AP(d_out, 0, [[1, BL * L], [BL * L, nsteps]]),
                          osb[0:BL * L, :])
        if debug:
            dma(bass.AP(d_dsc, 0, [[nsteps, 128], [1, nsteps]]),
                scores[:, :])

    nc.compile()
    return nc


_CACHE = {}


def _get(nsteps, debug=False, use_prelu=True, **kw):
    key = (nsteps, debug, use_prelu) + tuple(sorted(kw.items()))
    if key not in _CACHE:
        _CACHE[key] = _build(nsteps, debug=debug, use_prelu=use_prelu, **kw)
    return _CACHE[key]


def _in_maps(inputs, nsteps):
    consts = _host_consts()
    fs = np.ascontiguousarray(np.asarray(inputs["features_sentence"], dtype=np.float32))
    fe = np.ascontiguousarray(np.asarray(inputs["features_entity"], dtype=np.float32))
    shared = {k: np.ascontiguousarray(np.asarray(inputs[k], dtype=np.float32))
              for k in ("keys", "U", "V", "W", "alpha", "W_out", "b_out")}
    shared.update(consts)
    in_maps = []
    for c in range(NC):
        m = dict(shared)
        m["features_sentence"] = np.ascontiguousarray(fs[:, c * BL:(c + 1) * BL, :])
        m["features_entity"] = np.ascontiguousarray(fe[:, c * BL:(c + 1) * BL, :])
        in_maps.append(m)
    return in_maps


def kernel(**inputs):
    nsteps = inputs["features_sentence"].shape[0]
    nc = _get(nsteps, alpha_const=float(np.asarray(inputs["alpha"]).ravel()[0]))
    res = run_bass_kernel_spmd(nc, _in_maps(inputs, nsteps),
                               core_ids=list(range(NC)))
    outs = [r["preds"].reshape(nsteps, BL, L) for r in res.results]
    return np.concatenate(outs, axis=1).reshape(nsteps * B, L)


# revision 28
# speedup vs baseline: 1.8774x; 1.0564x over previous
"""EntNetHead Trainium2 kernel (v2: deferred normalization).

Data-parallel over batch B=64 across 8 NeuronCores (8 batch rows per core);
T=256 recurrent steps run on-chip per core.

Key idea vs v1: the per-step L2 normalization is linear, so the state is
kept UNNORMALIZED (hu) together with rn = 1/||hu|| per packed row.  The
matmuls use huT directly; rn folds into (a) the sigmoid's per-partition
scale, (b) one DVE scale of the z PSUM, (c) a lazily-materialized
h = rn*hu used by the state update.  The whole square/fold/rsqrt chain
runs OFF the critical path (ACT/Pool/PE idle slots).

Per-step PE block is 12 matmuls: the g-gram (s_t . hu) and q-gram
(q_{t-1} . hu) are fused into the z matmuls as 16 extra rhs columns,
staged next to the U chunks by the (otherwise idle) DMA engines.  The
gate's content bias s_t.keys is precomputed for all t as a per-partition
column (sKall) and enters through the sigmoid's bias AP. The z bias
keyV + s_t@W accumulates in a separate PSUM bank off the critical path.

Engine split per step:
  PE:   12 fused z/g/q matmuls, 4 bias matmuls (t+1), 3 transposes,
        1 fp16 norm-fold matmul
  DVE:  gate reduce, z-scale, z-add(bias), hu update, hT copy,
        q reduce, rsqrt seed
  ACT:  sigmoid, prelu, square+accum, bias PSUM->SBUF copy
  Pool: Newton rsqrt, h materialize, score scale, ss fp16 cast
  DMA:  per-step staging of sT/qT slices into the fused rhs buffer
"""

import sys

sys.path.insert(0, "/opt/trn_rl_repo")

from contextlib import ExitStack

import numpy as np

import concourse.bacc as bacc
import concourse.bass as bass
import concourse.tile as tile
from concourse import mybir
from concourse.bass_utils import run_bass_kernel_spmd

F32 = mybir.dt.float32
F16 = mybir.dt.float16
I32 = mybir.dt.int32
ALU = mybir.AluOpType
ACTF = mybir.ActivationFunctionType

T, B, H, K, L = 256, 64, 768, 5, 3
NC = 8
BL = B // NC          # 8 batch rows per core
R = K * BL            # 40 (k,b) rows
RP = 64               # padded stationary width
HC = H // 128         # 6 contraction chunks
HH = H // 2           # 384
BLK = 2 * (HH + 16)   # 800: per-chunk fused rhs block, 2 halves of FW
FW = HH + 16          # 400: fused rhs width per half: [U 0:192|sT|qT|U 192:384]
GA = 208              # group-A matmul width (192 z cols + 8 g + 8 q)
GZ = 192              # z cols in group A


def _host_consts():
    selK = np.zeros((K, RP), np.float16)
    for k in range(K):
        selK[k, k * BL:(k + 1) * BL] = 1.0
    selKb = np.zeros((K, 8 * RP), np.float16)
    for b in range(BL):
        for k in range(K):
            selKb[k, b * RP + k * BL + b] = 1.0
    selB = np.zeros((128, 16 * RP), np.float16)
    for m in range(16):
        for b in range(BL):
            for k in range(K):
                selB[m * BL + b, m * RP + k * BL + b] = 1.0
    maskG = np.zeros((128, BL), np.float16)
    for p in range(128):
        maskG[p, p % BL] = 1.0
    I64 = np.zeros((128, RP), np.float16)
    for j in range(RP):
        I64[j, j] = 1.0
        I64[64 + j, j] = 1.0
    I128 = np.eye(128, dtype=np.float32)
    mask24 = np.zeros((R, BL * L), np.float32)
    for k in range(K):
        for b in range(BL):
            mask24[k * BL + b, b * L:(b + 1) * L] = 1.0
    ones1x128 = np.ones((1, 128), np.float32)
    P64h = np.zeros((128, 128), np.float16)
    for i in range(128):
        P64h[i, i % 64] = 1.0
        P64h[i, i % 64 + 64] = 1.0
    selK32 = selK.astype(np.float32)
    return {
        "c_selK": selK, "c_selKb": selKb, "c_selB32": selB, "c_maskG": maskG,
        "c_I64": I64, "c_I128": I128, "c_selK32": selK32,
        "c_mask24": mask24, "c_ones": ones1x128, "c_P64h": P64h,
        "c_I128h": np.eye(128, dtype=np.float16),
    }


def _build(nsteps, debug=False, use_prelu=True,
           stage_dma=True, use_pool=True, use_ttr=False, newton_iters=1,
           alpha_const=None):
    # use_ttr: InstTensorTensorReduce wedges TRN2 here (NRT_EXEC_UNIT_
    # UNRECOVERABLE) despite passing CoreSim -- keep the 2-op fallback.
    nc = bacc.Bacc("TRN2", target_bir_lowering=False, debug=False)
    NR = nsteps * BL      # feature rows per core
    ntt = (NR + 127) // 128

    d_fs = nc.dram_tensor("features_sentence", [nsteps, BL, H], F32, kind="ExternalInput")
    d_fe = nc.dram_tensor("features_entity", [nsteps, BL, H], F32, kind="ExternalInput")
    d_keys = nc.dram_tensor("keys", [K, H], F32, kind="ExternalInput")
    d_U = nc.dram_tensor("U", [H, H], F32, kind="ExternalInput")
    d_V = nc.dram_tensor("V", [H, H], F32, kind="ExternalInput")
    d_W = nc.dram_tensor("W", [H, H], F32, kind="ExternalInput")
    d_alpha = nc.dram_tensor("alpha", [1], F32, kind="ExternalInput")
    d_Wout = nc.dram_tensor("W_out", [K, L], F32, kind="ExternalInput")
    d_bout = nc.dram_tensor("b_out", [L], F32, kind="ExternalInput")
    d_selK = nc.dram_tensor("c_selK", [K, RP], F16, kind="ExternalInput")
    d_selKb = nc.dram_tensor("c_selKb", [K, 8 * RP], F16, kind="ExternalInput")
    d_selB32 = nc.dram_tensor("c_selB32", [128, 16 * RP], F16, kind="ExternalInput")
    d_maskG = nc.dram_tensor("c_maskG", [128, BL], F16, kind="ExternalInput")
    d_I64 = nc.dram_tensor("c_I64", [128, RP], F16, kind="ExternalInput")
    d_I128 = nc.dram_tensor("c_I128", [128, 128], F32, kind="ExternalInput")
    d_selK32 = nc.dram_tensor("c_selK32", [K, RP], F32, kind="ExternalInput")
    d_mask24 = nc.dram_tensor("c_mask24", [R, BL * L], F32, kind="ExternalInput")
    d_ones = nc.dram_tensor("c_ones", [1, 128], F32, kind="ExternalInput")
    d_P64h = nc.dram_tensor("c_P64h", [128, 128], F16, kind="ExternalInput")
    d_I128h = nc.dram_tensor("c_I128h", [128, 128], F16, kind="ExternalInput")
    d_out = nc.dram_tensor("preds", [NR, L], F32, kind="ExternalOutput")
    if debug:
        d_dhu = nc.dram_tensor("dbg_hu", [nsteps + 1, 128, HH], F16, kind="ExternalOutput")
        d_drn = nc.dram_tensor("dbg_rn", [nsteps + 1, 128], F32, kind="ExternalOutput")
        d_dg = nc.dram_tensor("dbg_g", [nsteps, 128], F32, kind="ExternalOutput")
        d_dz = nc.dram_tensor("dbg_z", [nsteps, 128, HH], F16, kind="ExternalOutput")
        d_dsc = nc.dram_tensor("dbg_sc", [128, nsteps], F32, kind="ExternalOutput")

    with tile.TileContext(nc) as tc, ExitStack() as ctx:
        ep = ctx.enter_context
        pool = nc.gpsimd if use_pool else nc.vector

        p_sT = ep(tc.tile_pool(name="sT", bufs=1))
        p_qT = ep(tc.tile_pool(name="qT", bufs=1))
        p_sW = ep(tc.tile_pool(name="sW", bufs=1))
        p_rhs = ep(tc.tile_pool(name="rhs", bufs=1))
        p_prm = ep(tc.tile_pool(name="prm", bufs=1))
        p_hu = ep(tc.tile_pool(name="hu", bufs=2))
        p_hm = ep(tc.tile_pool(name="hm", bufs=2))
        p_hT = ep(tc.tile_pool(name="hT", bufs=2))
        p_rn = ep(tc.tile_pool(name="rn", bufs=2))
        p_b16 = ep(tc.tile_pool(name="b16", bufs=2))
        p_e16 = ep(tc.tile_pool(name="e16", bufs=2))
        p_sml = ep(tc.tile_pool(name="sml", bufs=3))

        sT = p_sT.tile([128, HC * NR], F16)   # [h%128, c*NR + t*8+b]
        qT = p_qT.tile([128, HC * NR], F16)
        sW = p_sW.tile([128, ntt * H], F16)   # [row%128, tile*768+h]
        rhs2 = p_rhs.tile([128, 2 * HC * BLK], F16)  # fused rhs, 2 parities
        keyV = p_prm.tile([128, H], F16, tag="keyV")
        keys16 = p_prm.tile([128, H], F16, tag="keys16")
        keysT = p_prm.tile([128, HC * 8], F16, tag="keysT")
        sK = p_prm.tile([128, NR], F16, tag="sK")
        sKall = p_prm.tile([128, nsteps], F32, tag="sKall")
        selK16 = p_prm.tile([128, RP], F16, tag="selK16")
        selKb16 = p_prm.tile([128, 8 * RP], F16, tag="selKb16")
        selB32 = p_prm.tile([128, 16 * RP], F16, tag="selB32")
        maskG = p_prm.tile([128, BL], F16, tag="maskG")
        I64 = p_prm.tile([128, RP], F16, tag="I64")
        I128 = p_prm.tile([128, 128], F32, tag="I128")
        selK32 = p_prm.tile([128, RP], F32, tag="selK32")
        mask24 = p_prm.tile([128, BL * L], F32, tag="mask24")
        ones_r = p_prm.tile([128, 128], F32, tag="ones_r")
        P64h = p_prm.tile([128, 128], F16, tag="P64h")
        I128h = p_prm.tile([128, 128], F16, tag="I128h")
        alphav = p_prm.tile([128, 1], F32, tag="alphav")
        av_n = p_prm.tile([128, 1], F32, tag="av_n")
        av_p = p_prm.tile([128, 1], F32, tag="av_p")
        Wsel = p_prm.tile([128, BL * L], F32, tag="Wsel")
        bvec = p_prm.tile([128, 1], F32, tag="bvec")
        scores = p_prm.tile([128, nsteps], F32, tag="scores")

        dma = nc.sync.dma_start

        dma(selK16[0:K, :], d_selK.ap())
        dma(selKb16[0:K, :], d_selKb.ap())
        dma(selB32[:, :], d_selB32.ap())
        dma(maskG[:, :], d_maskG.ap())
        dma(I64[:, :], d_I64.ap())
        dma(I128[:, :], d_I128.ap())
        dma(selK32[0:K, :], d_selK32.ap())
        dma(mask24[0:R, :], d_mask24.ap())
        dma(ones_r[0:1, :], d_ones.ap())
        dma(P64h[:, :], d_P64h.ap())
        dma(I128h[:, :], d_I128h.ap())
        for b in range(BL):
            dma(bvec[b * L:(b + 1) * L, 0:1], bass.AP(d_bout, 0, [[1, L], [1, 1]]))

        def hts(hT_tile, c):
            if c < 3:
                return hT_tile[:, c * 128:c * 128 + RP]
            return hT_tile[:, (c - 3) * 128 + RP:(c - 2) * 128]

        def emit_transposes(h_tile, out_psum):
            for cc in range(3):
                nc.tensor.transpose(out_psum[:, cc * 128:(cc + 1) * 128],
                                    h_tile[:, cc * 128:(cc + 1) * 128],
                                    I128h[:, :])

        def stage(t, par):
            """Copy sT_t and qT_{max(t-1,0)} slices into rhs2 parity par."""
            tq = max(t - 1, 0)
            mv = dma if stage_dma else pool.tensor_copy
            for i in range(2):
                mv(bass.AP(rhs2.tensor,
                           rhs2.offset + par * HC * BLK + i * FW + GZ,
                           [[2 * HC * BLK, 128], [BLK, HC], [1, 8]]),
                   bass.AP(sT.tensor, sT.offset + t * 8,
                           [[HC * NR, 128], [NR, HC], [1, 8]]))
                mv(bass.AP(rhs2.tensor,
                           rhs2.offset + par * HC * BLK + i * FW + GZ + 8,
                           [[2 * HC * BLK, 128], [BLK, HC], [1, 8]]),
                   bass.AP(qT.tensor, qT.offset + tq * 8,
                           [[HC * NR, 128], [NR, HC], [1, 8]]))

        def mask_reduce(out_sc, in_ap, msk, accum, eng=None):
            eng = eng or nc.vector
            if use_ttr:
                nc.vector.tensor_tensor_reduce(
                    out_sc, in_ap, msk, 1.0, 0.0, ALU.mult, ALU.add, accum)
            else:
                eng.tensor_mul(out_sc, in_ap, msk)
                eng.tensor_reduce(accum, out_sc,
                                  mybir.AxisListType.X, ALU.add)

        def emit_bias(bP, tn):
            """biasP for step tn: keyV broadcast + s_tn @ W broadcast."""
            m, tb = tn % 16, (tn * BL) // 128
            nc.tensor.matmul(bP[0:RP, :], selK16[0:K, :], keyV[0:K, 0:HH],
                             start=True, stop=False, skip_group_check=True)
            nc.tensor.matmul(bP[64:128, :], selK16[0:K, :], keyV[0:K, HH:H],
                             start=True, stop=False, skip_group_check=True)
            lsel = selB32[:, m * RP:(m + 1) * RP]
            nc.tensor.matmul(bP[0:RP, :], lsel, sW[:, tb * H:tb * H + HH],
                             start=False, stop=True, skip_group_check=True)
            nc.tensor.matmul(bP[64:128, :], lsel, sW[:, tb * H + HH:(tb + 1) * H],
                             start=False, stop=True, skip_group_check=True)

        with tc.tile_pool(name="pr32", bufs=2) as p32, \
             tc.tile_pool(name="prps", bufs=4, space="PSUM") as pps:

            def ptile(shape, dt):
                return pps.tile(shape, dt, tag="ps", name="ps")

            # keys
            kn = p32.tile([128, H], F32, tag="kn")
            dma(kn[0:K, :], d_keys.ap())
            nc.vector.tensor_copy(keys16[0:K, :], kn[0:K, :])
            # U (chunked fp32 load -> cast-copied into both parities of rhs2)
            for c in range(HC):
                u1 = p32.tile([128, H], F32, tag="u1")
                dma(u1[:, :], d_U.ap()[c * 128:(c + 1) * 128, :])
                for par in range(2):
                    for i in range(2):
                        base = par * HC * BLK + c * BLK + i * FW
                        dst = bass.AP(rhs2.tensor, rhs2.offset + base,
                                      [[2 * HC * BLK, 128], [1, GZ]])
                        nc.vector.tensor_copy(
                            dst, u1[:, i * HH:i * HH + GZ])
                        dst = bass.AP(rhs2.tensor, rhs2.offset + base + GA,
                                      [[2 * HC * BLK, 128], [1, HH - GZ]])
                        nc.vector.tensor_copy(
                            dst, u1[:, i * HH + GZ:(i + 1) * HH])
            # keysT via PE transpose
            tps_ = ptile([128, HC * 8], F16)
            for c in range(HC):
                nc.tensor.transpose(tps_[:, c * 8:c * 8 + K],
                                    keys16[0:K, c * 128:(c + 1) * 128],
                                    I64[0:K, 0:K])
            for c in range(HC):
                nc.vector.tensor_copy(keysT[:, c * 8:c * 8 + K],
                                      tps_[:, c * 8:c * 8 + K])
            # V -> keyV = keys @ V  (chunked)
            kvp0 = ptile([128, HH], F32)
            kvp1 = ptile([128, HH], F32)
            for c in range(HC):
                v1 = p32.tile([128, H], F32, tag="u1")
                dma(v1[:, :], d_V.ap()[c * 128:(c + 1) * 128, :])
                v16c = p32.tile([128, H], F16, tag="v16c")
                nc.vector.tensor_copy(v16c[:, :], v1[:, :])
                nc.tensor.matmul(kvp0[0:K, :], keysT[:, c * 8:c * 8 + K],
                                 v16c[:, 0:HH],
                                 start=(c == 0), stop=(c == HC - 1))
                nc.tensor.matmul(kvp1[0:K, :], keysT[:, c * 8:c * 8 + K],
                                 v16c[:, HH:H],
                                 start=(c == 0), stop=(c == HC - 1))
            nc.vector.tensor_copy(keyV[0:K, 0:HH], kvp0[0:K, :])
            nc.vector.tensor_copy(keyV[0:K, HH:H], kvp1[0:K, :])
            # W (fp16) for sW matmuls (chunked load)
            w16 = p32.tile([128, HC * H], F16, tag="big16w")
            for c in range(HC):
                w1 = p32.tile([128, H], F32, tag="u1")
                dma(w1[:, :], d_W.ap()[c * 128:(c + 1) * 128, :])
                nc.vector.tensor_copy(w16[:, c * H:(c + 1) * H], w1[:, :])

            # alpha -> all partitions
            asb = p32.tile([128, 1], F32, tag="asb")
            dma(asb[0:1, 0:1], bass.AP(d_alpha, 0, [[1, 1], [1, 1]]))
            alp = ptile([128, 1], F32)
            nc.tensor.matmul(alp[:, :], ones_r[0:1, :], asb[0:1, 0:1],
                             start=True, stop=True)
            nc.vector.tensor_copy(alphav[:, :], alp[:, :])
            nc.vector.tensor_scalar(av_n[:, :], alphav[:, :], -0.5, 0.5,
                                    ALU.mult, ALU.add)
            nc.vector.tensor_scalar(av_p[:, :], alphav[:, :], 0.5, 0.5,
                                    ALU.mult, ALU.add)

            # W_out -> Wsel
            wo = p32.tile([128, L], F32, tag="wo")
            dma(wo[0:K, :], d_Wout.ap())
            wrp = ptile([128, L], F32)
            nc.tensor.matmul(wrp[0:R, :], selK32[0:K, 0:R], wo[0:K, :],
                             start=True, stop=True)
            wrs = p32.tile([128, L], F32, tag="wrs")
            nc.vector.tensor_copy(wrs[0:R, :], wrp[0:R, :])
            for b in range(BL):
                nc.vector.tensor_mul(Wsel[0:R, b * L:(b + 1) * L],
                                     wrs[0:R, :], mask24[0:R, b * L:(b + 1) * L])

            # features: DMA, transpose (and sW for the sentence stream)
            def do_feat(dram, dstT, with_sw):
                for tb in range(ntt):
                    fn = p32.tile([128, H], F32, tag="fnat")
                    nrow = min(128, NR - tb * 128)
                    src = bass.AP(dram, tb * 128 * H, [[H, nrow], [1, H]])
                    dma(fn[0:nrow, :], src)
                    for grp in range(2):
                        tp = ptile([128, 3 * 128], F32)
                        for j in range(3):
                            c = grp * 3 + j
                            nc.tensor.transpose(
                                tp[:, j * nrow:(j + 1) * nrow],
                                fn[0:nrow, c * 128:(c + 1) * 128],
                                I128[0:nrow, 0:nrow])
                        dst = bass.AP(
                            dstT.tensor,
                            dstT.offset + (grp * 3) * NR + tb * 128,
                            [[HC * NR, 128], [NR, 3], [1, nrow]])
                        nc.vector.tensor_copy(
                            dst,
                            tp[:, 0:3 * nrow].rearrange("p (a b) -> p a b", a=3))
                    if with_sw:
                        for i in range(2):
                            swp = ptile([128, HH], F32)
                            for c in range(HC):
                                lhs = dstT[:, c * NR + tb * 128:
                                           c * NR + tb * 128 + nrow]
                                nc.tensor.matmul(
                                    swp[0:nrow, :], lhs,
                                    w16[:, c * H + i * HH:c * H + (i + 1) * HH],
                                    start=(c == 0), stop=(c == HC - 1))
                            nc.vector.tensor_copy(
                                sW[0:nrow, tb * H + i * HH:tb * H + (i + 1) * HH],
                                swp[0:nrow, :])

            if NR < 128:
                nc.vector.memset(sW[:, :], 0.0)
            do_feat(d_fs, sT, True)
            do_feat(d_fe, qT, False)

            # sK = keys @ s^T  [5, NR]
            for q in range((NR + 511) // 512):
                ncol = min(512, NR - q * 512)
                skp = ptile([128, 512], F32)
                for c in range(HC):
                    nc.tensor.matmul(
                        skp[0:K, 0:ncol], keysT[:, c * 8:c * 8 + K],
                        sT[:, c * NR + q * 512:c * NR + q * 512 + ncol],
                        start=(c == 0), stop=(c == HC - 1))
                nc.vector.tensor_copy(sK[0:K, q * 512:q * 512 + ncol],
                                      skp[0:K, 0:ncol])

            # sKall[p, t] = sK[k(p), t*8 + b(p)]  (gate content bias per row)
            skap = ptile([128, nsteps], F32)
            sKr = sK[:, :].rearrange("p (t b) -> p b t", b=BL)
            for b in range(BL):
                sel = selKb16[0:K, b * RP:(b + 1) * RP]
                src = sKr[0:K, b:b + 1, 0:nsteps]
                nc.tensor.matmul(skap[0:RP, :], sel, src,
                                 start=(b == 0), stop=(b == BL - 1),
                                 skip_group_check=True)
                nc.tensor.matmul(skap[64:128, :], sel, src,
                                 start=(b == 0), stop=(b == BL - 1),
                                 skip_group_check=True)
            nc.vector.tensor_copy(sKall[:, :], skap[:, :])

            # hu_0 = h0 = keys broadcast to (k,b) rows (zero into pad rows)
            h0p = ptile([128, HH], F32)
            nc.tensor.matmul(h0p[0:RP, :], selK16[0:K, :], keys16[0:K, 0:HH],
                             start=True, stop=True, skip_group_check=True)
            nc.tensor.matmul(h0p[64:128, :], selK16[0:K, :], keys16[0:K, HH:H],
                             start=True, stop=True, skip_group_check=True)
            hu_cur = p_hu.tile([128, HH], F16, tag="hu")
            nc.vector.tensor_copy(hu_cur[:, :], h0p[:, :])

            tp0 = ptile([128, 3 * 128], F16)
            emit_transposes(hu_cur, tp0)
            hT_cur = p_hT.tile([128, 3 * 128], F16, tag="hT")
            nc.vector.tensor_copy(hT_cur[:, :], tp0[:, :])

        # PSUM pools for the main loop (opened after the prologue pool frees)
        lctx = ExitStack()
        lep = lctx.enter_context
        p_zpa = lep(tc.tile_pool(name="zpa", bufs=2, space="PSUM"))
        p_zpb = lep(tc.tile_pool(name="zpb", bufs=2, space="PSUM"))
        p_bps = lep(tc.tile_pool(name="bps", bufs=2, space="PSUM"))
        p_tps = lep(tc.tile_pool(name="tps", bufs=1, space="PSUM"))
        p_sps = lep(tc.tile_pool(name="sps", bufs=1, space="PSUM"))

        # rn_0 = 1, h_mat_0 = hu_0
        rn_cur = p_rn.tile([128, 1], F32, tag="rn")
        nc.vector.memset(rn_cur[:, :], 1.0)
        nc.vector.memset(scores[:, :], 0.0)
        hm_cur = p_hm.tile([128, HH], F16, tag="hm")
        nc.vector.tensor_copy(hm_cur[:, :], hu_cur[:, :])

        # stage steps 0 and 1; bias for step 0
        stage(0, 0)
        if nsteps > 1:
            stage(1, 1)
        bP0 = p_bps.tile([128, HH], F32, tag="b")
        emit_bias(bP0, 0)
        b16_cur = p_b16.tile([128, HH], F16, tag="b16")
        nc.scalar.copy(b16_cur[:, :], bP0[:, :])
        if nsteps > 1:
            bP_next = p_bps.tile([128, HH], F32, tag="b")
            emit_bias(bP_next, 1)
        else:
            bP_next = None

        # ---- main loop ----
        rn_next = hm_next = None
        for t in range(nsteps):
            par = t % 2
            if t > 0:
                rn_cur, hm_cur = rn_next, hm_next

            # fused z/g/q block: group A (z 0:192 + g + q) first so its
            # combine/prelu overlaps group B's matmuls
            zPA = p_zpa.tile([128, GA], F32, tag="za")
            zPB = p_zpb.tile([128, HH - GZ], F32, tag="zb")
            for c in range(HC):
                lhs = hts(hT_cur, c)
                base = par * HC * BLK + c * BLK
                nc.tensor.matmul(zPA[0:RP, :], lhs,
                                 rhs2[:, base:base + GA],
                                 start=(c == 0), stop=(c == HC - 1),
                                 skip_group_check=True)
                nc.tensor.matmul(zPA[64:128, :], lhs,
                                 rhs2[:, base + FW:base + FW + GA],
                                 start=(c == 0), stop=(c == HC - 1),
                                 skip_group_check=True)
            for c in range(HC):
                lhs = hts(hT_cur, c)
                base = par * HC * BLK + c * BLK
                nc.tensor.matmul(zPB[0:RP, :], lhs,
                                 rhs2[:, base + GA:base + FW],
                                 start=(c == 0), stop=(c == HC - 1),
                                 skip_group_check=True)
                nc.tensor.matmul(zPB[64:128, :], lhs,
                                 rhs2[:, base + FW + GA:base + BLK],
                                 start=(c == 0), stop=(c == HC - 1),
                                 skip_group_check=True)

            # off-path work: bias t+2 fills the post-block PE idle window;
            # the PSUM->SBUF copy of bias_{t+1} runs early in the ACT queue
            if t + 2 < nsteps:
                stage(t + 2, t % 2)
                bP_next2 = p_bps.tile([128, HH], F32, tag="b")
                emit_bias(bP_next2, t + 2)
            else:
                bP_next2 = None
            if t + 1 < nsteps:
                b16_new = p_b16.tile([128, HH], F16, tag="b16")
                nc.scalar.copy(b16_new[:, :], bP_next[:, :])
            else:
                b16_new = b16_cur
            bP_next = bP_next2

            # gate: masked reduce + sigmoid(rn*gpre + s.keys)
            gsc = p_sml.tile([128, BL], F16, tag="gsc")
            gpre = p_sml.tile([128, 1], F32, tag="gpre")
            mask_reduce(gsc[:, :], zPA[:, GZ:GZ + 8], maskG[:, :],
                        gpre[:, :])
            gsig = p_sml.tile([128, 1], F32, tag="gsig")
            nc.scalar.activation(gsig[:, :], gpre[:, :], ACTF.Sigmoid,
                                 bias=sKall[:, t:t + 1], scale=rn_cur[:, :])

            # z = rn*zU + bias; cand = g*prelu(z)
            # zP col j<GZ -> z col j; col GA+j -> z col GZ+j
            zs16 = p_e16.tile([128, HH], F16, tag="zs16")
            z16 = p_e16.tile([128, HH], F16, tag="z16")
            nc.vector.tensor_scalar(zs16[:, 0:GZ], zPA[:, 0:GZ], rn_cur[:, :],
                                    None, ALU.mult)
            nc.vector.tensor_add(z16[:, 0:GZ], zs16[:, 0:GZ],
                                 b16_cur[:, 0:GZ])
            nc.vector.tensor_scalar(zs16[:, GZ:HH], zPB[:, :],
                                    rn_cur[:, :], None, ALU.mult)
            nc.vector.tensor_add(z16[:, GZ:HH], zs16[:, GZ:HH],
                                 b16_cur[:, GZ:HH])
            cand = p_e16.tile([128, HH], F16, tag="cand")
            hu_new = p_hu.tile([128, HH], F16, tag="hu")
            if use_prelu:
                al = alphav[:, :] if alpha_const is None else float(alpha_const)
                nc.scalar.activation(cand[:, 0:GZ], z16[:, 0:GZ], ACTF.Prelu,
                                     scale=gsig[:, :], alpha=al)
                nc.scalar.activation(cand[:, GZ:HH], z16[:, GZ:HH],
                                     ACTF.Prelu, scale=gsig[:, :], alpha=al)
                # hu_{t+1} = h_t + cand
                nc.vector.tensor_add(hu_new[:, :], hm_cur[:, :], cand[:, :])
            else:
                # sim fallback: g*prelu(z) = g(1-a)/2*|z| + g(1+a)/2*z
                ca = p_sml.tile([128, 1], F32, tag="ca")
                cb = p_sml.tile([128, 1], F32, tag="cb")
                nc.vector.tensor_mul(ca[:, :], gsig[:, :], av_n[:, :])
                nc.vector.tensor_mul(cb[:, :], gsig[:, :], av_p[:, :])
                nc.scalar.activation(cand[:, :], z16[:, :], ACTF.Abs,
                                     scale=ca[:, :])
                bv = p_e16.tile([128, HH], F16, tag="bv")
                nc.vector.tensor_scalar(bv[:, :], z16[:, :], cb[:, :], None,
                                        ALU.mult)
                nc.vector.tensor_add(hu_new[:, :], hm_cur[:, :], cand[:, :])
                nc.vector.tensor_add(hu_new[:, :], hu_new[:, :], bv[:, :])

            # transposes -> huT_{t+1}
            tP = p_tps.tile([128, 3 * 128], F16, tag="t")
            emit_transposes(hu_new, tP)
            hT_new = p_hT.tile([128, 3 * 128], F16, tag="hT")
            nc.vector.tensor_copy(hT_new[:, :], tP[:, :])

            # norm accumulators for rn_{t+1}
            squ = p_e16.tile([128, HH], F16, tag="squ")
            ss_n = p_sml.tile([128, 1], F32, tag="ss")
            nc.scalar.activation(squ[:, :], hu_new[:, :], ACTF.Square,
                                 accum_out=ss_n[:, :])
            ss16_n = p_sml.tile([128, 1], F16, tag="ss16")
            pool.tensor_copy(ss16_n[:, :], ss_n[:, :])

            # rn_{t+1} = rsqrt(fold(ss)): PE fold, DVE fast-inv-sqrt + 1 NR
            ssp = p_sps.tile([128, 1], F32, tag="ssp")
            nc.tensor.matmul(ssp[:, :], P64h[:, :], ss16_n[:, :],
                             start=True, stop=True)
            sdi = p_sml.tile([128, 1], I32, tag="sdi")
            nc.vector.tensor_scalar(sdi[:, :], ssp.bitcast(I32)[:, :], 1,
                                    None, ALU.logical_shift_right)
            nc.vector.tensor_scalar(sdi[:, :], sdi[:, :], -1, 0x5F3759DF,
                                    ALU.mult, ALU.add)
            rn_w = sdi.bitcast(F32)
            ra = p_sml.tile([128, 1], F32, tag="ra")
            rn_next = p_rn.tile([128, 1], F32, tag="rn")
            for it in range(newton_iters):
                nc.vector.tensor_mul(ra[:, :], rn_w[:, :], ssp[:, :])
                nc.vector.tensor_mul(ra[:, :], ra[:, :], rn_w[:, :])
                nc.vector.tensor_scalar(ra[:, :], ra[:, :], -0.5, 1.5,
                                        ALU.mult, ALU.add)
                dst = rn_next if it == newton_iters - 1 else rn_w
                nc.vector.tensor_mul(dst[:, :], rn_w[:, :], ra[:, :])
            hm_next = p_hm.tile([128, HH], F16, tag="hm")
            nc.scalar.mul(hm_next[:, :], hu_new[:, :], rn_next[:, :])

            # score for step t-1: rn_t * masked-reduce(q-gram cols)
            if t > 0:
                qsc = p_sml.tile([128, BL], F16, tag="qsc")
                qtmp = p_sml.tile([128, 1], F32, tag="qtmp")
                mask_reduce(qsc[:, :], zPA[:, GZ + 8:GA], maskG[:, :],
                            qtmp[:, :])
                pool.tensor_scalar(scores[:, t - 1:t], qtmp[:, :],
                                        rn_cur[:, :], None, ALU.mult)

            if debug:
                dma(bass.AP(d_dhu, t * 128 * HH, [[HH, 128], [1, HH]]),
                    hu_new[:, :])
                dma(bass.AP(d_drn, t * 128, [[1, 128], [1, 1]]), rn_cur[:, :])
                dma(bass.AP(d_dg, t * 128, [[1, 128], [1, 1]]), gsig[:, :])
                dma(bass.AP(d_dz, t * 128 * HH, [[HH, 128], [1, HH]]),
                    z16[:, :])

            hu_cur, hT_cur, b16_cur = hu_new, hT_new, b16_new

        # ---- epilogue ----
        lctx.close()
        p_eps = ep(tc.tile_pool(name="eps", bufs=1, space="PSUM"))
        rn_T = rn_next

        # final q-gram: q_{T-1} . hu_T
        qPf = p_eps.tile([128, BL], F32, tag="qf")
        for c in range(HC):
            nc.tensor.matmul(
                qPf[0:RP, 0:BL], hts(hT_cur, c),
                qT[:, c * NR + (nsteps - 1) * 8:c * NR + nsteps * 8],
                start=(c == 0), stop=(c == HC - 1))
        qsc = p_sml.tile([128, BL], F16, tag="qsc")
        qtmp = p_sml.tile([128, 1], F32, tag="qtmp")
        mask_reduce(qsc[0:RP, :], qPf[0:RP, 0:BL], maskG[0:RP, :],
                    qtmp[0:RP, :])
        pool.tensor_scalar(scores[0:RP, nsteps - 1:nsteps],
                                qtmp[0:RP, :], rn_T[0:RP, :], None, ALU.mult)

        # output head
        pP = p_eps.tile([128, nsteps], F32, tag="pp")
        nc.tensor.matmul(pP[0:BL * L, :], Wsel[0:R, 0:BL * L],
                         scores[0:R, 0:nsteps], start=True, stop=True)
        osb = p_prm.tile([128, nsteps], F32, tag="osb")
        nc.vector.tensor_scalar(osb[0:BL * L, :], pP[0:BL * L, :],
                                bvec[0:BL * L, :], None, ALU.add)
        nc.sync.dma_start(bass.AP(d_out, 0, [[1, BL * L], [BL * L, nsteps]]),
                          osb[0:BL * L, :])
        if debug:
            dma(bass.AP(d_dsc, 0, [[nsteps, 128], [1, nsteps]]),
                scores[:, :])

    nc.compile()
    return nc


_CACHE = {}


def _get(nsteps, debug=False, use_prelu=True, **kw):
    key = (nsteps, debug, use_prelu) + tuple(sorted(kw.items()))
    if key not in _CACHE:
        _CACHE[key] = _build(nsteps, debug=debug, use_prelu=use_prelu, **kw)
    return _CACHE[key]


def _in_maps(inputs, nsteps):
    consts = _host_consts()
    fs = np.ascontiguousarray(np.asarray(inputs["features_sentence"], dtype=np.float32))
    fe = np.ascontiguousarray(np.asarray(inputs["features_entity"], dtype=np.float32))
    shared = {k: np.ascontiguousarray(np.asarray(inputs[k], dtype=np.float32))
              for k in ("keys", "U", "V", "W", "alpha", "W_out", "b_out")}
    shared.update(consts)
    in_maps = []
    for c in range(NC):
        m = dict(shared)
        m["features_sentence"] = np.ascontiguousarray(fs[:, c * BL:(c + 1) * BL, :])
        m["features_entity"] = np.ascontiguousarray(fe[:, c * BL:(c + 1) * BL, :])
        in_maps.append(m)
    return in_maps


def kernel(**inputs):
    nsteps = inputs["features_sentence"].shape[0]
    nc = _get(nsteps, alpha_const=float(np.asarray(inputs["alpha"]).ravel()[0]))
    res = run_bass_kernel_spmd(nc, _in_maps(inputs, nsteps),
                               core_ids=list(range(NC)))
    outs = [r["preds"].reshape(nsteps, BL, L) for r in res.results]
    return np.concatenate(outs, axis=1).reshape(nsteps * B, L)
